# revision 38
# baseline (speedup 1.0000x reference)
"""Trainium2 Bass kernel for nn_LlamaAttention_61899068670751.

Sparse (streaming-LLM) attention layer, sharded tensor-parallel over heads
across 8 NeuronCores:
  - core c owns q-heads [4c..4c+3] and kv-head c (GQA group = 4)
  - QKV projections computed per-core with column-sharded weights
  - causal scores computed once per head; both softmax branches (full causal
    and sink+sliding-window) share exp(s) and are folded into a single PV
    matmul via per-row/per-region coefficients
  - the router MLP is computed redundantly on every core after a [128]
    feature AllReduce; its hard gate enters the coefficients
  - o is exchanged with an AllToAll so each core computes 256 rows of the
    final o @ Wo with the full head dimension; host concatenates row shards

All matmuls run as float32r (full-rate fp32 on the PE array).
"""
import numpy as np
from contextlib import ExitStack

import concourse.bacc as bacc
import concourse.mybir as mybir
import concourse.tile as tile
from concourse.bass_utils import run_bass_kernel_spmd

dt = mybir.dt
AF = mybir.ActivationFunctionType
ALU = mybir.AluOpType
AX = mybir.AxisListType

NCORES = 8
S, H, KV, D, HID = 2048, 32, 8, 128, 4096
SINK, WIN, POOL = 128, 1024, 100
HLOC = H // NCORES          # 4 q heads per core
NBLK = S // 128             # 16 row/col blocks
NCH = 4                     # s-chunks of 512
CH = 512
KT = HID // 128             # 32 contraction tiles
SCALE = 1.0 / float(np.sqrt(D))
NEG = -1.0e30
ROWS = S // NCORES          # 256 output rows per core


def _groups(I):
    """Right-aligned 4-block groups over causal blocks 0..I."""
    n = I + 1
    lo = n % 4
    g = [(0, lo)] if lo else []
    g += [(s, 4) for s in range(lo, n, 4)]
    return g


KNOBS = dict(phases=3, wo_bufs=24, w_bufs=8, hs_bufs=1, sc_bufs=3, pt_bufs=2,
             e_bufs=3, p_bufs=3, pT_bufs=2, o_bufs=1, tr_bufs=2)


def build():
    nc = bacc.Bacc("TRN2", target_bir_lowering=False, debug=False,
                   num_devices=NCORES)

    def din(name, shape, d=dt.float32r):
        return nc.dram_tensor(name, shape, d, kind="ExternalInput").ap()

    hs_d = din("hs", [S, HID])
    wqkv_d = din("wqkv", [HID, 768])
    wo_d = din("wo", [HID, HID])
    cos2_d = din("cos2", [128, S], dt.float32)
    sin2_d = din("sin2", [128, S], dt.float32)
    ident_d = din("ident", [128, 128])
    diagneg_d = din("diagneg", [128, 128], dt.float32)
    triup_d = din("triup", [128, 128], dt.float32)
    ones_d = din("ones", [1, 128], dt.float32)
    fe1_d = din("fe1", [128, 1024], dt.float32)
    fe2_d = din("fe2", [128, 8 * 256], dt.float32)
    r1_d = din("r1", [128, 2 * 512], dt.float32)
    r2_d = din("r2", [128, 4 * 128], dt.float32)
    r3_d = din("r3", [128, 1], dt.float32)
    b1_d = din("b1", [128, 8], dt.float32)
    b2_d = din("b2", [128, 2], dt.float32)
    rb1_d = din("rb1", [128, 4], dt.float32)
    rb2_d = din("rb2", [128, 1], dt.float32)
    rb3_d = din("rb3", [1, 1], dt.float32)
    noise_d = din("noise", [1, 1], dt.float32)
    eps_d = din("eps", [1, 1], dt.float32)

    out_d = nc.dram_tensor("out_rows", [ROWS, HID], dt.float32,
                           kind="ExternalOutput").ap()

    with tile.TileContext(nc) as tc, ExitStack() as top:
        # ---- long-lived pools -------------------------------------------
        const = top.enter_context(tc.tile_pool(name="const", bufs=1))
        persist = top.enter_context(tc.tile_pool(name="persist", bufs=1))
        dram = top.enter_context(tc.tile_pool(name="dram", bufs=1, space="DRAM"))

        ident = const.tile([128, 128], dt.float32r)
        diagneg = const.tile([128, 128], dt.float32)
        triup = const.tile([128, 128], dt.float32)
        ones_r = const.tile([1, 128], dt.float32)
        nc.sync.dma_start(ident[:], ident_d[:])
        nc.sync.dma_start(diagneg[:], diagneg_d[:])
        nc.sync.dma_start(triup[:], triup_d[:])
        nc.sync.dma_start(ones_r[:], ones_d[:])

        qT = [persist.tile([128, S], dt.float32r, name=f"qT{h}", tag=f"qT{h}")
              for h in range(HLOC)]
        kT = persist.tile([128, S], dt.float32r)
        vN = persist.tile([128, NBLK * 128], dt.float32r)   # v natural, per block

        # collective bounce buffers
        a2a_in0 = dram.tile([NCORES, 2 * 128, ROWS], dt.float32r)
        a2a_out0 = dram.tile([NCORES, 2 * 128, ROWS], dt.float32r)
        a2a_in1 = dram.tile([NCORES, 2 * 128, ROWS], dt.float32r)
        a2a_out1 = dram.tile([NCORES, 2 * 128, ROWS], dt.float32r)
        cc_in = dram.tile([128, 1], dt.float32)
        cc_out = dram.tile([128, 1], dt.float32, addr_space="Shared")

        # ---- phase 1: hs transpose + QKV projections + rope -------------
        with ExitStack() as ph1:
            p_hs = ph1.enter_context(tc.tile_pool(name="hs", bufs=KNOBS["hs_bufs"]))
            p_hsT = ph1.enter_context(tc.tile_pool(name="hsT", bufs=2))
            p_w = ph1.enter_context(tc.tile_pool(name="wslab", bufs=KNOBS["w_bufs"]))
            p_rope = ph1.enter_context(tc.tile_pool(name="rope", bufs=2))
            p_cs = ph1.enter_context(tc.tile_pool(name="cs", bufs=2))
            ps_tr = ph1.enter_context(
                tc.tile_pool(name="ps_tr", bufs=KNOBS["tr_bufs"], space="PSUM"))
            ps_acc = ph1.enter_context(
                tc.tile_pool(name="ps_acc", bufs=1, space="PSUM"))

            KH = KT // 2        # 16 k-tiles per half
            for g in range(NCH):
                s0 = g * CH
                accs = [ps_acc.tile([128, CH], dt.float32, tag=f"acc{i}",
                                    name=f"acc{i}")
                        for i in range(6)]
                for half in range(2):
                    k0 = half * KH
                    hsT = p_hsT.tile([128, KH * CH], dt.float32r, tag="hsT")
                    hsrows = []
                    for ss in range(4):
                        hsrow = p_hs.tile([128, KH * 128], dt.float32r,
                                          tag=f"hs{ss}", name=f"hs{ss}")
                        nc.sync.dma_start(
                            hsrow[:], hs_d[s0 + ss * 128: s0 + (ss + 1) * 128,
                                           k0 * 128:(k0 + KH) * 128])
                        hsrows.append(hsrow)
                    for kk in range(KH):
                        ptr = ps_tr.tile([128, 512], dt.float32r, tag="tr")
                        for ss in range(4):
                            nc.tensor.transpose(
                                ptr[:, ss * 128:(ss + 1) * 128],
                                hsrows[ss][:, kk * 128:(kk + 1) * 128],
                                ident[:])
                        nc.any.tensor_copy(
                            hsT[:, kk * CH:(kk + 1) * CH], ptr[:])
                    for kk in range(KH):
                        kt = k0 + kk
                        wsl = p_w.tile([128, 768], dt.float32r, tag="w")
                        nc.sync.dma_start(wsl[:],
                                          wqkv_d[kt * 128:(kt + 1) * 128, :])
                        for i in range(6):
                            nc.tensor.matmul(
                                accs[i][:], wsl[:, i * 128:(i + 1) * 128],
                                hsT[:, kk * CH:(kk + 1) * CH],
                                start=(kt == 0), stop=(kt == KT - 1))

                # rope for q heads (0..3) and k (4)
                cos_sl = p_cs.tile([128, CH], dt.float32, tag="cos")
                sin_sl = p_cs.tile([128, CH], dt.float32, tag="sin")
                nc.sync.dma_start(cos_sl[:], cos2_d[:, s0:s0 + CH])
                nc.sync.dma_start(sin_sl[:], sin2_d[:, s0:s0 + CH])
                for i in range(5):
                    dest = qT[i] if i < HLOC else kT
                    lin = p_rope.tile([128, CH], dt.float32, tag="lin")
                    rot = p_rope.tile([128, CH], dt.float32, tag="rot")
                    t1 = p_rope.tile([128, CH], dt.float32, tag="t1")
                    t2 = p_rope.tile([128, CH], dt.float32, tag="t2")
                    nc.scalar.copy(lin[:], accs[i][:])
                    nc.sync.dma_start(rot[0:64, :], lin[64:128, :])
                    nc.sync.dma_start(rot[64:128, :], lin[0:64, :])
                    nc.vector.tensor_tensor(t1[:], lin[:], cos_sl[:], ALU.mult)
                    nc.vector.tensor_tensor(t2[:], rot[:], sin_sl[:], ALU.mult)
                    nc.vector.tensor_tensor(dest[:, s0:s0 + CH], t1[:], t2[:],
                                            ALU.add)
                # v: copy then transpose to natural layout
                vT = p_rope.tile([128, CH], dt.float32r, tag="vT")
                nc.scalar.copy(vT[:], accs[5][:])
                for ss in range(4):
                    ptr = ps_tr.tile([128, 128], dt.float32r, tag="tr")
                    nc.tensor.transpose(ptr[:], vT[:, ss * 128:(ss + 1) * 128],
                                        ident[:])
                    nc.any.tensor_copy(
                        vN[:, (g * 4 + ss) * 128:(g * 4 + ss + 1) * 128], ptr[:])

        # ---- phase 2: router + attention --------------------------------
        with ExitStack() as ph2:
          if KNOBS["phases"] >= 2:
              p_mlp = ph2.enter_context(tc.tile_pool(name="mlp", bufs=1))
              p_e = ph2.enter_context(tc.tile_pool(name="eband", bufs=KNOBS["e_bufs"]))
              p_p = ph2.enter_context(tc.tile_pool(name="pband", bufs=KNOBS["p_bufs"]))
              p_pT = ph2.enter_context(tc.tile_pool(name="pT", bufs=KNOBS["pT_bufs"]))
              p_sm = ph2.enter_context(tc.tile_pool(name="sums", bufs=4))
              p_ob = ph2.enter_context(tc.tile_pool(name="obuf", bufs=2))
              ps_sc = ph2.enter_context(
                  tc.tile_pool(name="ps_sc", bufs=KNOBS["sc_bufs"], space="PSUM"))
              ps_pt = ph2.enter_context(
                  tc.tile_pool(name="ps_pt", bufs=KNOBS["pt_bufs"], space="PSUM"))
              ps_o = ph2.enter_context(
                  tc.tile_pool(name="ps_o", bufs=KNOBS["o_bufs"], space="PSUM"))
              mlp_ctx = ExitStack()
              ps_m = mlp_ctx.enter_context(
                  tc.tile_pool(name="ps_m", bufs=1, space="PSUM"))

              # --- router ---
              feat_acc = p_mlp.tile([128, 8], dt.float32)
              for h in range(HLOC):
                  nc.vector.tensor_reduce(feat_acc[:, h:h + 1],
                                          qT[h][:, 0:POOL], AX.X, ALU.add)
                  nc.vector.tensor_reduce(feat_acc[:, 4 + h:5 + h],
                                          qT[h][:, S - POOL:S], AX.X, ALU.add)
              feat_s = p_mlp.tile([128, 1], dt.float32)
              nc.vector.tensor_reduce(feat_s[:], feat_acc[:], AX.X, ALU.add)
              feat_r = p_mlp.tile([128, 1], dt.float32)
              nc.scalar.activation(feat_r[:], feat_s[:], AF.Copy,
                                   scale=1.0 / (2 * POOL * H))
              nc.sync.dma_start(cc_in[:], feat_r[:])
              nc.gpsimd.collective_compute(
                  "AllReduce", ALU.add,
                  replica_groups=[list(range(NCORES))],
                  ins=[cc_in.opt()], outs=[cc_out.opt()])
              featg = p_mlp.tile([128, 1], dt.float32)
              nc.sync.dma_start(featg[:], cc_out[:])

              # MLP weights
              fe1 = p_mlp.tile([128, 1024], dt.float32)
              fe2 = p_mlp.tile([128, 8 * 256], dt.float32)
              r1w = p_mlp.tile([128, 2 * 512], dt.float32)
              r2w = p_mlp.tile([128, 4 * 128], dt.float32)
              r3w = p_mlp.tile([128, 1], dt.float32)
              b1 = p_mlp.tile([128, 8], dt.float32)
              b2 = p_mlp.tile([128, 2], dt.float32)
              rb1 = p_mlp.tile([128, 4], dt.float32)
              rb2 = p_mlp.tile([128, 1], dt.float32)
              rb3 = p_mlp.tile([1, 1], dt.float32)
              noise = p_mlp.tile([1, 1], dt.float32)
              epsb = p_mlp.tile([1, 1], dt.float32)
              nc.sync.dma_start(epsb[:], eps_d[:])
              for t_, d_ in ((fe1, fe1_d), (fe2, fe2_d), (r1w, r1_d),
                             (r2w, r2_d), (r3w, r3_d), (b1, b1_d), (b2, b2_d),
                             (rb1, rb1_d), (rb2, rb2_d), (rb3, rb3_d),
                             (noise, noise_d)):
                  nc.sync.dma_start(t_[:], d_[:])

              def mlp_layer(vec_in, w_sb, ktiles, ntiles, bias, act, nwidth=128):
                  """vec_in: [128, ktiles] fp32r columns; returns [128, ntiles]."""
                  out_r = p_mlp.tile([128, max(ntiles, 1)], dt.float32,
                                     name=f"mlpv{len(mlp_tmp)}")
                  mlp_tmp.append(out_r)
                  ps = ps_m.tile([128, max(ntiles, 1)], dt.float32, tag="mlp",
                               name="mlpps")
                  for t in range(ntiles):
                      for k in range(ktiles):
                          nc.tensor.matmul(
                              ps[:, t:t + 1],
                              w_sb[:, (k * ntiles + t) * nwidth:
                                   (k * ntiles + t) * nwidth + nwidth],
                              vec_in[:, k:k + 1],
                              start=(k == 0), stop=(k == ktiles - 1))
                  for t in range(ntiles):
                      nc.scalar.activation(out_r[:, t:t + 1], ps[:, t:t + 1],
                                           act, bias=bias[:, t:t + 1])
                  return out_r

              mlp_tmp = []
              h1 = mlp_layer(featg, fe1, 1, 8, b1, AF.Silu)
              h2 = mlp_layer(h1, fe2, 8, 2, b2, AF.Identity)
              h3 = mlp_layer(h2, r1w, 2, 4, rb1, AF.Silu)
              h4 = mlp_layer(h3, r2w, 4, 1, rb2, AF.Silu)
              lps = ps_m.tile([1, 1], dt.float32, tag="mlp")
              nc.tensor.matmul(lps[:], r3w[:], h4[:], start=True, stop=True)
              logits = p_mlp.tile([1, 1], dt.float32)
              nc.scalar.activation(logits[:], lps[:], AF.Identity, bias=rb3[:])
              l1 = p_mlp.tile([1, 1], dt.float32)
              l2 = p_mlp.tile([1, 1], dt.float32)
              nc.scalar.activation(l1[:], noise[:], AF.Ln, bias=epsb[:])
              nc.scalar.activation(l2[:], l1[:], AF.Ln, bias=epsb[:], scale=-1.0)
              zin = p_mlp.tile([1, 1], dt.float32)
              nc.vector.tensor_tensor(zin[:], logits[:], l2[:], ALU.subtract)
              zsoft = p_mlp.tile([1, 1], dt.float32)
              nc.scalar.activation(zsoft[:], zin[:], AF.Sigmoid)
              zhard = p_mlp.tile([1, 1], dt.float32)
              nc.vector.tensor_scalar(zhard[:], zsoft[:], 0.5, None, ALU.is_gt)
              mps = ps_m.tile([128, 1], dt.float32, tag="mlp")
              nc.tensor.matmul(mps[:], ones_r[:], zhard[:], start=True, stop=True)
              mix = p_mlp.tile([128, 1], dt.float32)
              nc.scalar.copy(mix[:], mps[:])
              onem = p_mlp.tile([128, 1], dt.float32)
              nc.vector.tensor_scalar(onem[:], mix[:], -1.0, 1.0, ALU.mult,
                                      ALU.add)
              mlp_ctx.close()

              # --- attention ---
              for h in range(HLOC):
                  for g in range(NCH):
                      pT = p_pT.tile([128, NBLK * CH], dt.float32r, tag="pT")
                      for Ii in range(4):
                          I = 4 * g + Ii
                          nb_tot = (I + 1) * 128
                          e = p_e.tile([128, S], dt.float32, tag="e")
                          pband = p_p.tile([128, S], dt.float32r, tag="p")
                          sums = p_sm.tile([128, 16], dt.float32, tag="sums")
                          grps = _groups(I)
                          ng = len(grps)
                          for gi, (sb, nb) in enumerate(grps):
                              w = nb * 128
                              col = 4 - ng + gi
                              sc = ps_sc.tile([128, 512], dt.float32, tag="sc")
                              nc.tensor.matmul(
                                  sc[:, 0:w], qT[h][:, I * 128:(I + 1) * 128],
                                  kT[:, sb * 128: sb * 128 + w],
                                  start=True, stop=True)
                              if gi == ng - 1:
                                  nc.vector.tensor_tensor(
                                      sc[:, w - 128:w], sc[:, w - 128:w],
                                      diagneg[:], ALU.add)
                              nc.scalar.activation(
                                  e[:, sb * 128: sb * 128 + w], sc[:, 0:w],
                                  AF.Exp, scale=SCALE,
                                  accum_out=sums[:, col:col + 1])
                          if I >= 9:
                              tmask = p_sm.tile([128, 128], dt.float32,
                                                tag="tmask")
                              nc.vector.tensor_reduce(
                                  sums[:, 4:5], e[:, 0:128], AX.X, ALU.add)
                              nc.vector.tensor_tensor(
                                  tmask[:], e[:, (I - 8) * 128:(I - 7) * 128],
                                  triup[:], ALU.mult)
                              nc.vector.tensor_reduce(
                                  sums[:, 5:6], tmask[:], AX.X, ALU.add)
                              nc.vector.tensor_reduce(
                                  sums[:, 6:7], sums[:, 4 - ng:4], AX.X, ALU.add)
                              nc.vector.tensor_reduce(
                                  sums[:, 7:8], sums[:, 2:6], AX.X, ALU.add)
                              nc.vector.reciprocal(sums[:, 8:9], sums[:, 6:7])
                              nc.vector.reciprocal(sums[:, 9:10], sums[:, 7:8])
                              nc.vector.tensor_tensor(
                                  sums[:, 10:11], sums[:, 8:9], onem[:], ALU.mult)
                              nc.vector.tensor_tensor(
                                  sums[:, 11:12], sums[:, 9:10], mix[:], ALU.mult)
                              nc.vector.tensor_tensor(
                                  sums[:, 12:13], sums[:, 10:11], sums[:, 11:12],
                                  ALU.add)
                              a_ap = sums[:, 12:13]
                              b_ap = sums[:, 10:11]
                              amb_ap = sums[:, 11:12]
                              nc.vector.tensor_scalar(
                                  pband[:, 0:128], e[:, 0:128], a_ap, None,
                                  ALU.mult)
                              if I >= 10:
                                  nc.vector.tensor_scalar(
                                      pband[:, 128:(I - 8) * 128],
                                      e[:, 128:(I - 8) * 128], b_ap, None,
                                      ALU.mult)
                              nc.vector.tensor_scalar(
                                  pband[:, (I - 8) * 128:(I - 7) * 128],
                                  e[:, (I - 8) * 128:(I - 7) * 128], b_ap, None,
                                  ALU.mult)
                              nc.vector.scalar_tensor_tensor(
                                  pband[:, (I - 8) * 128:(I - 7) * 128],
                                  tmask[:], amb_ap,
                                  pband[:, (I - 8) * 128:(I - 7) * 128],
                                  ALU.mult, ALU.add)
                              nc.vector.tensor_scalar(
                                  pband[:, (I - 7) * 128:nb_tot],
                                  e[:, (I - 7) * 128:nb_tot], a_ap, None,
                                  ALU.mult)
                          else:
                              nc.vector.tensor_reduce(
                                  sums[:, 6:7], sums[:, 4 - ng:4], AX.X, ALU.add)
                              nc.vector.reciprocal(sums[:, 8:9], sums[:, 6:7])
                              nc.vector.tensor_scalar(
                                  pband[:, 0:nb_tot], e[:, 0:nb_tot],
                                  sums[:, 8:9], None, ALU.mult)
                          pT3 = pT[:, :].rearrange("p (J c) -> p J c", c=CH)
                          for J0 in range(0, I + 1, 4):
                              nb4 = min(4, I + 1 - J0)
                              ptp = ps_pt.tile([128, 512], dt.float32r, tag="pt")
                              for jj in range(nb4):
                                  nc.tensor.transpose(
                                      ptp[:, jj * 128:(jj + 1) * 128],
                                      pband[:, (J0 + jj) * 128:
                                            (J0 + jj + 1) * 128],
                                      ident[:])
                              nc.any.tensor_copy(
                                  pT3[:, J0:J0 + nb4,
                                      Ii * 128:(Ii + 1) * 128],
                                  ptp[:, 0:nb4 * 128].rearrange(
                                      "p (J c) -> p J c", c=128))
                      # PV for this (h, chunk)
                      ops = ps_o.tile([128, CH], dt.float32, tag="o")
                      last_J = 4 * g + 3
                      for J in range(last_J + 1):
                          k = J - 4 * g
                          if k <= 0:
                              nc.tensor.matmul(
                                  ops[:], vN[:, J * 128:(J + 1) * 128],
                                  pT[:, J * CH:(J + 1) * CH],
                                  start=(J == 0), stop=(J == last_J))
                          else:
                              nc.tensor.matmul(
                                  ops[:, k * 128:CH],
                                  vN[:, J * 128:(J + 1) * 128],
                                  pT[:, J * CH + k * 128:(J + 1) * CH],
                                  start=False, stop=(J == last_J))
                      osb = p_ob.tile([128, CH], dt.float32r, tag="osb")
                      nc.scalar.copy(osb[:], ops[:])
                      a2a_in_h = a2a_in0 if h < 2 else a2a_in1
                      hh = h % 2
                      nc.sync.dma_start(
                          a2a_in_h[2 * g, hh * 128:(hh + 1) * 128, :],
                          osb[:, 0:ROWS])
                      nc.sync.dma_start(
                          a2a_in_h[2 * g + 1, hh * 128:(hh + 1) * 128, :],
                          osb[:, ROWS:CH])
                  if h == 1:
                      nc.gpsimd.collective_compute(
                          "AllToAll", ALU.bypass,
                          replica_groups=[list(range(NCORES))],
                          ins=[a2a_in0.opt()], outs=[a2a_out0.opt()])
                  if h == 3:
                      nc.gpsimd.collective_compute(
                          "AllToAll", ALU.bypass,
                          replica_groups=[list(range(NCORES))],
                          ins=[a2a_in1.opt()], outs=[a2a_out1.opt()])

        # ---- phase 3: AllToAll + output projection ----------------------
        with ExitStack() as ph3:
          if KNOBS["phases"] >= 3:
              p_oT = ph3.enter_context(tc.tile_pool(name="oT", bufs=1))
              p_wo = ph3.enter_context(tc.tile_pool(name="wo", bufs=KNOBS["wo_bufs"]))
              p_os = ph3.enter_context(tc.tile_pool(name="outsb", bufs=2))
              ps_w = ph3.enter_context(
                  tc.tile_pool(name="ps_w", bufs=2, space="PSUM"))

              KT_ORDER = [4 * p + t for t in (0, 1, 2, 3) for p in range(NCORES)]
              oT = p_oT.tile([128, KT * ROWS], dt.float32r)
              for kt in KT_ORDER:
                  p, t = kt // HLOC, kt % HLOC
                  src = a2a_out0 if t < 2 else a2a_out1
                  nc.sync.dma_start(
                      oT[:, kt * ROWS:(kt + 1) * ROWS],
                      src[p, (t % 2) * 128:(t % 2 + 1) * 128, :])

              for ngi in range(8):
                  pso = [ps_w.tile([128, 512], dt.float32, tag=f"wo{st}",
                                  name=f"wo{st}")
                         for st in range(2)]
                  for ki, kt in enumerate(KT_ORDER):
                      wsl = p_wo.tile([128, 512], dt.float32r, tag="wo")
                      nc.sync.dma_start(
                          wsl[:], wo_d[kt * 128:(kt + 1) * 128,
                                       ngi * 512:(ngi + 1) * 512])
                      for st in range(2):
                          nc.tensor.matmul(
                              pso[st][:],
                              oT[:, kt * ROWS + st * 128: kt * ROWS + (st + 1) * 128],
                              wsl[:], start=(ki == 0), stop=(ki == KT - 1))
                  for st in range(2):
                      osb = p_os.tile([128, 512], dt.float32, tag="os")
                      nc.scalar.copy(osb[:], pso[st][:])
                      nc.sync.dma_start(
                          out_d[st * 128:(st + 1) * 128,
                                ngi * 512:(ngi + 1) * 512], osb[:])

    nc.compile()
    return nc


_CACHE = {}


def _host_constants():
    inv = 10000.0 ** (-np.arange(0, D, 2, dtype=np.float64) / D)
    t = np.arange(S, dtype=np.float64)
    fr = np.outer(t, inv)                      # [S, 64]
    cos = np.cos(fr).T.astype(np.float32)      # [64, S]
    sin = np.sin(fr).T.astype(np.float32)
    cos2 = np.vstack([cos, cos])
    sin2 = np.vstack([-sin, sin])
    ident = np.eye(128, dtype=np.float32)
    a = np.arange(128)
    diagneg = np.where(a[None, :] <= a[:, None], 0.0, NEG).astype(np.float32)
    triup = (a[None, :] > a[:, None]).astype(np.float32)
    ones = np.ones((1, 128), dtype=np.float32)
    return cos2, sin2, ident, diagneg, triup, ones


def kernel(hidden_states, Wq, Wk, Wv, Wo, fe1_w, fe1_b, fe2_w, fe2_b,
           r1_w, r1_b, r2_w, r2_b, r3_w, r3_b, router_noise):
    if "nc" not in _CACHE:
        _CACHE["nc"] = build()
    nc = _CACHE["nc"]

    hs = np.ascontiguousarray(
        np.asarray(hidden_states, dtype=np.float32).reshape(S, HID))
    Wq = np.asarray(Wq, np.float32)
    Wk = np.asarray(Wk, np.float32)
    Wv = np.asarray(Wv, np.float32)
    Wo = np.ascontiguousarray(np.asarray(Wo, np.float32))
    cos2, sin2, ident, diagneg, triup, ones = _host_constants()

    def ktile_cols(w, ktiles, ntiles, nwidth):
        # [K, N] -> [128, ktiles*ntiles*nwidth] with (k, t) slab layout
        return np.ascontiguousarray(
              np.concatenate([w[k * 128:(k + 1) * 128, :] for k in range(ktiles)],
                             axis=1))

    fe1 = np.asarray(fe1_w, np.float32)                       # [128,1024]
    fe2 = ktile_cols(np.asarray(fe2_w, np.float32), 8, 2, 128)
    r1 = ktile_cols(np.asarray(r1_w, np.float32), 2, 4, 128)
    r2 = ktile_cols(np.asarray(r2_w, np.float32), 4, 1, 128)
    r3 = np.asarray(r3_w, np.float32)                         # [128,1]
    b1 = np.asarray(fe1_b, np.float32).reshape(8, 128).T.copy()
    b2 = np.asarray(fe2_b, np.float32).reshape(2, 128).T.copy()
    rb1 = np.asarray(r1_b, np.float32).reshape(4, 128).T.copy()
    rb2 = np.asarray(r2_b, np.float32).reshape(1, 128).T.copy()
    rb3 = np.asarray(r3_b, np.float32).reshape(1, 1)
    noise = np.asarray(router_noise, np.float32).reshape(1, 1)

    in_maps = []
    for c in range(NCORES):
        wqkv = np.ascontiguousarray(np.concatenate(
              [Wq[:, c * 512:(c + 1) * 512],
               Wk[:, c * 128:(c + 1) * 128],
               Wv[:, c * 128:(c + 1) * 128]], axis=1))
        in_maps.append(dict(
              hs=hs, wqkv=wqkv, wo=Wo, cos2=cos2, sin2=sin2, ident=ident,
              diagneg=diagneg, triup=triup, ones=ones, fe1=fe1, fe2=fe2,
              r1=r1, r2=r2, r3=r3, b1=b1, b2=b2, rb1=rb1, rb2=rb2, rb3=rb3,
              noise=noise, eps=np.full((1, 1), 1e-8, np.float32)))

    res = run_bass_kernel_spmd(nc, in_maps, list(range(NCORES)))
    out = np.concatenate([res.results[c]["out_rows"] for c in range(NCORES)],
                           axis=0)
    return out.reshape(1, S, HID).astype(np.float32)



# revision 39
# speedup vs baseline: 1.0818x; 1.0818x over previous
"""Trainium2 Bass kernel for nn_LlamaAttention_61899068670751.

Sparse (streaming-LLM) attention layer, sharded tensor-parallel over heads
across 8 NeuronCores:
  - core c owns q-heads [4c..4c+3] and kv-head c (GQA group = 4)
  - QKV projections computed per-core with column-sharded weights
  - causal scores computed once per head; both softmax branches (full causal
    and sink+sliding-window) share exp(s) and are folded into a single PV
    matmul via per-row/per-region coefficients
  - the router MLP is computed redundantly on every core after a [128]
    feature AllReduce; its hard gate enters the coefficients
  - o is exchanged with an AllToAll so each core computes 256 rows of the
    final o @ Wo with the full head dimension; host concatenates row shards

All matmuls run as float32r (full-rate fp32 on the PE array).
"""
import numpy as np
from contextlib import ExitStack

import concourse.bacc as bacc
import concourse.mybir as mybir
import concourse.tile as tile
from concourse.bass_utils import run_bass_kernel_spmd

dt = mybir.dt
AF = mybir.ActivationFunctionType
ALU = mybir.AluOpType
AX = mybir.AxisListType

NCORES = 8
S, H, KV, D, HID = 2048, 32, 8, 128, 4096
SINK, WIN, POOL = 128, 1024, 100
HLOC = H // NCORES          # 4 q heads per core
NBLK = S // 128             # 16 row/col blocks
NCH = 4                     # s-chunks of 512
CH = 512
KT = HID // 128             # 32 contraction tiles
SCALE = 1.0 / float(np.sqrt(D))
NEG = -1.0e30
ROWS = S // NCORES          # 256 output rows per core


def _groups(I):
    """Right-aligned 4-block groups over causal blocks 0..I."""
    n = I + 1
    lo = n % 4
    g = [(0, lo)] if lo else []
    g += [(s, 4) for s in range(lo, n, 4)]
    return g


KNOBS = dict(phases=3, wo_bufs=24, w_bufs=8, hs_bufs=1, sc_bufs=3, pt_bufs=2,
             e_bufs=3, p_bufs=3, pT_bufs=2, o_bufs=1, tr_bufs=2)


def build():
    nc = bacc.Bacc("TRN2", target_bir_lowering=False, debug=False,
                   num_devices=NCORES)

    def din(name, shape, d=dt.float32r):
        return nc.dram_tensor(name, shape, d, kind="ExternalInput").ap()

    hsT_d = din("hsT", [HID, S], dt.bfloat16)
    wqkv_d = din("wqkv", [HID, 768], dt.bfloat16)
    wo_d = din("wo", [HID, HID], dt.bfloat16)
    cos2_d = din("cos2", [128, S], dt.float32)
    sin2_d = din("sin2", [128, S], dt.float32)
    ident_d = din("ident", [128, 128])
    diagneg_d = din("diagneg", [128, 128], dt.float32)
    triup_d = din("triup", [128, 128], dt.float32)
    ones_d = din("ones", [1, 128], dt.float32)
    fe1_d = din("fe1", [128, 1024], dt.float32)
    fe2_d = din("fe2", [128, 8 * 256], dt.float32)
    r1_d = din("r1", [128, 2 * 512], dt.float32)
    r2_d = din("r2", [128, 4 * 128], dt.float32)
    r3_d = din("r3", [128, 1], dt.float32)
    b1_d = din("b1", [128, 8], dt.float32)
    b2_d = din("b2", [128, 2], dt.float32)
    rb1_d = din("rb1", [128, 4], dt.float32)
    rb2_d = din("rb2", [128, 1], dt.float32)
    rb3_d = din("rb3", [1, 1], dt.float32)
    noise_d = din("noise", [1, 1], dt.float32)
    eps_d = din("eps", [1, 1], dt.float32)

    out_d = nc.dram_tensor("out_rows", [ROWS, HID], dt.float32,
                           kind="ExternalOutput").ap()

    with tile.TileContext(nc) as tc, ExitStack() as top:
        # ---- long-lived pools -------------------------------------------
        const = top.enter_context(tc.tile_pool(name="const", bufs=1))
        persist = top.enter_context(tc.tile_pool(name="persist", bufs=1))
        dram = top.enter_context(tc.tile_pool(name="dram", bufs=1, space="DRAM"))

        ident = const.tile([128, 128], dt.float32r)
        diagneg = const.tile([128, 128], dt.float32)
        triup = const.tile([128, 128], dt.float32)
        ones_r = const.tile([1, 128], dt.float32)
        nc.sync.dma_start(ident[:], ident_d[:])
        nc.sync.dma_start(diagneg[:], diagneg_d[:])
        nc.sync.dma_start(triup[:], triup_d[:])
        nc.sync.dma_start(ones_r[:], ones_d[:])

        qT = [persist.tile([128, S], dt.float32r, name=f"qT{h}", tag=f"qT{h}")
              for h in range(HLOC)]
        kT = persist.tile([128, S], dt.float32r)
        vN = persist.tile([128, NBLK * 128], dt.float32r)   # v natural, per block

        # collective bounce buffers
        a2a_in0 = dram.tile([NCORES, 2 * 128, ROWS], dt.float32r)
        a2a_out0 = dram.tile([NCORES, 2 * 128, ROWS], dt.float32r)
        a2a_in1 = dram.tile([NCORES, 2 * 128, ROWS], dt.float32r)
        a2a_out1 = dram.tile([NCORES, 2 * 128, ROWS], dt.float32r)
        cc_in = dram.tile([128, 1], dt.float32)
        cc_out = dram.tile([128, 1], dt.float32, addr_space="Shared")

        # ---- phase 1: hs transpose + QKV projections + rope -------------
        with ExitStack() as ph1:
            p_hs = ph1.enter_context(tc.tile_pool(name="hs", bufs=KNOBS["hs_bufs"]))
            p_hsT = ph1.enter_context(tc.tile_pool(name="hsT", bufs=2))
            p_w = ph1.enter_context(tc.tile_pool(name="wslab", bufs=KNOBS["w_bufs"]))
            p_rope = ph1.enter_context(tc.tile_pool(name="rope", bufs=2))
            p_cs = ph1.enter_context(tc.tile_pool(name="cs", bufs=2))
            ps_tr = ph1.enter_context(
                tc.tile_pool(name="ps_tr", bufs=KNOBS["tr_bufs"], space="PSUM"))
            ps_acc = ph1.enter_context(
                tc.tile_pool(name="ps_acc", bufs=1, space="PSUM"))

            KH = KT // 2        # 16 k-tiles per half
            for g in range(NCH):
                s0 = g * CH
                accs = [ps_acc.tile([128, CH], dt.float32, tag=f"acc{i}",
                                    name=f"acc{i}")
                        for i in range(6)]
                for half in range(2):
                    k0 = half * KH
                    hsT = p_hsT.tile([128, KH, CH], dt.bfloat16, tag="hsT")
                    nc.sync.dma_start(
                        hsT[:], hsT_d[k0 * 128:(k0 + KH) * 128,
                                      s0:s0 + CH].rearrange(
                            "(k p) n -> p k n", p=128))
                    for kk in range(KH):
                        kt = k0 + kk
                        wsl = p_w.tile([128, 768], dt.bfloat16, tag="w")
                        nc.sync.dma_start(wsl[:],
                                          wqkv_d[kt * 128:(kt + 1) * 128, :])
                        for i in range(6):
                            nc.tensor.matmul(
                                accs[i][:], wsl[:, i * 128:(i + 1) * 128],
                                hsT[:, kk, :],
                                start=(kt == 0), stop=(kt == KT - 1))

                # rope for q heads (0..3) and k (4)
                cos_sl = p_cs.tile([128, CH], dt.float32, tag="cos")
                sin_sl = p_cs.tile([128, CH], dt.float32, tag="sin")
                nc.sync.dma_start(cos_sl[:], cos2_d[:, s0:s0 + CH])
                nc.sync.dma_start(sin_sl[:], sin2_d[:, s0:s0 + CH])
                for i in range(5):
                    dest = qT[i] if i < HLOC else kT
                    lin = p_rope.tile([128, CH], dt.float32, tag="lin")
                    rot = p_rope.tile([128, CH], dt.float32, tag="rot")
                    t1 = p_rope.tile([128, CH], dt.float32, tag="t1")
                    t2 = p_rope.tile([128, CH], dt.float32, tag="t2")
                    nc.scalar.copy(lin[:], accs[i][:])
                    nc.sync.dma_start(rot[0:64, :], lin[64:128, :])
                    nc.sync.dma_start(rot[64:128, :], lin[0:64, :])
                    nc.vector.tensor_tensor(t1[:], lin[:], cos_sl[:], ALU.mult)
                    nc.vector.tensor_tensor(t2[:], rot[:], sin_sl[:], ALU.mult)
                    nc.vector.tensor_tensor(dest[:, s0:s0 + CH], t1[:], t2[:],
                                            ALU.add)
                # v: copy then transpose to natural layout
                vT = p_rope.tile([128, CH], dt.float32r, tag="vT")
                nc.scalar.copy(vT[:], accs[5][:])
                for ss in range(4):
                    ptr = ps_tr.tile([128, 128], dt.float32r, tag="tr")
                    nc.tensor.transpose(ptr[:], vT[:, ss * 128:(ss + 1) * 128],
                                        ident[:])
                    nc.any.tensor_copy(
                        vN[:, (g * 4 + ss) * 128:(g * 4 + ss + 1) * 128], ptr[:])

        # ---- phase 2: router + attention --------------------------------
        with ExitStack() as ph2:
          if KNOBS["phases"] >= 2:
              p_mlp = ph2.enter_context(tc.tile_pool(name="mlp", bufs=1))
              p_e = ph2.enter_context(tc.tile_pool(name="eband", bufs=KNOBS["e_bufs"]))
              p_p = ph2.enter_context(tc.tile_pool(name="pband", bufs=KNOBS["p_bufs"]))
              p_pT = ph2.enter_context(tc.tile_pool(name="pT", bufs=KNOBS["pT_bufs"]))
              p_sm = ph2.enter_context(tc.tile_pool(name="sums", bufs=4))
              p_ob = ph2.enter_context(tc.tile_pool(name="obuf", bufs=2))
              ps_sc = ph2.enter_context(
                  tc.tile_pool(name="ps_sc", bufs=KNOBS["sc_bufs"], space="PSUM"))
              ps_pt = ph2.enter_context(
                  tc.tile_pool(name="ps_pt", bufs=KNOBS["pt_bufs"], space="PSUM"))
              ps_o = ph2.enter_context(
                  tc.tile_pool(name="ps_o", bufs=KNOBS["o_bufs"], space="PSUM"))
              mlp_ctx = ExitStack()
              ps_m = mlp_ctx.enter_context(
                  tc.tile_pool(name="ps_m", bufs=1, space="PSUM"))

              # --- router ---
              feat_acc = p_mlp.tile([128, 8], dt.float32)
              for h in range(HLOC):
                  nc.vector.tensor_reduce(feat_acc[:, h:h + 1],
                                          qT[h][:, 0:POOL], AX.X, ALU.add)
                  nc.vector.tensor_reduce(feat_acc[:, 4 + h:5 + h],
                                          qT[h][:, S - POOL:S], AX.X, ALU.add)
              feat_s = p_mlp.tile([128, 1], dt.float32)
              nc.vector.tensor_reduce(feat_s[:], feat_acc[:], AX.X, ALU.add)
              feat_r = p_mlp.tile([128, 1], dt.float32)
              nc.scalar.activation(feat_r[:], feat_s[:], AF.Copy,
                                   scale=1.0 / (2 * POOL * H))
              nc.sync.dma_start(cc_in[:], feat_r[:])
              nc.gpsimd.collective_compute(
                  "AllReduce", ALU.add,
                  replica_groups=[list(range(NCORES))],
                  ins=[cc_in.opt()], outs=[cc_out.opt()])
              featg = p_mlp.tile([128, 1], dt.float32)
              nc.sync.dma_start(featg[:], cc_out[:])

              # MLP weights
              fe1 = p_mlp.tile([128, 1024], dt.float32)
              fe2 = p_mlp.tile([128, 8 * 256], dt.float32)
              r1w = p_mlp.tile([128, 2 * 512], dt.float32)
              r2w = p_mlp.tile([128, 4 * 128], dt.float32)
              r3w = p_mlp.tile([128, 1], dt.float32)
              b1 = p_mlp.tile([128, 8], dt.float32)
              b2 = p_mlp.tile([128, 2], dt.float32)
              rb1 = p_mlp.tile([128, 4], dt.float32)
              rb2 = p_mlp.tile([128, 1], dt.float32)
              rb3 = p_mlp.tile([1, 1], dt.float32)
              noise = p_mlp.tile([1, 1], dt.float32)
              epsb = p_mlp.tile([1, 1], dt.float32)
              nc.sync.dma_start(epsb[:], eps_d[:])
              for t_, d_ in ((fe1, fe1_d), (fe2, fe2_d), (r1w, r1_d),
                             (r2w, r2_d), (r3w, r3_d), (b1, b1_d), (b2, b2_d),
                             (rb1, rb1_d), (rb2, rb2_d), (rb3, rb3_d),
                             (noise, noise_d)):
                  nc.sync.dma_start(t_[:], d_[:])

              def mlp_layer(vec_in, w_sb, ktiles, ntiles, bias, act, nwidth=128):
                  """vec_in: [128, ktiles] fp32r columns; returns [128, ntiles]."""
                  out_r = p_mlp.tile([128, max(ntiles, 1)], dt.float32,
                                     name=f"mlpv{len(mlp_tmp)}")
                  mlp_tmp.append(out_r)
                  ps = ps_m.tile([128, max(ntiles, 1)], dt.float32, tag="mlp",
                               name="mlpps")
                  for t in range(ntiles):
                      for k in range(ktiles):
                          nc.tensor.matmul(
                              ps[:, t:t + 1],
                              w_sb[:, (k * ntiles + t) * nwidth:
                                   (k * ntiles + t) * nwidth + nwidth],
                              vec_in[:, k:k + 1],
                              start=(k == 0), stop=(k == ktiles - 1))
                  for t in range(ntiles):
                      nc.scalar.activation(out_r[:, t:t + 1], ps[:, t:t + 1],
                                           act, bias=bias[:, t:t + 1])
                  return out_r

              mlp_tmp = []
              h1 = mlp_layer(featg, fe1, 1, 8, b1, AF.Silu)
              h2 = mlp_layer(h1, fe2, 8, 2, b2, AF.Identity)
              h3 = mlp_layer(h2, r1w, 2, 4, rb1, AF.Silu)
              h4 = mlp_layer(h3, r2w, 4, 1, rb2, AF.Silu)
              lps = ps_m.tile([1, 1], dt.float32, tag="mlp")
              nc.tensor.matmul(lps[:], r3w[:], h4[:], start=True, stop=True)
              logits = p_mlp.tile([1, 1], dt.float32)
              nc.scalar.activation(logits[:], lps[:], AF.Identity, bias=rb3[:])
              l1 = p_mlp.tile([1, 1], dt.float32)
              l2 = p_mlp.tile([1, 1], dt.float32)
              nc.scalar.activation(l1[:], noise[:], AF.Ln, bias=epsb[:])
              nc.scalar.activation(l2[:], l1[:], AF.Ln, bias=epsb[:], scale=-1.0)
              zin = p_mlp.tile([1, 1], dt.float32)
              nc.vector.tensor_tensor(zin[:], logits[:], l2[:], ALU.subtract)
              zsoft = p_mlp.tile([1, 1], dt.float32)
              nc.scalar.activation(zsoft[:], zin[:], AF.Sigmoid)
              zhard = p_mlp.tile([1, 1], dt.float32)
              nc.vector.tensor_scalar(zhard[:], zsoft[:], 0.5, None, ALU.is_gt)
              mps = ps_m.tile([128, 1], dt.float32, tag="mlp")
              nc.tensor.matmul(mps[:], ones_r[:], zhard[:], start=True, stop=True)
              mix = p_mlp.tile([128, 1], dt.float32)
              nc.scalar.copy(mix[:], mps[:])
              onem = p_mlp.tile([128, 1], dt.float32)
              nc.vector.tensor_scalar(onem[:], mix[:], -1.0, 1.0, ALU.mult,
                                      ALU.add)
              mlp_ctx.close()

              # --- attention ---
              for h in range(HLOC):
                  for g in range(NCH):
                      pT = p_pT.tile([128, NBLK * CH], dt.float32r, tag="pT")
                      for Ii in range(4):
                          I = 4 * g + Ii
                          nb_tot = (I + 1) * 128
                          e = p_e.tile([128, S], dt.float32, tag="e")
                          pband = p_p.tile([128, S], dt.float32r, tag="p")
                          sums = p_sm.tile([128, 16], dt.float32, tag="sums")
                          grps = _groups(I)
                          ng = len(grps)
                          for gi, (sb, nb) in enumerate(grps):
                              w = nb * 128
                              col = 4 - ng + gi
                              sc = ps_sc.tile([128, 512], dt.float32, tag="sc")
                              nc.tensor.matmul(
                                  sc[:, 0:w], qT[h][:, I * 128:(I + 1) * 128],
                                  kT[:, sb * 128: sb * 128 + w],
                                  start=True, stop=True)
                              if gi == ng - 1:
                                  nc.vector.tensor_tensor(
                                      sc[:, w - 128:w], sc[:, w - 128:w],
                                      diagneg[:], ALU.add)
                              nc.scalar.activation(
                                  e[:, sb * 128: sb * 128 + w], sc[:, 0:w],
                                  AF.Exp, scale=SCALE,
                                  accum_out=sums[:, col:col + 1])
                          if I >= 9:
                              tmask = p_sm.tile([128, 128], dt.float32,
                                                tag="tmask")
                              nc.vector.tensor_reduce(
                                  sums[:, 4:5], e[:, 0:128], AX.X, ALU.add)
                              nc.vector.tensor_tensor(
                                  tmask[:], e[:, (I - 8) * 128:(I - 7) * 128],
                                  triup[:], ALU.mult)
                              nc.vector.tensor_reduce(
                                  sums[:, 5:6], tmask[:], AX.X, ALU.add)
                              nc.vector.tensor_reduce(
                                  sums[:, 6:7], sums[:, 4 - ng:4], AX.X, ALU.add)
                              nc.vector.tensor_reduce(
                                  sums[:, 7:8], sums[:, 2:6], AX.X, ALU.add)
                              nc.vector.reciprocal(sums[:, 8:9], sums[:, 6:7])
                              nc.vector.reciprocal(sums[:, 9:10], sums[:, 7:8])
                              nc.vector.tensor_tensor(
                                  sums[:, 10:11], sums[:, 8:9], onem[:], ALU.mult)
                              nc.vector.tensor_tensor(
                                  sums[:, 11:12], sums[:, 9:10], mix[:], ALU.mult)
                              nc.vector.tensor_tensor(
                                  sums[:, 12:13], sums[:, 10:11], sums[:, 11:12],
                                  ALU.add)
                              a_ap = sums[:, 12:13]
                              b_ap = sums[:, 10:11]
                              amb_ap = sums[:, 11:12]
                              nc.vector.tensor_scalar(
                                  pband[:, 0:128], e[:, 0:128], a_ap, None,
                                  ALU.mult)
                              if I >= 10:
                                  nc.vector.tensor_scalar(
                                      pband[:, 128:(I - 8) * 128],
                                      e[:, 128:(I - 8) * 128], b_ap, None,
                                      ALU.mult)
                              nc.vector.tensor_scalar(
                                  pband[:, (I - 8) * 128:(I - 7) * 128],
                                  e[:, (I - 8) * 128:(I - 7) * 128], b_ap, None,
                                  ALU.mult)
                              nc.vector.scalar_tensor_tensor(
                                  pband[:, (I - 8) * 128:(I - 7) * 128],
                                  tmask[:], amb_ap,
                                  pband[:, (I - 8) * 128:(I - 7) * 128],
                                  ALU.mult, ALU.add)
                              nc.vector.tensor_scalar(
                                  pband[:, (I - 7) * 128:nb_tot],
                                  e[:, (I - 7) * 128:nb_tot], a_ap, None,
                                  ALU.mult)
                          else:
                              nc.vector.tensor_reduce(
                                  sums[:, 6:7], sums[:, 4 - ng:4], AX.X, ALU.add)
                              nc.vector.reciprocal(sums[:, 8:9], sums[:, 6:7])
                              nc.vector.tensor_scalar(
                                  pband[:, 0:nb_tot], e[:, 0:nb_tot],
                                  sums[:, 8:9], None, ALU.mult)
                          pT3 = pT[:, :].rearrange("p (J c) -> p J c", c=CH)
                          for J0 in range(0, I + 1, 4):
                              nb4 = min(4, I + 1 - J0)
                              ptp = ps_pt.tile([128, 512], dt.float32r, tag="pt")
                              for jj in range(nb4):
                                  nc.tensor.transpose(
                                      ptp[:, jj * 128:(jj + 1) * 128],
                                      pband[:, (J0 + jj) * 128:
                                            (J0 + jj + 1) * 128],
                                      ident[:])
                              nc.any.tensor_copy(
                                  pT3[:, J0:J0 + nb4,
                                      Ii * 128:(Ii + 1) * 128],
                                  ptp[:, 0:nb4 * 128].rearrange(
                                      "p (J c) -> p J c", c=128))
                      # PV for this (h, chunk)
                      ops = ps_o.tile([128, CH], dt.float32, tag="o")
                      last_J = 4 * g + 3
                      for J in range(last_J + 1):
                          k = J - 4 * g
                          if k <= 0:
                              nc.tensor.matmul(
                                  ops[:], vN[:, J * 128:(J + 1) * 128],
                                  pT[:, J * CH:(J + 1) * CH],
                                  start=(J == 0), stop=(J == last_J))
                          else:
                              nc.tensor.matmul(
                                  ops[:, k * 128:CH],
                                  vN[:, J * 128:(J + 1) * 128],
                                  pT[:, J * CH + k * 128:(J + 1) * CH],
                                  start=False, stop=(J == last_J))
                      osb = p_ob.tile([128, CH], dt.float32r, tag="osb")
                      nc.scalar.copy(osb[:], ops[:])
                      a2a_in_h = a2a_in0 if h < 2 else a2a_in1
                      hh = h % 2
                      nc.sync.dma_start(
                          a2a_in_h[2 * g, hh * 128:(hh + 1) * 128, :],
                          osb[:, 0:ROWS])
                      nc.sync.dma_start(
                          a2a_in_h[2 * g + 1, hh * 128:(hh + 1) * 128, :],
                          osb[:, ROWS:CH])
                  if h == 1:
                      nc.gpsimd.collective_compute(
                          "AllToAll", ALU.bypass,
                          replica_groups=[list(range(NCORES))],
                          ins=[a2a_in0.opt()], outs=[a2a_out0.opt()])
                  if h == 3:
                      nc.gpsimd.collective_compute(
                          "AllToAll", ALU.bypass,
                          replica_groups=[list(range(NCORES))],
                          ins=[a2a_in1.opt()], outs=[a2a_out1.opt()])

        # ---- phase 3: AllToAll + output projection ----------------------
        with ExitStack() as ph3:
          if KNOBS["phases"] >= 3:
              p_oT = ph3.enter_context(tc.tile_pool(name="oT", bufs=1))
              p_wo = ph3.enter_context(tc.tile_pool(name="wo", bufs=KNOBS["wo_bufs"]))
              p_os = ph3.enter_context(tc.tile_pool(name="outsb", bufs=2))
              ps_w = ph3.enter_context(
                  tc.tile_pool(name="ps_w", bufs=2, space="PSUM"))

              KT_ORDER = [4 * p + t for t in (0, 1, 2, 3) for p in range(NCORES)]
              oT = p_oT.tile([128, KT * ROWS], dt.float32r)
              oTb = p_oT.tile([128, KT * ROWS], dt.bfloat16, name="oTb",
                              tag="oTb")
              for kt in KT_ORDER:
                  p, t = kt // HLOC, kt % HLOC
                  src = a2a_out0 if t < 2 else a2a_out1
                  nc.sync.dma_start(
                      oT[:, kt * ROWS:(kt + 1) * ROWS],
                      src[p, (t % 2) * 128:(t % 2 + 1) * 128, :])
              nc.any.tensor_copy(oTb[:], oT[:])

              for ngi in range(8):
                  pso = [ps_w.tile([128, 512], dt.float32, tag=f"wo{st}",
                                  name=f"wo{st}")
                         for st in range(2)]
                  for ki, kt in enumerate(KT_ORDER):
                      wsl = p_wo.tile([128, 512], dt.bfloat16, tag="wo")
                      nc.sync.dma_start(
                          wsl[:], wo_d[kt * 128:(kt + 1) * 128,
                                       ngi * 512:(ngi + 1) * 512])
                      for st in range(2):
                          nc.tensor.matmul(
                              pso[st][:],
                              oTb[:, kt * ROWS + st * 128:
                                  kt * ROWS + (st + 1) * 128],
                              wsl[:], start=(ki == 0), stop=(ki == KT - 1))
                  for st in range(2):
                      osb = p_os.tile([128, 512], dt.float32, tag="os")
                      nc.scalar.copy(osb[:], pso[st][:])
                      nc.sync.dma_start(
                          out_d[st * 128:(st + 1) * 128,
                                ngi * 512:(ngi + 1) * 512], osb[:])

    nc.compile()
    return nc


_CACHE = {}


def _host_constants():
    inv = 10000.0 ** (-np.arange(0, D, 2, dtype=np.float64) / D)
    t = np.arange(S, dtype=np.float64)
    fr = np.outer(t, inv)                      # [S, 64]
    cos = np.cos(fr).T.astype(np.float32)      # [64, S]
    sin = np.sin(fr).T.astype(np.float32)
    cos2 = np.vstack([cos, cos])
    sin2 = np.vstack([-sin, sin])
    ident = np.eye(128, dtype=np.float32)
    a = np.arange(128)
    diagneg = np.where(a[None, :] <= a[:, None], 0.0, NEG).astype(np.float32)
    triup = (a[None, :] > a[:, None]).astype(np.float32)
    ones = np.ones((1, 128), dtype=np.float32)
    return cos2, sin2, ident, diagneg, triup, ones


def kernel(hidden_states, Wq, Wk, Wv, Wo, fe1_w, fe1_b, fe2_w, fe2_b,
           r1_w, r1_b, r2_w, r2_b, r3_w, r3_b, router_noise):
    if "nc" not in _CACHE:
        _CACHE["nc"] = build()
    nc = _CACHE["nc"]

    import ml_dtypes
    bf16 = ml_dtypes.bfloat16
    hsT = np.ascontiguousarray(
        np.asarray(hidden_states, dtype=np.float32).reshape(S, HID).T
    ).astype(bf16)
    Wq = np.asarray(Wq, np.float32)
    Wk = np.asarray(Wk, np.float32)
    Wv = np.asarray(Wv, np.float32)
    Wo = np.ascontiguousarray(np.asarray(Wo, np.float32)).astype(bf16)
    cos2, sin2, ident, diagneg, triup, ones = _host_constants()

    def ktile_cols(w, ktiles, ntiles, nwidth):
        # [K, N] -> [128, ktiles*ntiles*nwidth] with (k, t) slab layout
        return np.ascontiguousarray(
              np.concatenate([w[k * 128:(k + 1) * 128, :] for k in range(ktiles)],
                             axis=1))

    fe1 = np.asarray(fe1_w, np.float32)                       # [128,1024]
    fe2 = ktile_cols(np.asarray(fe2_w, np.float32), 8, 2, 128)
    r1 = ktile_cols(np.asarray(r1_w, np.float32), 2, 4, 128)
    r2 = ktile_cols(np.asarray(r2_w, np.float32), 4, 1, 128)
    r3 = np.asarray(r3_w, np.float32)                         # [128,1]
    b1 = np.asarray(fe1_b, np.float32).reshape(8, 128).T.copy()
    b2 = np.asarray(fe2_b, np.float32).reshape(2, 128).T.copy()
    rb1 = np.asarray(r1_b, np.float32).reshape(4, 128).T.copy()
    rb2 = np.asarray(r2_b, np.float32).reshape(1, 128).T.copy()
    rb3 = np.asarray(r3_b, np.float32).reshape(1, 1)
    noise = np.asarray(router_noise, np.float32).reshape(1, 1)

    in_maps = []
    for c in range(NCORES):
        wqkv = np.ascontiguousarray(np.concatenate(
              [Wq[:, c * 512:(c + 1) * 512],
               Wk[:, c * 128:(c + 1) * 128],
               Wv[:, c * 128:(c + 1) * 128]], axis=1)).astype(bf16)
        in_maps.append(dict(
              hsT=hsT, wqkv=wqkv, wo=Wo, cos2=cos2, sin2=sin2, ident=ident,
              diagneg=diagneg, triup=triup, ones=ones, fe1=fe1, fe2=fe2,
              r1=r1, r2=r2, r3=r3, b1=b1, b2=b2, rb1=rb1, rb2=rb2, rb3=rb3,
              noise=noise, eps=np.full((1, 1), 1e-8, np.float32)))

    res = run_bass_kernel_spmd(nc, in_maps, list(range(NCORES)))
    out = np.concatenate([res.results[c]["out_rows"] for c in range(NCORES)],
                           axis=0)
    return out.reshape(1, S, HID).astype(np.float32)



# revision 40
# speedup vs baseline: 1.2251x; 1.1325x over previous
"""Trainium2 Bass kernel for nn_LlamaAttention_61899068670751.

Sparse (streaming-LLM) attention layer, sharded tensor-parallel over heads
across 8 NeuronCores:
  - core c owns q-heads [4c..4c+3] and kv-head c (GQA group = 4)
  - QKV projections computed per-core with column-sharded weights
  - causal scores computed once per head; both softmax branches (full causal
    and sink+sliding-window) share exp(s) and are folded into a single PV
    matmul via per-row/per-region coefficients
  - the router MLP is computed redundantly on every core after a [128]
    feature AllReduce; its hard gate enters the coefficients
  - o is exchanged with an AllToAll so each core computes 256 rows of the
    final o @ Wo with the full head dimension; host concatenates row shards

All matmuls run as float32r (full-rate fp32 on the PE array).
"""
import numpy as np
from contextlib import ExitStack

import concourse.bacc as bacc
import concourse.mybir as mybir
import concourse.tile as tile
from concourse.bass_utils import run_bass_kernel_spmd

dt = mybir.dt
AF = mybir.ActivationFunctionType
ALU = mybir.AluOpType
AX = mybir.AxisListType

NCORES = 8
S, H, KV, D, HID = 2048, 32, 8, 128, 4096
SINK, WIN, POOL = 128, 1024, 100
HLOC = H // NCORES          # 4 q heads per core
NBLK = S // 128             # 16 row/col blocks
NCH = 4                     # s-chunks of 512
CH = 512
KT = HID // 128             # 32 contraction tiles
SCALE = 1.0 / float(np.sqrt(D))
NEG = -1.0e30
ROWS = S // NCORES          # 256 output rows per core


def _groups(I):
    """Right-aligned 4-block groups over causal blocks 0..I."""
    n = I + 1
    lo = n % 4
    g = [(0, lo)] if lo else []
    g += [(s, 4) for s in range(lo, n, 4)]
    return g


KNOBS = dict(phases=3, wo_bufs=6, w_bufs=4, hs_bufs=1, sc_bufs=3, pt_bufs=2,
             e_bufs=3, p_bufs=3, pT_bufs=2, o_bufs=1, tr_bufs=2)


def build():
    nc = bacc.Bacc("TRN2", target_bir_lowering=False, debug=False,
                   num_devices=NCORES)

    def din(name, shape, d=dt.float32r):
        return nc.dram_tensor(name, shape, d, kind="ExternalInput").ap()

    hsT_d = din("hsT", [HID, S], dt.bfloat16)
    wqkv_d = din("wqkv", [HID, 768], dt.bfloat16)
    wo_d = din("wo", [HID, HID], dt.bfloat16)
    cos2_d = din("cos2", [128, S], dt.float32)
    sin2_d = din("sin2", [128, S], dt.float32)
    ident_d = din("ident", [128, 128])
    diagneg_d = din("diagneg", [128, 128], dt.float32)
    triup_d = din("triup", [128, 128], dt.float32)
    ones_d = din("ones", [1, 128], dt.float32)
    fe1_d = din("fe1", [128, 1024], dt.float32)
    fe2_d = din("fe2", [128, 8 * 256], dt.float32)
    r1_d = din("r1", [128, 2 * 512], dt.float32)
    r2_d = din("r2", [128, 4 * 128], dt.float32)
    r3_d = din("r3", [128, 1], dt.float32)
    b1_d = din("b1", [128, 8], dt.float32)
    b2_d = din("b2", [128, 2], dt.float32)
    rb1_d = din("rb1", [128, 4], dt.float32)
    rb2_d = din("rb2", [128, 1], dt.float32)
    rb3_d = din("rb3", [1, 1], dt.float32)
    noise_d = din("noise", [1, 1], dt.float32)
    eps_d = din("eps", [1, 1], dt.float32)

    out_d = nc.dram_tensor("out_rows", [ROWS, HID], dt.float32,
                           kind="ExternalOutput").ap()

    with tile.TileContext(nc) as tc, ExitStack() as top:
        # ---- long-lived pools -------------------------------------------
        const = top.enter_context(tc.tile_pool(name="const", bufs=1))
        persist = top.enter_context(tc.tile_pool(name="persist", bufs=1))
        dram = top.enter_context(tc.tile_pool(name="dram", bufs=1, space="DRAM"))

        ident = const.tile([128, 128], dt.float32r)
        diagneg = const.tile([128, 128], dt.float32)
        triup = const.tile([128, 128], dt.float32)
        ones_r = const.tile([1, 128], dt.float32)
        nc.sync.dma_start(ident[:], ident_d[:])
        nc.sync.dma_start(diagneg[:], diagneg_d[:])
        nc.sync.dma_start(triup[:], triup_d[:])
        nc.sync.dma_start(ones_r[:], ones_d[:])

        qT = [persist.tile([128, S], dt.float32r, name=f"qT{h}", tag=f"qT{h}")
              for h in range(HLOC)]
        kT = persist.tile([128, S], dt.float32r)
        vN = persist.tile([128, NBLK * 128], dt.float32r)   # v natural, per block

        # collective bounce buffers
        a2a_in0 = dram.tile([NCORES, 2 * 128, ROWS], dt.float32r)
        a2a_out0 = dram.tile([NCORES, 2 * 128, ROWS], dt.float32r)
        a2a_in1 = dram.tile([NCORES, 2 * 128, ROWS], dt.float32r)
        a2a_out1 = dram.tile([NCORES, 2 * 128, ROWS], dt.float32r)
        cc_in = dram.tile([128, 1], dt.float32)
        cc_out = dram.tile([128, 1], dt.float32, addr_space="Shared")

        # ---- phase 1: hs transpose + QKV projections + rope -------------
        with ExitStack() as ph1:
            p_hs = ph1.enter_context(tc.tile_pool(name="hs", bufs=KNOBS["hs_bufs"]))
            p_hsT = ph1.enter_context(tc.tile_pool(name="hsT", bufs=2))
            p_w = ph1.enter_context(tc.tile_pool(name="wslab", bufs=KNOBS["w_bufs"]))
            p_rope = ph1.enter_context(tc.tile_pool(name="rope", bufs=2))
            p_cs = ph1.enter_context(tc.tile_pool(name="cs", bufs=2))
            ps_tr = ph1.enter_context(
                tc.tile_pool(name="ps_tr", bufs=KNOBS["tr_bufs"], space="PSUM"))
            ps_acc = ph1.enter_context(
                tc.tile_pool(name="ps_acc", bufs=1, space="PSUM"))

            KH = KT // 2        # 16 k-tiles per half
            for g in range(NCH):
                s0 = g * CH
                accs = [ps_acc.tile([128, CH], dt.float32, tag=f"acc{i}",
                                    name=f"acc{i}")
                        for i in range(6)]
                for half in range(2):
                    k0 = half * KH
                    hsT = p_hsT.tile([128, KH, CH], dt.bfloat16, tag="hsT")
                    nc.sync.dma_start(
                        hsT[:], hsT_d[k0 * 128:(k0 + KH) * 128,
                                      s0:s0 + CH].rearrange(
                            "(k p) n -> p k n", p=128))
                    for k4 in range(KH // 4):
                        wsl = p_w.tile([128, 4, 768], dt.bfloat16, tag="w")
                        kb = k0 + 4 * k4
                        nc.sync.dma_start(
                            wsl[:], wqkv_d[kb * 128:(kb + 4) * 128,
                                           :].rearrange(
                                "(k p) n -> p k n", p=128))
                        for kk in range(4):
                            kt = kb + kk
                            for i in range(6):
                                nc.tensor.matmul(
                                    accs[i][:],
                                    wsl[:, kk, i * 128:(i + 1) * 128],
                                    hsT[:, 4 * k4 + kk, :],
                                    start=(kt == 0), stop=(kt == KT - 1))

                # rope for q heads (0..3) and k (4)
                cos_sl = p_cs.tile([128, CH], dt.float32, tag="cos")
                sin_sl = p_cs.tile([128, CH], dt.float32, tag="sin")
                nc.sync.dma_start(cos_sl[:], cos2_d[:, s0:s0 + CH])
                nc.sync.dma_start(sin_sl[:], sin2_d[:, s0:s0 + CH])
                for i in range(5):
                    dest = qT[i] if i < HLOC else kT
                    lin = p_rope.tile([128, CH], dt.float32, tag="lin")
                    rot = p_rope.tile([128, CH], dt.float32, tag="rot")
                    t1 = p_rope.tile([128, CH], dt.float32, tag="t1")
                    t2 = p_rope.tile([128, CH], dt.float32, tag="t2")
                    nc.scalar.copy(lin[:], accs[i][:])
                    nc.sync.dma_start(rot[0:64, :], lin[64:128, :])
                    nc.sync.dma_start(rot[64:128, :], lin[0:64, :])
                    nc.vector.tensor_tensor(t1[:], lin[:], cos_sl[:], ALU.mult)
                    nc.vector.tensor_tensor(t2[:], rot[:], sin_sl[:], ALU.mult)
                    nc.vector.tensor_tensor(dest[:, s0:s0 + CH], t1[:], t2[:],
                                            ALU.add)
                # v: copy then transpose to natural layout
                vT = p_rope.tile([128, CH], dt.float32r, tag="vT")
                nc.scalar.copy(vT[:], accs[5][:])
                for ss in range(4):
                    ptr = ps_tr.tile([128, 128], dt.float32r, tag="tr")
                    nc.tensor.transpose(ptr[:], vT[:, ss * 128:(ss + 1) * 128],
                                        ident[:])
                    nc.any.tensor_copy(
                        vN[:, (g * 4 + ss) * 128:(g * 4 + ss + 1) * 128], ptr[:])

        # ---- phase 2: router + attention --------------------------------
        with ExitStack() as ph2:
          if KNOBS["phases"] >= 2:
              p_mlp = ph2.enter_context(tc.tile_pool(name="mlp", bufs=1))
              p_e = ph2.enter_context(tc.tile_pool(name="eband", bufs=KNOBS["e_bufs"]))
              p_p = ph2.enter_context(tc.tile_pool(name="pband", bufs=KNOBS["p_bufs"]))
              p_pT = ph2.enter_context(tc.tile_pool(name="pT", bufs=KNOBS["pT_bufs"]))
              p_sm = ph2.enter_context(tc.tile_pool(name="sums", bufs=4))
              p_ob = ph2.enter_context(tc.tile_pool(name="obuf", bufs=2))
              ps_sc = ph2.enter_context(
                  tc.tile_pool(name="ps_sc", bufs=KNOBS["sc_bufs"], space="PSUM"))
              ps_pt = ph2.enter_context(
                  tc.tile_pool(name="ps_pt", bufs=KNOBS["pt_bufs"], space="PSUM"))
              ps_o = ph2.enter_context(
                  tc.tile_pool(name="ps_o", bufs=KNOBS["o_bufs"], space="PSUM"))
              mlp_ctx = ExitStack()
              ps_m = mlp_ctx.enter_context(
                  tc.tile_pool(name="ps_m", bufs=1, space="PSUM"))

              # --- router ---
              feat_acc = p_mlp.tile([128, 8], dt.float32)
              for h in range(HLOC):
                  nc.vector.tensor_reduce(feat_acc[:, h:h + 1],
                                          qT[h][:, 0:POOL], AX.X, ALU.add)
                  nc.vector.tensor_reduce(feat_acc[:, 4 + h:5 + h],
                                          qT[h][:, S - POOL:S], AX.X, ALU.add)
              feat_s = p_mlp.tile([128, 1], dt.float32)
              nc.vector.tensor_reduce(feat_s[:], feat_acc[:], AX.X, ALU.add)
              feat_r = p_mlp.tile([128, 1], dt.float32)
              nc.scalar.activation(feat_r[:], feat_s[:], AF.Copy,
                                   scale=1.0 / (2 * POOL * H))
              nc.sync.dma_start(cc_in[:], feat_r[:])
              nc.gpsimd.collective_compute(
                  "AllReduce", ALU.add,
                  replica_groups=[list(range(NCORES))],
                  ins=[cc_in.opt()], outs=[cc_out.opt()])
              featg = p_mlp.tile([128, 1], dt.float32)
              nc.sync.dma_start(featg[:], cc_out[:])

              # MLP weights
              fe1 = p_mlp.tile([128, 1024], dt.float32)
              fe2 = p_mlp.tile([128, 8 * 256], dt.float32)
              r1w = p_mlp.tile([128, 2 * 512], dt.float32)
              r2w = p_mlp.tile([128, 4 * 128], dt.float32)
              r3w = p_mlp.tile([128, 1], dt.float32)
              b1 = p_mlp.tile([128, 8], dt.float32)
              b2 = p_mlp.tile([128, 2], dt.float32)
              rb1 = p_mlp.tile([128, 4], dt.float32)
              rb2 = p_mlp.tile([128, 1], dt.float32)
              rb3 = p_mlp.tile([1, 1], dt.float32)
              noise = p_mlp.tile([1, 1], dt.float32)
              epsb = p_mlp.tile([1, 1], dt.float32)
              nc.sync.dma_start(epsb[:], eps_d[:])
              for t_, d_ in ((fe1, fe1_d), (fe2, fe2_d), (r1w, r1_d),
                             (r2w, r2_d), (r3w, r3_d), (b1, b1_d), (b2, b2_d),
                             (rb1, rb1_d), (rb2, rb2_d), (rb3, rb3_d),
                             (noise, noise_d)):
                  nc.sync.dma_start(t_[:], d_[:])

              def mlp_layer(vec_in, w_sb, ktiles, ntiles, bias, act, nwidth=128):
                  """vec_in: [128, ktiles] fp32r columns; returns [128, ntiles]."""
                  out_r = p_mlp.tile([128, max(ntiles, 1)], dt.float32,
                                     name=f"mlpv{len(mlp_tmp)}")
                  mlp_tmp.append(out_r)
                  ps = ps_m.tile([128, max(ntiles, 1)], dt.float32, tag="mlp",
                               name="mlpps")
                  for t in range(ntiles):
                      for k in range(ktiles):
                          nc.tensor.matmul(
                              ps[:, t:t + 1],
                              w_sb[:, (k * ntiles + t) * nwidth:
                                   (k * ntiles + t) * nwidth + nwidth],
                              vec_in[:, k:k + 1],
                              start=(k == 0), stop=(k == ktiles - 1))
                  for t in range(ntiles):
                      nc.scalar.activation(out_r[:, t:t + 1], ps[:, t:t + 1],
                                           act, bias=bias[:, t:t + 1])
                  return out_r

              mlp_tmp = []
              h1 = mlp_layer(featg, fe1, 1, 8, b1, AF.Silu)
              h2 = mlp_layer(h1, fe2, 8, 2, b2, AF.Identity)
              h3 = mlp_layer(h2, r1w, 2, 4, rb1, AF.Silu)
              h4 = mlp_layer(h3, r2w, 4, 1, rb2, AF.Silu)
              lps = ps_m.tile([1, 1], dt.float32, tag="mlp")
              nc.tensor.matmul(lps[:], r3w[:], h4[:], start=True, stop=True)
              logits = p_mlp.tile([1, 1], dt.float32)
              nc.scalar.activation(logits[:], lps[:], AF.Identity, bias=rb3[:])
              l1 = p_mlp.tile([1, 1], dt.float32)
              l2 = p_mlp.tile([1, 1], dt.float32)
              nc.scalar.activation(l1[:], noise[:], AF.Ln, bias=epsb[:])
              nc.scalar.activation(l2[:], l1[:], AF.Ln, bias=epsb[:], scale=-1.0)
              zin = p_mlp.tile([1, 1], dt.float32)
              nc.vector.tensor_tensor(zin[:], logits[:], l2[:], ALU.subtract)
              zsoft = p_mlp.tile([1, 1], dt.float32)
              nc.scalar.activation(zsoft[:], zin[:], AF.Sigmoid)
              zhard = p_mlp.tile([1, 1], dt.float32)
              nc.vector.tensor_scalar(zhard[:], zsoft[:], 0.5, None, ALU.is_gt)
              mps = ps_m.tile([128, 1], dt.float32, tag="mlp")
              nc.tensor.matmul(mps[:], ones_r[:], zhard[:], start=True, stop=True)
              mix = p_mlp.tile([128, 1], dt.float32)
              nc.scalar.copy(mix[:], mps[:])
              onem = p_mlp.tile([128, 1], dt.float32)
              nc.vector.tensor_scalar(onem[:], mix[:], -1.0, 1.0, ALU.mult,
                                      ALU.add)
              mlp_ctx.close()

              # --- attention ---
              for h in range(HLOC):
                  for g in range(NCH):
                      pT = p_pT.tile([128, NBLK * CH], dt.float32r, tag="pT")
                      for Ii in range(4):
                          I = 4 * g + Ii
                          nb_tot = (I + 1) * 128
                          e = p_e.tile([128, S], dt.float32, tag="e")
                          pband = p_p.tile([128, S], dt.float32r, tag="p")
                          sums = p_sm.tile([128, 16], dt.float32, tag="sums")
                          grps = _groups(I)
                          ng = len(grps)
                          for gi, (sb, nb) in enumerate(grps):
                              w = nb * 128
                              col = 4 - ng + gi
                              sc = ps_sc.tile([128, 512], dt.float32, tag="sc")
                              nc.tensor.matmul(
                                  sc[:, 0:w], qT[h][:, I * 128:(I + 1) * 128],
                                  kT[:, sb * 128: sb * 128 + w],
                                  start=True, stop=True)
                              if gi == ng - 1:
                                  nc.vector.tensor_tensor(
                                      sc[:, w - 128:w], sc[:, w - 128:w],
                                      diagneg[:], ALU.add)
                              nc.scalar.activation(
                                  e[:, sb * 128: sb * 128 + w], sc[:, 0:w],
                                  AF.Exp, scale=SCALE,
                                  accum_out=sums[:, col:col + 1])
                          if I >= 9:
                              tmask = p_sm.tile([128, 128], dt.float32,
                                                tag="tmask")
                              nc.vector.tensor_reduce(
                                  sums[:, 4:5], e[:, 0:128], AX.X, ALU.add)
                              nc.vector.tensor_tensor(
                                  tmask[:], e[:, (I - 8) * 128:(I - 7) * 128],
                                  triup[:], ALU.mult)
                              nc.vector.tensor_reduce(
                                  sums[:, 5:6], tmask[:], AX.X, ALU.add)
                              nc.vector.tensor_reduce(
                                  sums[:, 6:7], sums[:, 4 - ng:4], AX.X, ALU.add)
                              nc.vector.tensor_reduce(
                                  sums[:, 7:8], sums[:, 2:6], AX.X, ALU.add)
                              nc.vector.reciprocal(sums[:, 8:9], sums[:, 6:7])
                              nc.vector.reciprocal(sums[:, 9:10], sums[:, 7:8])
                              nc.vector.tensor_tensor(
                                  sums[:, 10:11], sums[:, 8:9], onem[:], ALU.mult)
                              nc.vector.tensor_tensor(
                                  sums[:, 11:12], sums[:, 9:10], mix[:], ALU.mult)
                              nc.vector.tensor_tensor(
                                  sums[:, 12:13], sums[:, 10:11], sums[:, 11:12],
                                  ALU.add)
                              a_ap = sums[:, 12:13]
                              b_ap = sums[:, 10:11]
                              amb_ap = sums[:, 11:12]
                              nc.vector.tensor_scalar(
                                  pband[:, 0:128], e[:, 0:128], a_ap, None,
                                  ALU.mult)
                              if I >= 10:
                                  nc.vector.tensor_scalar(
                                      pband[:, 128:(I - 8) * 128],
                                      e[:, 128:(I - 8) * 128], b_ap, None,
                                      ALU.mult)
                              nc.vector.tensor_scalar(
                                  pband[:, (I - 8) * 128:(I - 7) * 128],
                                  e[:, (I - 8) * 128:(I - 7) * 128], b_ap, None,
                                  ALU.mult)
                              nc.vector.scalar_tensor_tensor(
                                  pband[:, (I - 8) * 128:(I - 7) * 128],
                                  tmask[:], amb_ap,
                                  pband[:, (I - 8) * 128:(I - 7) * 128],
                                  ALU.mult, ALU.add)
                              nc.vector.tensor_scalar(
                                  pband[:, (I - 7) * 128:nb_tot],
                                  e[:, (I - 7) * 128:nb_tot], a_ap, None,
                                  ALU.mult)
                          else:
                              nc.vector.tensor_reduce(
                                  sums[:, 6:7], sums[:, 4 - ng:4], AX.X, ALU.add)
                              nc.vector.reciprocal(sums[:, 8:9], sums[:, 6:7])
                              nc.vector.tensor_scalar(
                                  pband[:, 0:nb_tot], e[:, 0:nb_tot],
                                  sums[:, 8:9], None, ALU.mult)
                          pT3 = pT[:, :].rearrange("p (J c) -> p J c", c=CH)
                          for J0 in range(0, I + 1, 4):
                              nb4 = min(4, I + 1 - J0)
                              ptp = ps_pt.tile([128, 512], dt.float32r, tag="pt")
                              for jj in range(nb4):
                                  nc.tensor.transpose(
                                      ptp[:, jj * 128:(jj + 1) * 128],
                                      pband[:, (J0 + jj) * 128:
                                            (J0 + jj + 1) * 128],
                                      ident[:])
                              nc.any.tensor_copy(
                                  pT3[:, J0:J0 + nb4,
                                      Ii * 128:(Ii + 1) * 128],
                                  ptp[:, 0:nb4 * 128].rearrange(
                                      "p (J c) -> p J c", c=128))
                      # PV for this (h, chunk)
                      ops = ps_o.tile([128, CH], dt.float32, tag="o")
                      last_J = 4 * g + 3
                      for J in range(last_J + 1):
                          k = J - 4 * g
                          if k <= 0:
                              nc.tensor.matmul(
                                  ops[:], vN[:, J * 128:(J + 1) * 128],
                                  pT[:, J * CH:(J + 1) * CH],
                                  start=(J == 0), stop=(J == last_J))
                          else:
                              nc.tensor.matmul(
                                  ops[:, k * 128:CH],
                                  vN[:, J * 128:(J + 1) * 128],
                                  pT[:, J * CH + k * 128:(J + 1) * CH],
                                  start=False, stop=(J == last_J))
                      osb = p_ob.tile([128, CH], dt.float32r, tag="osb")
                      nc.scalar.copy(osb[:], ops[:])
                      a2a_in_h = a2a_in0 if h < 2 else a2a_in1
                      hh = h % 2
                      nc.sync.dma_start(
                          a2a_in_h[2 * g, hh * 128:(hh + 1) * 128, :],
                          osb[:, 0:ROWS])
                      nc.sync.dma_start(
                          a2a_in_h[2 * g + 1, hh * 128:(hh + 1) * 128, :],
                          osb[:, ROWS:CH])
                  if h == 1:
                      nc.gpsimd.collective_compute(
                          "AllToAll", ALU.bypass,
                          replica_groups=[list(range(NCORES))],
                          ins=[a2a_in0.opt()], outs=[a2a_out0.opt()])
                  if h == 3:
                      nc.gpsimd.collective_compute(
                          "AllToAll", ALU.bypass,
                          replica_groups=[list(range(NCORES))],
                          ins=[a2a_in1.opt()], outs=[a2a_out1.opt()])

        # ---- phase 3: AllToAll + output projection ----------------------
        with ExitStack() as ph3:
          if KNOBS["phases"] >= 3:
              p_oT = ph3.enter_context(tc.tile_pool(name="oT", bufs=1))
              p_wo = ph3.enter_context(tc.tile_pool(name="wo", bufs=KNOBS["wo_bufs"]))
              p_os = ph3.enter_context(tc.tile_pool(name="outsb", bufs=2))
              ps_w = ph3.enter_context(
                  tc.tile_pool(name="ps_w", bufs=2, space="PSUM"))

              KT_ORDER = [4 * p + t for t in (0, 1, 2, 3) for p in range(NCORES)]
              oT = p_oT.tile([128, KT * ROWS], dt.float32r)
              oTb = p_oT.tile([128, KT * ROWS], dt.bfloat16, name="oTb",
                              tag="oTb")
              for kt in KT_ORDER:
                  p, t = kt // HLOC, kt % HLOC
                  src = a2a_out0 if t < 2 else a2a_out1
                  nc.sync.dma_start(
                      oT[:, kt * ROWS:(kt + 1) * ROWS],
                      src[p, (t % 2) * 128:(t % 2 + 1) * 128, :])
              nc.any.tensor_copy(oTb[:], oT[:])

              for ngi in range(8):
                  pso = [ps_w.tile([128, 512], dt.float32, tag=f"wo{st}",
                                  name=f"wo{st}")
                         for st in range(2)]
                  for b4 in range(KT // 4):
                      wsl = p_wo.tile([128, 4, 512], dt.bfloat16, tag="wo")
                      nc.sync.dma_start(
                          wsl[:], wo_d[b4 * 512:(b4 + 1) * 512,
                                       ngi * 512:(ngi + 1) * 512].rearrange(
                              "(k p) n -> p k n", p=128))
                      for kk in range(4):
                          kt = 4 * b4 + kk
                          for st in range(2):
                              nc.tensor.matmul(
                                  pso[st][:],
                                  oTb[:, kt * ROWS + st * 128:
                                      kt * ROWS + (st + 1) * 128],
                                  wsl[:, kk, :], start=(kt == 0),
                                  stop=(kt == KT - 1))
                  for st in range(2):
                      osb = p_os.tile([128, 512], dt.float32, tag="os")
                      nc.scalar.copy(osb[:], pso[st][:])
                      nc.sync.dma_start(
                          out_d[st * 128:(st + 1) * 128,
                                ngi * 512:(ngi + 1) * 512], osb[:])

    nc.compile()
    return nc


_CACHE = {}


def _host_constants():
    inv = 10000.0 ** (-np.arange(0, D, 2, dtype=np.float64) / D)
    t = np.arange(S, dtype=np.float64)
    fr = np.outer(t, inv)                      # [S, 64]
    cos = np.cos(fr).T.astype(np.float32)      # [64, S]
    sin = np.sin(fr).T.astype(np.float32)
    cos2 = np.vstack([cos, cos])
    sin2 = np.vstack([-sin, sin])
    ident = np.eye(128, dtype=np.float32)
    a = np.arange(128)
    diagneg = np.where(a[None, :] <= a[:, None], 0.0, NEG).astype(np.float32)
    triup = (a[None, :] > a[:, None]).astype(np.float32)
    ones = np.ones((1, 128), dtype=np.float32)
    return cos2, sin2, ident, diagneg, triup, ones


def kernel(hidden_states, Wq, Wk, Wv, Wo, fe1_w, fe1_b, fe2_w, fe2_b,
           r1_w, r1_b, r2_w, r2_b, r3_w, r3_b, router_noise):
    if "nc" not in _CACHE:
        _CACHE["nc"] = build()
    nc = _CACHE["nc"]

    import ml_dtypes
    bf16 = ml_dtypes.bfloat16
    hsT = np.ascontiguousarray(
        np.asarray(hidden_states, dtype=np.float32).reshape(S, HID).T
    ).astype(bf16)
    Wq = np.asarray(Wq, np.float32)
    Wk = np.asarray(Wk, np.float32)
    Wv = np.asarray(Wv, np.float32)
    Wo = np.ascontiguousarray(np.asarray(Wo, np.float32)).astype(bf16)
    cos2, sin2, ident, diagneg, triup, ones = _host_constants()

    def ktile_cols(w, ktiles, ntiles, nwidth):
        # [K, N] -> [128, ktiles*ntiles*nwidth] with (k, t) slab layout
        return np.ascontiguousarray(
              np.concatenate([w[k * 128:(k + 1) * 128, :] for k in range(ktiles)],
                             axis=1))

    fe1 = np.asarray(fe1_w, np.float32)                       # [128,1024]
    fe2 = ktile_cols(np.asarray(fe2_w, np.float32), 8, 2, 128)
    r1 = ktile_cols(np.asarray(r1_w, np.float32), 2, 4, 128)
    r2 = ktile_cols(np.asarray(r2_w, np.float32), 4, 1, 128)
    r3 = np.asarray(r3_w, np.float32)                         # [128,1]
    b1 = np.asarray(fe1_b, np.float32).reshape(8, 128).T.copy()
    b2 = np.asarray(fe2_b, np.float32).reshape(2, 128).T.copy()
    rb1 = np.asarray(r1_b, np.float32).reshape(4, 128).T.copy()
    rb2 = np.asarray(r2_b, np.float32).reshape(1, 128).T.copy()
    rb3 = np.asarray(r3_b, np.float32).reshape(1, 1)
    noise = np.asarray(router_noise, np.float32).reshape(1, 1)

    in_maps = []
    for c in range(NCORES):
        wqkv = np.ascontiguousarray(np.concatenate(
              [Wq[:, c * 512:(c + 1) * 512],
               Wk[:, c * 128:(c + 1) * 128],
               Wv[:, c * 128:(c + 1) * 128]], axis=1)).astype(bf16)
        in_maps.append(dict(
              hsT=hsT, wqkv=wqkv, wo=Wo, cos2=cos2, sin2=sin2, ident=ident,
              diagneg=diagneg, triup=triup, ones=ones, fe1=fe1, fe2=fe2,
              r1=r1, r2=r2, r3=r3, b1=b1, b2=b2, rb1=rb1, rb2=rb2, rb3=rb3,
              noise=noise, eps=np.full((1, 1), 1e-8, np.float32)))

    res = run_bass_kernel_spmd(nc, in_maps, list(range(NCORES)))
    out = np.concatenate([res.results[c]["out_rows"] for c in range(NCORES)],
                           axis=0)
    return out.reshape(1, S, HID).astype(np.float32)



# revision 42
# speedup vs baseline: 1.3375x; 1.0917x over previous
"""Trainium2 Bass kernel for nn_LlamaAttention_61899068670751.

Sparse (streaming-LLM) attention layer, sharded tensor-parallel over heads
across 8 NeuronCores:
  - core c owns q-heads [4c..4c+3] and kv-head c (GQA group = 4)
  - QKV projections computed per-core with column-sharded weights
  - causal scores computed once per head; both softmax branches (full causal
    and sink+sliding-window) share exp(s) and are folded into a single PV
    matmul via per-row/per-region coefficients
  - the router MLP is computed redundantly on every core after a [128]
    feature AllReduce; its hard gate enters the coefficients
  - o is exchanged with an AllToAll so each core computes 256 rows of the
    final o @ Wo with the full head dimension; host concatenates row shards

All matmuls run as float32r (full-rate fp32 on the PE array).
"""
import numpy as np
from contextlib import ExitStack

import concourse.bacc as bacc
import concourse.mybir as mybir
import concourse.tile as tile
from concourse.bass_utils import run_bass_kernel_spmd

dt = mybir.dt
AF = mybir.ActivationFunctionType
ALU = mybir.AluOpType
AX = mybir.AxisListType

NCORES = 8
S, H, KV, D, HID = 2048, 32, 8, 128, 4096
SINK, WIN, POOL = 128, 1024, 100
HLOC = H // NCORES          # 4 q heads per core
NBLK = S // 128             # 16 row/col blocks
NCH = 4                     # s-chunks of 512
CH = 512
KT = HID // 128             # 32 contraction tiles
SCALE = 1.0 / float(np.sqrt(D))
NEG = -1.0e30
ROWS = S // NCORES          # 256 output rows per core


def _groups(I):
    """Right-aligned 4-block groups over causal blocks 0..I."""
    n = I + 1
    lo = n % 4
    g = [(0, lo)] if lo else []
    g += [(s, 4) for s in range(lo, n, 4)]
    return g


KNOBS = dict(phases=3, wo_bufs=16, w_bufs=4, hs_bufs=1, sc_bufs=3, pt_bufs=2,
             e_bufs=3, p_bufs=3, pT_bufs=2, o_bufs=1, tr_bufs=2)


def build():
    nc = bacc.Bacc("TRN2", target_bir_lowering=False, debug=False,
                   num_devices=NCORES)

    def din(name, shape, d=dt.float32r):
        return nc.dram_tensor(name, shape, d, kind="ExternalInput").ap()

    hsT_d = din("hsT", [HID, S], dt.bfloat16)
    wqkv_d = din("wqkv", [HID, 768], dt.bfloat16)
    wo_d = din("wo", [HID, HID], dt.bfloat16)
    cos2_d = din("cos2", [128, S], dt.float32)
    sin2_d = din("sin2", [128, S], dt.float32)
    ident_d = din("ident", [128, 128])
    diagneg_d = din("diagneg", [128, 128], dt.float32)
    triup_d = din("triup", [128, 128], dt.float32)
    ones_d = din("ones", [1, 128], dt.float32)
    fe1_d = din("fe1", [128, 1024], dt.float32)
    fe2_d = din("fe2", [128, 8 * 256], dt.float32)
    r1_d = din("r1", [128, 2 * 512], dt.float32)
    r2_d = din("r2", [128, 4 * 128], dt.float32)
    r3_d = din("r3", [128, 1], dt.float32)
    b1_d = din("b1", [128, 8], dt.float32)
    b2_d = din("b2", [128, 2], dt.float32)
    rb1_d = din("rb1", [128, 4], dt.float32)
    rb2_d = din("rb2", [128, 1], dt.float32)
    rb3_d = din("rb3", [1, 1], dt.float32)
    noise_d = din("noise", [1, 1], dt.float32)
    eps_d = din("eps", [1, 1], dt.float32)

    out_d = nc.dram_tensor("out_rows", [ROWS, HID], dt.float32,
                           kind="ExternalOutput").ap()

    with tile.TileContext(nc) as tc, ExitStack() as top:
        # ---- long-lived pools -------------------------------------------
        const = top.enter_context(tc.tile_pool(name="const", bufs=1))
        persist = top.enter_context(tc.tile_pool(name="persist", bufs=1))
        dram = top.enter_context(tc.tile_pool(name="dram", bufs=1, space="DRAM"))

        ident = const.tile([128, 128], dt.float32r)
        diagneg = const.tile([128, 128], dt.float32)
        triup = const.tile([128, 128], dt.float32)
        ones_r = const.tile([1, 128], dt.float32)
        nc.sync.dma_start(ident[:], ident_d[:])
        nc.sync.dma_start(diagneg[:], diagneg_d[:])
        nc.sync.dma_start(triup[:], triup_d[:])
        nc.sync.dma_start(ones_r[:], ones_d[:])

        qT = [persist.tile([128, S], dt.float32r, name=f"qT{h}", tag=f"qT{h}")
              for h in range(HLOC)]
        kT = persist.tile([128, S], dt.float32r)
        vN = persist.tile([128, NBLK * 128], dt.float32r)   # v natural, per block

        # collective bounce buffers
        a2a_in0 = dram.tile([NCORES, 2 * 128, ROWS], dt.bfloat16)
        a2a_out0 = dram.tile([NCORES, 2 * 128, ROWS], dt.bfloat16)
        a2a_in1 = dram.tile([NCORES, 2 * 128, ROWS], dt.bfloat16)
        a2a_out1 = dram.tile([NCORES, 2 * 128, ROWS], dt.bfloat16)
        cc_in = dram.tile([128, 1], dt.float32)
        cc_out = dram.tile([128, 1], dt.float32, addr_space="Shared")

        # ---- phase 1: hs transpose + QKV projections + rope -------------
        with ExitStack() as ph1:
            p_hs = ph1.enter_context(tc.tile_pool(name="hs", bufs=KNOBS["hs_bufs"]))
            p_hsT = ph1.enter_context(tc.tile_pool(name="hsT", bufs=2))
            p_w = ph1.enter_context(tc.tile_pool(name="wslab", bufs=KNOBS["w_bufs"]))
            p_rope = ph1.enter_context(tc.tile_pool(name="rope", bufs=2))
            p_cs = ph1.enter_context(tc.tile_pool(name="cs", bufs=2))
            ps_tr = ph1.enter_context(
                tc.tile_pool(name="ps_tr", bufs=KNOBS["tr_bufs"], space="PSUM"))
            ps_acc = ph1.enter_context(
                tc.tile_pool(name="ps_acc", bufs=1, space="PSUM"))

            KH = KT // 2        # 16 k-tiles per half
            for g in range(NCH):
                s0 = g * CH
                accs = [ps_acc.tile([128, CH], dt.float32, tag=f"acc{i}",
                                    name=f"acc{i}")
                        for i in range(6)]
                for half in range(2):
                    k0 = half * KH
                    hsT = p_hsT.tile([128, KH, CH], dt.bfloat16, tag="hsT")
                    nc.sync.dma_start(
                        hsT[:], hsT_d[k0 * 128:(k0 + KH) * 128,
                                      s0:s0 + CH].rearrange(
                            "(k p) n -> p k n", p=128))
                    for k4 in range(KH // 4):
                        wsl = p_w.tile([128, 4, 768], dt.bfloat16, tag="w")
                        kb = k0 + 4 * k4
                        nc.sync.dma_start(
                            wsl[:], wqkv_d[kb * 128:(kb + 4) * 128,
                                           :].rearrange(
                                "(k p) n -> p k n", p=128))
                        for kk in range(4):
                            kt = kb + kk
                            for i in range(6):
                                nc.tensor.matmul(
                                    accs[i][:],
                                    wsl[:, kk, i * 128:(i + 1) * 128],
                                    hsT[:, 4 * k4 + kk, :],
                                    start=(kt == 0), stop=(kt == KT - 1))

                # rope for q heads (0..3) and k (4)
                cos_sl = p_cs.tile([128, CH], dt.float32, tag="cos")
                sin_sl = p_cs.tile([128, CH], dt.float32, tag="sin")
                nc.sync.dma_start(cos_sl[:], cos2_d[:, s0:s0 + CH])
                nc.sync.dma_start(sin_sl[:], sin2_d[:, s0:s0 + CH])
                for i in range(5):
                    dest = qT[i] if i < HLOC else kT
                    lin = p_rope.tile([128, CH], dt.float32, tag="lin")
                    rot = p_rope.tile([128, CH], dt.float32, tag="rot")
                    t1 = p_rope.tile([128, CH], dt.float32, tag="t1")
                    t2 = p_rope.tile([128, CH], dt.float32, tag="t2")
                    nc.scalar.copy(lin[:], accs[i][:])
                    nc.sync.dma_start(rot[0:64, :], lin[64:128, :])
                    nc.sync.dma_start(rot[64:128, :], lin[0:64, :])
                    nc.vector.tensor_tensor(t1[:], lin[:], cos_sl[:], ALU.mult)
                    nc.vector.tensor_tensor(t2[:], rot[:], sin_sl[:], ALU.mult)
                    nc.vector.tensor_tensor(dest[:, s0:s0 + CH], t1[:], t2[:],
                                            ALU.add)
                # v: copy then transpose to natural layout
                vT = p_rope.tile([128, CH], dt.float32r, tag="vT")
                nc.scalar.copy(vT[:], accs[5][:])
                for ss in range(4):
                    ptr = ps_tr.tile([128, 128], dt.float32r, tag="tr")
                    nc.tensor.transpose(ptr[:], vT[:, ss * 128:(ss + 1) * 128],
                                        ident[:])
                    nc.any.tensor_copy(
                        vN[:, (g * 4 + ss) * 128:(g * 4 + ss + 1) * 128], ptr[:])

        # ---- phase 2: router + attention --------------------------------
        with ExitStack() as ph2:
          if KNOBS["phases"] >= 2:
              p_mlp = ph2.enter_context(tc.tile_pool(name="mlp", bufs=1))
              p_e = ph2.enter_context(tc.tile_pool(name="eband", bufs=KNOBS["e_bufs"]))
              p_p = ph2.enter_context(tc.tile_pool(name="pband", bufs=KNOBS["p_bufs"]))
              p_pT = ph2.enter_context(tc.tile_pool(name="pT", bufs=KNOBS["pT_bufs"]))
              p_sm = ph2.enter_context(tc.tile_pool(name="sums", bufs=4))
              p_ob = ph2.enter_context(tc.tile_pool(name="obuf", bufs=2))
              ps_sc = ph2.enter_context(
                  tc.tile_pool(name="ps_sc", bufs=KNOBS["sc_bufs"], space="PSUM"))
              ps_pt = ph2.enter_context(
                  tc.tile_pool(name="ps_pt", bufs=KNOBS["pt_bufs"], space="PSUM"))
              ps_o = ph2.enter_context(
                  tc.tile_pool(name="ps_o", bufs=KNOBS["o_bufs"], space="PSUM"))
              mlp_ctx = ExitStack()
              ps_m = mlp_ctx.enter_context(
                  tc.tile_pool(name="ps_m", bufs=1, space="PSUM"))

              # --- router ---
              feat_acc = p_mlp.tile([128, 8], dt.float32)
              for h in range(HLOC):
                  nc.vector.tensor_reduce(feat_acc[:, h:h + 1],
                                          qT[h][:, 0:POOL], AX.X, ALU.add)
                  nc.vector.tensor_reduce(feat_acc[:, 4 + h:5 + h],
                                          qT[h][:, S - POOL:S], AX.X, ALU.add)
              feat_s = p_mlp.tile([128, 1], dt.float32)
              nc.vector.tensor_reduce(feat_s[:], feat_acc[:], AX.X, ALU.add)
              feat_r = p_mlp.tile([128, 1], dt.float32)
              nc.scalar.activation(feat_r[:], feat_s[:], AF.Copy,
                                   scale=1.0 / (2 * POOL * H))
              nc.sync.dma_start(cc_in[:], feat_r[:])
              nc.gpsimd.collective_compute(
                  "AllReduce", ALU.add,
                  replica_groups=[list(range(NCORES))],
                  ins=[cc_in.opt()], outs=[cc_out.opt()])
              featg = p_mlp.tile([128, 1], dt.float32)
              nc.sync.dma_start(featg[:], cc_out[:])

              # MLP weights
              fe1 = p_mlp.tile([128, 1024], dt.float32)
              fe2 = p_mlp.tile([128, 8 * 256], dt.float32)
              r1w = p_mlp.tile([128, 2 * 512], dt.float32)
              r2w = p_mlp.tile([128, 4 * 128], dt.float32)
              r3w = p_mlp.tile([128, 1], dt.float32)
              b1 = p_mlp.tile([128, 8], dt.float32)
              b2 = p_mlp.tile([128, 2], dt.float32)
              rb1 = p_mlp.tile([128, 4], dt.float32)
              rb2 = p_mlp.tile([128, 1], dt.float32)
              rb3 = p_mlp.tile([1, 1], dt.float32)
              noise = p_mlp.tile([1, 1], dt.float32)
              epsb = p_mlp.tile([1, 1], dt.float32)
              nc.sync.dma_start(epsb[:], eps_d[:])
              for t_, d_ in ((fe1, fe1_d), (fe2, fe2_d), (r1w, r1_d),
                             (r2w, r2_d), (r3w, r3_d), (b1, b1_d), (b2, b2_d),
                             (rb1, rb1_d), (rb2, rb2_d), (rb3, rb3_d),
                             (noise, noise_d)):
                  nc.sync.dma_start(t_[:], d_[:])

              def mlp_layer(vec_in, w_sb, ktiles, ntiles, bias, act, nwidth=128):
                  """vec_in: [128, ktiles] fp32r columns; returns [128, ntiles]."""
                  out_r = p_mlp.tile([128, max(ntiles, 1)], dt.float32,
                                     name=f"mlpv{len(mlp_tmp)}")
                  mlp_tmp.append(out_r)
                  ps = ps_m.tile([128, max(ntiles, 1)], dt.float32, tag="mlp",
                               name="mlpps")
                  for t in range(ntiles):
                      for k in range(ktiles):
                          nc.tensor.matmul(
                              ps[:, t:t + 1],
                              w_sb[:, (k * ntiles + t) * nwidth:
                                   (k * ntiles + t) * nwidth + nwidth],
                              vec_in[:, k:k + 1],
                              start=(k == 0), stop=(k == ktiles - 1))
                  for t in range(ntiles):
                      nc.scalar.activation(out_r[:, t:t + 1], ps[:, t:t + 1],
                                           act, bias=bias[:, t:t + 1])
                  return out_r

              mlp_tmp = []
              h1 = mlp_layer(featg, fe1, 1, 8, b1, AF.Silu)
              h2 = mlp_layer(h1, fe2, 8, 2, b2, AF.Identity)
              h3 = mlp_layer(h2, r1w, 2, 4, rb1, AF.Silu)
              h4 = mlp_layer(h3, r2w, 4, 1, rb2, AF.Silu)
              lps = ps_m.tile([1, 1], dt.float32, tag="mlp")
              nc.tensor.matmul(lps[:], r3w[:], h4[:], start=True, stop=True)
              logits = p_mlp.tile([1, 1], dt.float32)
              nc.scalar.activation(logits[:], lps[:], AF.Identity, bias=rb3[:])
              l1 = p_mlp.tile([1, 1], dt.float32)
              l2 = p_mlp.tile([1, 1], dt.float32)
              nc.scalar.activation(l1[:], noise[:], AF.Ln, bias=epsb[:])
              nc.scalar.activation(l2[:], l1[:], AF.Ln, bias=epsb[:], scale=-1.0)
              zin = p_mlp.tile([1, 1], dt.float32)
              nc.vector.tensor_tensor(zin[:], logits[:], l2[:], ALU.subtract)
              zsoft = p_mlp.tile([1, 1], dt.float32)
              nc.scalar.activation(zsoft[:], zin[:], AF.Sigmoid)
              zhard = p_mlp.tile([1, 1], dt.float32)
              nc.vector.tensor_scalar(zhard[:], zsoft[:], 0.5, None, ALU.is_gt)
              mps = ps_m.tile([128, 1], dt.float32, tag="mlp")
              nc.tensor.matmul(mps[:], ones_r[:], zhard[:], start=True, stop=True)
              mix = p_mlp.tile([128, 1], dt.float32)
              nc.scalar.copy(mix[:], mps[:])
              onem = p_mlp.tile([128, 1], dt.float32)
              nc.vector.tensor_scalar(onem[:], mix[:], -1.0, 1.0, ALU.mult,
                                      ALU.add)
              mlp_ctx.close()

              # --- attention ---
              for h in range(HLOC):
                  for g in range(NCH):
                      pT = p_pT.tile([128, NBLK * CH], dt.float32r, tag="pT")
                      for Ii in range(4):
                          I = 4 * g + Ii
                          nb_tot = (I + 1) * 128
                          e = p_e.tile([128, S], dt.float32, tag="e")
                          pband = p_p.tile([128, S], dt.float32r, tag="p")
                          sums = p_sm.tile([128, 16], dt.float32, tag="sums")
                          grps = _groups(I)
                          ng = len(grps)
                          for gi, (sb, nb) in enumerate(grps):
                              w = nb * 128
                              col = 4 - ng + gi
                              sc = ps_sc.tile([128, 512], dt.float32, tag="sc")
                              nc.tensor.matmul(
                                  sc[:, 0:w], qT[h][:, I * 128:(I + 1) * 128],
                                  kT[:, sb * 128: sb * 128 + w],
                                  start=True, stop=True)
                              if gi == ng - 1:
                                  nc.vector.tensor_tensor(
                                      sc[:, w - 128:w], sc[:, w - 128:w],
                                      diagneg[:], ALU.add)
                              nc.scalar.activation(
                                  e[:, sb * 128: sb * 128 + w], sc[:, 0:w],
                                  AF.Exp, scale=SCALE,
                                  accum_out=sums[:, col:col + 1])
                          if I >= 9:
                              tmask = p_sm.tile([128, 128], dt.float32,
                                                tag="tmask")
                              nc.vector.tensor_reduce(
                                  sums[:, 4:5], e[:, 0:128], AX.X, ALU.add)
                              nc.vector.tensor_tensor(
                                  tmask[:], e[:, (I - 8) * 128:(I - 7) * 128],
                                  triup[:], ALU.mult)
                              nc.vector.tensor_reduce(
                                  sums[:, 5:6], tmask[:], AX.X, ALU.add)
                              nc.vector.tensor_reduce(
                                  sums[:, 6:7], sums[:, 4 - ng:4], AX.X, ALU.add)
                              nc.vector.tensor_reduce(
                                  sums[:, 7:8], sums[:, 2:6], AX.X, ALU.add)
                              nc.vector.reciprocal(sums[:, 8:9], sums[:, 6:7])
                              nc.vector.reciprocal(sums[:, 9:10], sums[:, 7:8])
                              nc.vector.tensor_tensor(
                                  sums[:, 10:11], sums[:, 8:9], onem[:], ALU.mult)
                              nc.vector.tensor_tensor(
                                  sums[:, 11:12], sums[:, 9:10], mix[:], ALU.mult)
                              nc.vector.tensor_tensor(
                                  sums[:, 12:13], sums[:, 10:11], sums[:, 11:12],
                                  ALU.add)
                              a_ap = sums[:, 12:13]
                              b_ap = sums[:, 10:11]
                              amb_ap = sums[:, 11:12]
                              nc.vector.tensor_scalar(
                                  pband[:, 0:128], e[:, 0:128], a_ap, None,
                                  ALU.mult)
                              if I >= 10:
                                  nc.vector.tensor_scalar(
                                      pband[:, 128:(I - 8) * 128],
                                      e[:, 128:(I - 8) * 128], b_ap, None,
                                      ALU.mult)
                              nc.vector.tensor_scalar(
                                  pband[:, (I - 8) * 128:(I - 7) * 128],
                                  e[:, (I - 8) * 128:(I - 7) * 128], b_ap, None,
                                  ALU.mult)
                              nc.vector.scalar_tensor_tensor(
                                  pband[:, (I - 8) * 128:(I - 7) * 128],
                                  tmask[:], amb_ap,
                                  pband[:, (I - 8) * 128:(I - 7) * 128],
                                  ALU.mult, ALU.add)
                              nc.vector.tensor_scalar(
                                  pband[:, (I - 7) * 128:nb_tot],
                                  e[:, (I - 7) * 128:nb_tot], a_ap, None,
                                  ALU.mult)
                          else:
                              nc.vector.tensor_reduce(
                                  sums[:, 6:7], sums[:, 4 - ng:4], AX.X, ALU.add)
                              nc.vector.reciprocal(sums[:, 8:9], sums[:, 6:7])
                              nc.vector.tensor_scalar(
                                  pband[:, 0:nb_tot], e[:, 0:nb_tot],
                                  sums[:, 8:9], None, ALU.mult)
                          pT3 = pT[:, :].rearrange("p (J c) -> p J c", c=CH)
                          for J0 in range(0, I + 1, 4):
                              nb4 = min(4, I + 1 - J0)
                              ptp = ps_pt.tile([128, 512], dt.float32r, tag="pt")
                              for jj in range(nb4):
                                  nc.tensor.transpose(
                                      ptp[:, jj * 128:(jj + 1) * 128],
                                      pband[:, (J0 + jj) * 128:
                                            (J0 + jj + 1) * 128],
                                      ident[:])
                              nc.any.tensor_copy(
                                  pT3[:, J0:J0 + nb4,
                                      Ii * 128:(Ii + 1) * 128],
                                  ptp[:, 0:nb4 * 128].rearrange(
                                      "p (J c) -> p J c", c=128))
                      # PV for this (h, chunk)
                      ops = ps_o.tile([128, CH], dt.float32, tag="o")
                      last_J = 4 * g + 3
                      for J in range(last_J + 1):
                          k = J - 4 * g
                          if k <= 0:
                              nc.tensor.matmul(
                                  ops[:], vN[:, J * 128:(J + 1) * 128],
                                  pT[:, J * CH:(J + 1) * CH],
                                  start=(J == 0), stop=(J == last_J))
                          else:
                              nc.tensor.matmul(
                                  ops[:, k * 128:CH],
                                  vN[:, J * 128:(J + 1) * 128],
                                  pT[:, J * CH + k * 128:(J + 1) * CH],
                                  start=False, stop=(J == last_J))
                      osb = p_ob.tile([128, CH], dt.bfloat16, tag="osb")
                      nc.scalar.copy(osb[:], ops[:])
                      a2a_in_h = a2a_in0 if h < 2 else a2a_in1
                      hh = h % 2
                      nc.sync.dma_start(
                          a2a_in_h[2 * g, hh * 128:(hh + 1) * 128, :],
                          osb[:, 0:ROWS])
                      nc.sync.dma_start(
                          a2a_in_h[2 * g + 1, hh * 128:(hh + 1) * 128, :],
                          osb[:, ROWS:CH])
                  if h == 1:
                      nc.gpsimd.collective_compute(
                          "AllToAll", ALU.bypass,
                          replica_groups=[list(range(NCORES))],
                          ins=[a2a_in0.opt()], outs=[a2a_out0.opt()])
                  if h == 3:
                      nc.gpsimd.collective_compute(
                          "AllToAll", ALU.bypass,
                          replica_groups=[list(range(NCORES))],
                          ins=[a2a_in1.opt()], outs=[a2a_out1.opt()])

        # ---- phase 3: AllToAll + output projection ----------------------
        with ExitStack() as ph3:
          if KNOBS["phases"] >= 3:
              p_oT = ph3.enter_context(tc.tile_pool(name="oT", bufs=1))
              p_wo = ph3.enter_context(tc.tile_pool(name="wo", bufs=KNOBS["wo_bufs"]))
              p_os = ph3.enter_context(tc.tile_pool(name="outsb", bufs=2))
              ps_w = ph3.enter_context(
                  tc.tile_pool(name="ps_w", bufs=2, space="PSUM"))

              KT_ORDER = [4 * p + t for t in (0, 1, 2, 3) for p in range(NCORES)]
              oTb = p_oT.tile([128, KT * ROWS], dt.bfloat16, name="oTb",
                              tag="oTb")
              for kt in KT_ORDER:
                  p, t = kt // HLOC, kt % HLOC
                  src = a2a_out0 if t < 2 else a2a_out1
                  tt = t % 2
                  nc.sync.dma_start(
                      oTb[:, kt * ROWS:(kt + 1) * ROWS],
                      src[p, tt * 128:(tt + 1) * 128, :])

              for ngi in range(8):
                  pso = [ps_w.tile([128, 512], dt.float32, tag=f"wo{st}",
                                  name=f"wo{st}")
                         for st in range(2)]
                  for b4 in range(KT // 4):
                      wsl = p_wo.tile([128, 4, 512], dt.bfloat16, tag="wo")
                      nc.sync.dma_start(
                          wsl[:], wo_d[b4 * 512:(b4 + 1) * 512,
                                       ngi * 512:(ngi + 1) * 512].rearrange(
                              "(k p) n -> p k n", p=128))
                      for kk in range(4):
                          kt = 4 * b4 + kk
                          for st in range(2):
                              nc.tensor.matmul(
                                  pso[st][:],
                                  oTb[:, kt * ROWS + st * 128:
                                      kt * ROWS + (st + 1) * 128],
                                  wsl[:, kk, :], start=(kt == 0),
                                  stop=(kt == KT - 1))
                  for st in range(2):
                      osb = p_os.tile([128, 512], dt.float32, tag="os")
                      nc.scalar.copy(osb[:], pso[st][:])
                      nc.sync.dma_start(
                          out_d[st * 128:(st + 1) * 128,
                                ngi * 512:(ngi + 1) * 512], osb[:])

    nc.compile()
    return nc


_CACHE = {}


def _host_constants():
    inv = 10000.0 ** (-np.arange(0, D, 2, dtype=np.float64) / D)
    t = np.arange(S, dtype=np.float64)
    fr = np.outer(t, inv)                      # [S, 64]
    cos = np.cos(fr).T.astype(np.float32)      # [64, S]
    sin = np.sin(fr).T.astype(np.float32)
    cos2 = np.vstack([cos, cos])
    sin2 = np.vstack([-sin, sin])
    ident = np.eye(128, dtype=np.float32)
    a = np.arange(128)
    diagneg = np.where(a[None, :] <= a[:, None], 0.0, NEG).astype(np.float32)
    triup = (a[None, :] > a[:, None]).astype(np.float32)
    ones = np.ones((1, 128), dtype=np.float32)
    return cos2, sin2, ident, diagneg, triup, ones


def kernel(hidden_states, Wq, Wk, Wv, Wo, fe1_w, fe1_b, fe2_w, fe2_b,
           r1_w, r1_b, r2_w, r2_b, r3_w, r3_b, router_noise):
    if "nc" not in _CACHE:
        _CACHE["nc"] = build()
    nc = _CACHE["nc"]

    import ml_dtypes
    bf16 = ml_dtypes.bfloat16
    hsT = np.ascontiguousarray(
        np.asarray(hidden_states, dtype=np.float32).reshape(S, HID).T
    ).astype(bf16)
    Wq = np.asarray(Wq, np.float32)
    Wk = np.asarray(Wk, np.float32)
    Wv = np.asarray(Wv, np.float32)
    Wo = np.ascontiguousarray(np.asarray(Wo, np.float32)).astype(bf16)
    cos2, sin2, ident, diagneg, triup, ones = _host_constants()

    def ktile_cols(w, ktiles, ntiles, nwidth):
        # [K, N] -> [128, ktiles*ntiles*nwidth] with (k, t) slab layout
        return np.ascontiguousarray(
              np.concatenate([w[k * 128:(k + 1) * 128, :] for k in range(ktiles)],
                             axis=1))

    fe1 = np.asarray(fe1_w, np.float32)                       # [128,1024]
    fe2 = ktile_cols(np.asarray(fe2_w, np.float32), 8, 2, 128)
    r1 = ktile_cols(np.asarray(r1_w, np.float32), 2, 4, 128)
    r2 = ktile_cols(np.asarray(r2_w, np.float32), 4, 1, 128)
    r3 = np.asarray(r3_w, np.float32)                         # [128,1]
    b1 = np.asarray(fe1_b, np.float32).reshape(8, 128).T.copy()
    b2 = np.asarray(fe2_b, np.float32).reshape(2, 128).T.copy()
    rb1 = np.asarray(r1_b, np.float32).reshape(4, 128).T.copy()
    rb2 = np.asarray(r2_b, np.float32).reshape(1, 128).T.copy()
    rb3 = np.asarray(r3_b, np.float32).reshape(1, 1)
    noise = np.asarray(router_noise, np.float32).reshape(1, 1)

    in_maps = []
    for c in range(NCORES):
        wqkv = np.ascontiguousarray(np.concatenate(
              [Wq[:, c * 512:(c + 1) * 512],
               Wk[:, c * 128:(c + 1) * 128],
               Wv[:, c * 128:(c + 1) * 128]], axis=1)).astype(bf16)
        in_maps.append(dict(
              hsT=hsT, wqkv=wqkv, wo=Wo, cos2=cos2, sin2=sin2, ident=ident,
              diagneg=diagneg, triup=triup, ones=ones, fe1=fe1, fe2=fe2,
              r1=r1, r2=r2, r3=r3, b1=b1, b2=b2, rb1=rb1, rb2=rb2, rb3=rb3,
              noise=noise, eps=np.full((1, 1), 1e-8, np.float32)))

    res = run_bass_kernel_spmd(nc, in_maps, list(range(NCORES)))
    out = np.concatenate([res.results[c]["out_rows"] for c in range(NCORES)],
                           axis=0)
    return out.reshape(1, S, HID).astype(np.float32)



# revision 50
# speedup vs baseline: 1.4202x; 1.0618x over previous
"""Trainium2 Bass kernel for nn_LlamaAttention_61899068670751.

Sparse (streaming-LLM) attention layer, sharded tensor-parallel over heads
across 8 NeuronCores:
  - core c owns q-heads [4c..4c+3] and kv-head c (GQA group = 4)
  - QKV projections computed per-core with column-sharded weights
  - causal scores computed once per head; both softmax branches (full causal
    and sink+sliding-window) share exp(s) and are folded into a single PV
    matmul via per-row/per-region coefficients
  - the router MLP is computed redundantly on every core after a [128]
    feature AllReduce; its hard gate enters the coefficients
  - o is exchanged with an AllToAll so each core computes 256 rows of the
    final o @ Wo with the full head dimension; host concatenates row shards

All matmuls run as float32r (full-rate fp32 on the PE array).
"""
import numpy as np
from contextlib import ExitStack

import concourse.bacc as bacc
import concourse.mybir as mybir
import concourse.tile as tile
from concourse.bass_utils import run_bass_kernel_spmd

dt = mybir.dt
AF = mybir.ActivationFunctionType
ALU = mybir.AluOpType
AX = mybir.AxisListType

NCORES = 8
S, H, KV, D, HID = 2048, 32, 8, 128, 4096
SINK, WIN, POOL = 128, 1024, 100
HLOC = H // NCORES          # 4 q heads per core
NBLK = S // 128             # 16 row/col blocks
NCH = 4                     # s-chunks of 512
CH = 512
KT = HID // 128             # 32 contraction tiles
SCALE = 1.0 / float(np.sqrt(D))
NEG = -1.0e30
ROWS = S // NCORES          # 256 output rows per core


def _groups(I):
    """Right-aligned 4-block groups over causal blocks 0..I."""
    n = I + 1
    lo = n % 4
    g = [(0, lo)] if lo else []
    g += [(s, 4) for s in range(lo, n, 4)]
    return g


KNOBS = dict(phases=3, wo_bufs=16, w_bufs=4, hs_bufs=1, sc_bufs=3, pt_bufs=2,
             e_bufs=3, p_bufs=3, pT_bufs=2, o_bufs=1, tr_bufs=2)


def build():
    nc = bacc.Bacc("TRN2", target_bir_lowering=False, debug=False,
                   num_devices=NCORES)

    def din(name, shape, d=dt.float32r):
        return nc.dram_tensor(name, shape, d, kind="ExternalInput").ap()

    hsT_d = din("hsT", [HID, S], dt.bfloat16)
    wqkv_d = din("wqkv", [HID, 768], dt.bfloat16)
    wo_d = din("wo", [HID, HID], dt.bfloat16)
    cos2_d = din("cos2", [128, S], dt.float32)
    sin2_d = din("sin2", [128, S], dt.float32)
    ident_d = din("ident", [128, 128])
    diagneg_d = din("diagneg", [128, 128], dt.float32)
    triup_d = din("triup", [128, 128], dt.float32)
    ones_d = din("ones", [1, 128], dt.float32)
    fe1_d = din("fe1", [128, 1024], dt.float32)
    fe2_d = din("fe2", [128, 8 * 256], dt.float32)
    r1_d = din("r1", [128, 2 * 512], dt.float32)
    r2_d = din("r2", [128, 4 * 128], dt.float32)
    r3_d = din("r3", [128, 1], dt.float32)
    b1_d = din("b1", [128, 8], dt.float32)
    b2_d = din("b2", [128, 2], dt.float32)
    rb1_d = din("rb1", [128, 4], dt.float32)
    rb2_d = din("rb2", [128, 1], dt.float32)
    rb3_d = din("rb3", [1, 1], dt.float32)
    noise_d = din("noise", [1, 1], dt.float32)
    eps_d = din("eps", [1, 1], dt.float32)

    out_d = nc.dram_tensor("out_rows", [ROWS, HID], dt.float32,
                           kind="ExternalOutput").ap()

    with tile.TileContext(nc) as tc, ExitStack() as top:
        # ---- long-lived pools -------------------------------------------
        const = top.enter_context(tc.tile_pool(name="const", bufs=1))
        persist = top.enter_context(tc.tile_pool(name="persist", bufs=1))
        dram = top.enter_context(tc.tile_pool(name="dram", bufs=1, space="DRAM"))

        ident = const.tile([128, 128], dt.float32r)
        diagneg = const.tile([128, 128], dt.float32)
        triup = const.tile([128, 128], dt.float32)
        ones_r = const.tile([1, 128], dt.float32)
        nc.sync.dma_start(ident[:], ident_d[:])
        nc.sync.dma_start(diagneg[:], diagneg_d[:])
        nc.sync.dma_start(triup[:], triup_d[:])
        nc.sync.dma_start(ones_r[:], ones_d[:])

        qT = [persist.tile([128, S], dt.float32r, name=f"qT{h}", tag=f"qT{h}")
              for h in range(HLOC)]
        kT = persist.tile([128, S], dt.float32r)
        vN = persist.tile([128, NBLK * 128], dt.float32r)   # v natural, per block

        # collective bounce buffers
        a2a_in0 = dram.tile([NCORES, 2 * 128, ROWS], dt.bfloat16)
        a2a_out0 = dram.tile([NCORES, 2 * 128, ROWS], dt.bfloat16)
        a2a_in1 = dram.tile([NCORES, 2 * 128, ROWS], dt.bfloat16)
        a2a_out1 = dram.tile([NCORES, 2 * 128, ROWS], dt.bfloat16)
        cc_in = dram.tile([128, 1], dt.float32)
        cc_out = dram.tile([128, 1], dt.float32, addr_space="Shared")

        # ---- phase 1: hs transpose + QKV projections + rope -------------
        with ExitStack() as ph1:
            p_hs = ph1.enter_context(tc.tile_pool(name="hs", bufs=KNOBS["hs_bufs"]))
            p_hsT = ph1.enter_context(tc.tile_pool(name="hsT", bufs=2))
            p_w = ph1.enter_context(tc.tile_pool(name="wslab", bufs=KNOBS["w_bufs"]))
            p_rope = ph1.enter_context(tc.tile_pool(name="rope", bufs=2))
            p_cs = ph1.enter_context(tc.tile_pool(name="cs", bufs=2))
            ps_tr = ph1.enter_context(
                tc.tile_pool(name="ps_tr", bufs=KNOBS["tr_bufs"], space="PSUM"))
            ps_acc = ph1.enter_context(
                tc.tile_pool(name="ps_acc", bufs=1, space="PSUM"))

            KH = KT // 2        # 16 k-tiles per half
            for g in range(NCH):
                s0 = g * CH
                accs = [ps_acc.tile([128, CH], dt.float32, tag=f"acc{i}",
                                    name=f"acc{i}")
                        for i in range(6)]
                for half in range(2):
                    k0 = half * KH
                    hsT = p_hsT.tile([128, KH, CH], dt.bfloat16, tag="hsT")
                    nc.sync.dma_start(
                        hsT[:], hsT_d[k0 * 128:(k0 + KH) * 128,
                                      s0:s0 + CH].rearrange(
                            "(k p) n -> p k n", p=128))
                    for k4 in range(KH // 4):
                        wsl = p_w.tile([128, 4, 768], dt.bfloat16, tag="w")
                        kb = k0 + 4 * k4
                        nc.sync.dma_start(
                            wsl[:], wqkv_d[kb * 128:(kb + 4) * 128,
                                           :].rearrange(
                                "(k p) n -> p k n", p=128))
                        for kk in range(4):
                            kt = kb + kk
                            for i in range(6):
                                nc.tensor.matmul(
                                    accs[i][:],
                                    wsl[:, kk, i * 128:(i + 1) * 128],
                                    hsT[:, 4 * k4 + kk, :],
                                    start=(kt == 0), stop=(kt == KT - 1))

                # rope for q heads (0..3) and k (4)
                cos_sl = p_cs.tile([128, CH], dt.float32, tag="cos")
                sin_sl = p_cs.tile([128, CH], dt.float32, tag="sin")
                nc.sync.dma_start(cos_sl[:], cos2_d[:, s0:s0 + CH])
                nc.sync.dma_start(sin_sl[:], sin2_d[:, s0:s0 + CH])
                for i in range(5):
                    dest = qT[i] if i < HLOC else kT
                    lin = p_rope.tile([128, CH], dt.float32, tag="lin")
                    rot = p_rope.tile([128, CH], dt.float32, tag="rot")
                    t1 = p_rope.tile([128, CH], dt.float32, tag="t1")
                    t2 = p_rope.tile([128, CH], dt.float32, tag="t2")
                    nc.scalar.copy(lin[:], accs[i][:])
                    nc.sync.dma_start(rot[0:64, :], lin[64:128, :])
                    nc.sync.dma_start(rot[64:128, :], lin[0:64, :])
                    nc.vector.tensor_tensor(t1[:], lin[:], cos_sl[:], ALU.mult)
                    nc.vector.tensor_tensor(t2[:], rot[:], sin_sl[:], ALU.mult)
                    nc.vector.tensor_tensor(dest[:, s0:s0 + CH], t1[:], t2[:],
                                            ALU.add)
                # v: copy then transpose to natural layout
                vT = p_rope.tile([128, CH], dt.float32r, tag="vT")
                nc.scalar.copy(vT[:], accs[5][:])
                for ss in range(4):
                    ptr = ps_tr.tile([128, 128], dt.float32r, tag="tr")
                    nc.tensor.transpose(ptr[:], vT[:, ss * 128:(ss + 1) * 128],
                                        ident[:])
                    nc.any.tensor_copy(
                        vN[:, (g * 4 + ss) * 128:(g * 4 + ss + 1) * 128], ptr[:])

        # ---- phase 2: router + attention --------------------------------
        with ExitStack() as ph2:
          if KNOBS["phases"] >= 2:
              p_mlp = ph2.enter_context(tc.tile_pool(name="mlp", bufs=1))
              p_e = ph2.enter_context(tc.tile_pool(name="eband", bufs=KNOBS["e_bufs"]))
              p_p = ph2.enter_context(tc.tile_pool(name="pband", bufs=KNOBS["p_bufs"]))
              p_pT = ph2.enter_context(tc.tile_pool(name="pT", bufs=KNOBS["pT_bufs"]))
              p_sm = ph2.enter_context(tc.tile_pool(name="sums", bufs=4))
              p_ob = ph2.enter_context(tc.tile_pool(name="obuf", bufs=2))
              ps_sc = ph2.enter_context(
                  tc.tile_pool(name="ps_sc", bufs=KNOBS["sc_bufs"], space="PSUM"))
              ps_pt = ph2.enter_context(
                  tc.tile_pool(name="ps_pt", bufs=KNOBS["pt_bufs"], space="PSUM"))
              ps_o = ph2.enter_context(
                  tc.tile_pool(name="ps_o", bufs=KNOBS["o_bufs"], space="PSUM"))
              mlp_ctx = ExitStack()
              ps_m = mlp_ctx.enter_context(
                  tc.tile_pool(name="ps_m", bufs=1, space="PSUM"))

              # --- router ---
              feat_acc = p_mlp.tile([128, 8], dt.float32)
              for h in range(HLOC):
                  nc.vector.tensor_reduce(feat_acc[:, h:h + 1],
                                          qT[h][:, 0:POOL], AX.X, ALU.add)
                  nc.vector.tensor_reduce(feat_acc[:, 4 + h:5 + h],
                                          qT[h][:, S - POOL:S], AX.X, ALU.add)
              feat_s = p_mlp.tile([128, 1], dt.float32)
              nc.vector.tensor_reduce(feat_s[:], feat_acc[:], AX.X, ALU.add)
              feat_r = p_mlp.tile([128, 1], dt.float32)
              nc.scalar.activation(feat_r[:], feat_s[:], AF.Copy,
                                   scale=1.0 / (2 * POOL * H))
              nc.sync.dma_start(cc_in[:], feat_r[:])
              nc.gpsimd.collective_compute(
                  "AllReduce", ALU.add,
                  replica_groups=[list(range(NCORES))],
                  ins=[cc_in.opt()], outs=[cc_out.opt()])
              featg = p_mlp.tile([128, 1], dt.float32)
              nc.sync.dma_start(featg[:], cc_out[:])

              # MLP weights
              fe1 = p_mlp.tile([128, 1024], dt.float32)
              fe2 = p_mlp.tile([128, 8 * 256], dt.float32)
              r1w = p_mlp.tile([128, 2 * 512], dt.float32)
              r2w = p_mlp.tile([128, 4 * 128], dt.float32)
              r3w = p_mlp.tile([128, 1], dt.float32)
              b1 = p_mlp.tile([128, 8], dt.float32)
              b2 = p_mlp.tile([128, 2], dt.float32)
              rb1 = p_mlp.tile([128, 4], dt.float32)
              rb2 = p_mlp.tile([128, 1], dt.float32)
              rb3 = p_mlp.tile([1, 1], dt.float32)
              noise = p_mlp.tile([1, 1], dt.float32)
              epsb = p_mlp.tile([1, 1], dt.float32)
              nc.sync.dma_start(epsb[:], eps_d[:])
              for t_, d_ in ((fe1, fe1_d), (fe2, fe2_d), (r1w, r1_d),
                             (r2w, r2_d), (r3w, r3_d), (b1, b1_d), (b2, b2_d),
                             (rb1, rb1_d), (rb2, rb2_d), (rb3, rb3_d),
                             (noise, noise_d)):
                  nc.sync.dma_start(t_[:], d_[:])

              def mlp_layer(vec_in, w_sb, ktiles, ntiles, bias, act, nwidth=128):
                  """vec_in: [128, ktiles] fp32r columns; returns [128, ntiles]."""
                  out_r = p_mlp.tile([128, max(ntiles, 1)], dt.float32,
                                     name=f"mlpv{len(mlp_tmp)}")
                  mlp_tmp.append(out_r)
                  ps = ps_m.tile([128, max(ntiles, 1)], dt.float32, tag="mlp",
                               name="mlpps")
                  for t in range(ntiles):
                      for k in range(ktiles):
                          nc.tensor.matmul(
                              ps[:, t:t + 1],
                              w_sb[:, (k * ntiles + t) * nwidth:
                                   (k * ntiles + t) * nwidth + nwidth],
                              vec_in[:, k:k + 1],
                              start=(k == 0), stop=(k == ktiles - 1))
                  for t in range(ntiles):
                      nc.scalar.activation(out_r[:, t:t + 1], ps[:, t:t + 1],
                                           act, bias=bias[:, t:t + 1])
                  return out_r

              mlp_tmp = []
              h1 = mlp_layer(featg, fe1, 1, 8, b1, AF.Silu)
              h2 = mlp_layer(h1, fe2, 8, 2, b2, AF.Identity)
              h3 = mlp_layer(h2, r1w, 2, 4, rb1, AF.Silu)
              h4 = mlp_layer(h3, r2w, 4, 1, rb2, AF.Silu)
              lps = ps_m.tile([1, 1], dt.float32, tag="mlp")
              nc.tensor.matmul(lps[:], r3w[:], h4[:], start=True, stop=True)
              logits = p_mlp.tile([1, 1], dt.float32)
              nc.scalar.activation(logits[:], lps[:], AF.Identity, bias=rb3[:])
              l1 = p_mlp.tile([1, 1], dt.float32)
              l2 = p_mlp.tile([1, 1], dt.float32)
              nc.scalar.activation(l1[:], noise[:], AF.Ln, bias=epsb[:])
              nc.scalar.activation(l2[:], l1[:], AF.Ln, bias=epsb[:], scale=-1.0)
              zin = p_mlp.tile([1, 1], dt.float32)
              nc.vector.tensor_tensor(zin[:], logits[:], l2[:], ALU.subtract)
              zsoft = p_mlp.tile([1, 1], dt.float32)
              nc.scalar.activation(zsoft[:], zin[:], AF.Sigmoid)
              zhard = p_mlp.tile([1, 1], dt.float32)
              nc.vector.tensor_scalar(zhard[:], zsoft[:], 0.5, None, ALU.is_gt)
              mps = ps_m.tile([128, 1], dt.float32, tag="mlp")
              nc.tensor.matmul(mps[:], ones_r[:], zhard[:], start=True, stop=True)
              mix = p_mlp.tile([128, 1], dt.float32)
              nc.scalar.copy(mix[:], mps[:])
              onem = p_mlp.tile([128, 1], dt.float32)
              nc.vector.tensor_scalar(onem[:], mix[:], -1.0, 1.0, ALU.mult,
                                      ALU.add)
              mlp_ctx.close()

              # --- attention ---
              for h in range(HLOC):
                  for g in range(NCH):
                      pT = p_pT.tile([128, NBLK * CH], dt.float32r, tag="pT")
                      for Ii in range(4):
                          I = 4 * g + Ii
                          nb_tot = (I + 1) * 128
                          e = p_e.tile([128, S], dt.float32, tag="e")
                          pband = p_p.tile([128, S], dt.float32r, tag="p")
                          sums = p_sm.tile([128, 16], dt.float32, tag="sums")
                          grps = _groups(I)
                          ng = len(grps)
                          for gi, (sb, nb) in enumerate(grps):
                              w = nb * 128
                              col = 4 - ng + gi
                              sc = ps_sc.tile([128, 512], dt.float32, tag="sc")
                              nc.tensor.matmul(
                                  sc[:, 0:w], qT[h][:, I * 128:(I + 1) * 128],
                                  kT[:, sb * 128: sb * 128 + w],
                                  start=True, stop=True)
                              if gi == ng - 1:
                                  nc.vector.tensor_tensor(
                                      sc[:, w - 128:w], sc[:, w - 128:w],
                                      diagneg[:], ALU.add)
                              nc.scalar.activation(
                                  e[:, sb * 128: sb * 128 + w], sc[:, 0:w],
                                  AF.Exp, scale=SCALE,
                                  accum_out=sums[:, col:col + 1])
                          if I >= 9:
                              tmask = p_sm.tile([128, 128], dt.float32,
                                                tag="tmask")
                              nc.vector.tensor_reduce(
                                  sums[:, 4:5], e[:, 0:128], AX.X, ALU.add)
                              nc.vector.tensor_tensor(
                                  tmask[:], e[:, (I - 8) * 128:(I - 7) * 128],
                                  triup[:], ALU.mult)
                              nc.vector.tensor_reduce(
                                  sums[:, 5:6], tmask[:], AX.X, ALU.add)
                              nc.vector.tensor_reduce(
                                  sums[:, 6:7], sums[:, 4 - ng:4], AX.X, ALU.add)
                              nc.vector.tensor_reduce(
                                  sums[:, 7:8], sums[:, 2:6], AX.X, ALU.add)
                              nc.vector.reciprocal(sums[:, 8:9], sums[:, 6:7])
                              nc.vector.reciprocal(sums[:, 9:10], sums[:, 7:8])
                              nc.vector.tensor_tensor(
                                  sums[:, 10:11], sums[:, 8:9], onem[:], ALU.mult)
                              nc.vector.tensor_tensor(
                                  sums[:, 11:12], sums[:, 9:10], mix[:], ALU.mult)
                              nc.vector.tensor_tensor(
                                  sums[:, 12:13], sums[:, 10:11], sums[:, 11:12],
                                  ALU.add)
                              a_ap = sums[:, 12:13]
                              b_ap = sums[:, 10:11]
                              amb_ap = sums[:, 11:12]
                              nc.vector.tensor_scalar(
                                  pband[:, 0:128], e[:, 0:128], a_ap, None,
                                  ALU.mult)
                              if I >= 10:
                                  nc.vector.tensor_scalar(
                                      pband[:, 128:(I - 8) * 128],
                                      e[:, 128:(I - 8) * 128], b_ap, None,
                                      ALU.mult)
                              nc.vector.tensor_scalar(
                                  pband[:, (I - 8) * 128:(I - 7) * 128],
                                  e[:, (I - 8) * 128:(I - 7) * 128], b_ap, None,
                                  ALU.mult)
                              nc.vector.scalar_tensor_tensor(
                                  pband[:, (I - 8) * 128:(I - 7) * 128],
                                  tmask[:], amb_ap,
                                  pband[:, (I - 8) * 128:(I - 7) * 128],
                                  ALU.mult, ALU.add)
                              nc.vector.tensor_scalar(
                                  pband[:, (I - 7) * 128:nb_tot],
                                  e[:, (I - 7) * 128:nb_tot], a_ap, None,
                                  ALU.mult)
                          else:
                              nc.vector.tensor_reduce(
                                  sums[:, 6:7], sums[:, 4 - ng:4], AX.X, ALU.add)
                              nc.vector.reciprocal(sums[:, 8:9], sums[:, 6:7])
                              nc.vector.tensor_scalar(
                                  pband[:, 0:nb_tot], e[:, 0:nb_tot],
                                  sums[:, 8:9], None, ALU.mult)
                          pT3 = pT[:, :].rearrange("p (J c) -> p J c", c=CH)
                          for J0 in range(0, I + 1, 4):
                              nb4 = min(4, I + 1 - J0)
                              ptp = ps_pt.tile([128, 512], dt.float32r, tag="pt")
                              for jj in range(nb4):
                                  nc.tensor.transpose(
                                      ptp[:, jj * 128:(jj + 1) * 128],
                                      pband[:, (J0 + jj) * 128:
                                            (J0 + jj + 1) * 128],
                                      ident[:])
                              nc.any.tensor_copy(
                                  pT3[:, J0:J0 + nb4,
                                      Ii * 128:(Ii + 1) * 128],
                                  ptp[:, 0:nb4 * 128].rearrange(
                                      "p (J c) -> p J c", c=128))
                      # PV for this (h, chunk)
                      ops = ps_o.tile([128, CH], dt.float32, tag="o")
                      last_J = 4 * g + 3
                      for J in range(last_J + 1):
                          k = J - 4 * g
                          if k <= 0:
                              nc.tensor.matmul(
                                  ops[:], vN[:, J * 128:(J + 1) * 128],
                                  pT[:, J * CH:(J + 1) * CH],
                                  start=(J == 0), stop=(J == last_J))
                          else:
                              nc.tensor.matmul(
                                  ops[:, k * 128:CH],
                                  vN[:, J * 128:(J + 1) * 128],
                                  pT[:, J * CH + k * 128:(J + 1) * CH],
                                  start=False, stop=(J == last_J))
                      osb = p_ob.tile([128, CH], dt.bfloat16, tag="osb")
                      nc.scalar.copy(osb[:], ops[:])
                      a2a_in_h = a2a_in0 if h < 2 else a2a_in1
                      hh = h % 2
                      nc.sync.dma_start(
                          a2a_in_h[2 * g, hh * 128:(hh + 1) * 128, :],
                          osb[:, 0:ROWS])
                      nc.sync.dma_start(
                          a2a_in_h[2 * g + 1, hh * 128:(hh + 1) * 128, :],
                          osb[:, ROWS:CH])
                  if h == 1:
                      nc.gpsimd.collective_compute(
                          "AllToAll", ALU.bypass,
                          replica_groups=[list(range(NCORES))],
                          ins=[a2a_in0.opt()], outs=[a2a_out0.opt()])
                  if h == 3:
                      nc.gpsimd.collective_compute(
                          "AllToAll", ALU.bypass,
                          replica_groups=[list(range(NCORES))],
                          ins=[a2a_in1.opt()], outs=[a2a_out1.opt()])

        # ---- phase 3: AllToAll + output projection ----------------------
        with ExitStack() as ph3:
          if KNOBS["phases"] >= 3:
              p_oT = ph3.enter_context(tc.tile_pool(name="oT", bufs=1))
              p_part = ph3.enter_context(tc.tile_pool(name="part", bufs=1))
              p_wo = ph3.enter_context(tc.tile_pool(name="wo", bufs=KNOBS["wo_bufs"]))
              p_os = ph3.enter_context(tc.tile_pool(name="outsb", bufs=2))
              ps_w = ph3.enter_context(
                  tc.tile_pool(name="ps_w", bufs=2, space="PSUM"))

              # two 16-ktile halves: half 0 (heads 0-1, from a2a_out0)
              # overlaps AllToAll#2; half 1 adds the partials and stores.
              oTb = p_oT.tile([128, KT * ROWS], dt.bfloat16, name="oTb",
                              tag="oTb")
              parts = [p_part.tile([128, 512], dt.float32, name=f"part{i}",
                                   tag=f"part{i}") for i in range(16)]
              for half in range(2):
                  buf = a2a_out0 if half == 0 else a2a_out1
                  for p in range(NCORES):
                      for tt in range(2):
                          kt = 4 * p + 2 * half + tt
                          nc.sync.dma_start(
                              oTb[:, kt * ROWS:(kt + 1) * ROWS],
                              buf[p, tt * 128:(tt + 1) * 128, :])
                  for ngi in range(8):
                      pso = [ps_w.tile([128, 512], dt.float32, tag=f"wo{st}",
                                      name=f"wo{st}")
                             for st in range(2)]
                      for b2 in range(NCORES):
                          r0 = (4 * b2 + 2 * half) * 128
                          wsl = p_wo.tile([128, 2, 512], dt.bfloat16,
                                          tag="wo")
                          nc.sync.dma_start(
                              wsl[:], wo_d[r0:r0 + 256,
                                           ngi * 512:(ngi + 1) * 512
                                           ].rearrange(
                                  "(k p) n -> p k n", p=128))
                          for kk in range(2):
                              kt = 4 * b2 + 2 * half + kk
                              ki = 2 * b2 + kk
                              for st in range(2):
                                  nc.tensor.matmul(
                                      pso[st][:],
                                      oTb[:, kt * ROWS + st * 128:
                                          kt * ROWS + (st + 1) * 128],
                                      wsl[:, kk, :], start=(ki == 0),
                                      stop=(ki == 15))
                      for st in range(2):
                          if half == 0:
                              nc.scalar.copy(parts[2 * ngi + st][:],
                                             pso[st][:])
                          else:
                              osb = p_os.tile([128, 512], dt.float32,
                                              tag="os")
                              nc.vector.tensor_tensor(
                                  osb[:], pso[st][:],
                                  parts[2 * ngi + st][:], ALU.add)
                              nc.sync.dma_start(
                                  out_d[st * 128:(st + 1) * 128,
                                        ngi * 512:(ngi + 1) * 512], osb[:])

    nc.compile()
    return nc


_CACHE = {}


def _host_constants():
    inv = 10000.0 ** (-np.arange(0, D, 2, dtype=np.float64) / D)
    t = np.arange(S, dtype=np.float64)
    fr = np.outer(t, inv)                      # [S, 64]
    cos = np.cos(fr).T.astype(np.float32)      # [64, S]
    sin = np.sin(fr).T.astype(np.float32)
    cos2 = np.vstack([cos, cos])
    sin2 = np.vstack([-sin, sin])
    ident = np.eye(128, dtype=np.float32)
    a = np.arange(128)
    diagneg = np.where(a[None, :] <= a[:, None], 0.0, NEG).astype(np.float32)
    triup = (a[None, :] > a[:, None]).astype(np.float32)
    ones = np.ones((1, 128), dtype=np.float32)
    return cos2, sin2, ident, diagneg, triup, ones


def kernel(hidden_states, Wq, Wk, Wv, Wo, fe1_w, fe1_b, fe2_w, fe2_b,
           r1_w, r1_b, r2_w, r2_b, r3_w, r3_b, router_noise):
    if "nc" not in _CACHE:
        _CACHE["nc"] = build()
    nc = _CACHE["nc"]

    import ml_dtypes
    bf16 = ml_dtypes.bfloat16
    hsT = np.ascontiguousarray(
        np.asarray(hidden_states, dtype=np.float32).reshape(S, HID).T
    ).astype(bf16)
    Wq = np.asarray(Wq, np.float32)
    Wk = np.asarray(Wk, np.float32)
    Wv = np.asarray(Wv, np.float32)
    Wo = np.ascontiguousarray(np.asarray(Wo, np.float32)).astype(bf16)
    cos2, sin2, ident, diagneg, triup, ones = _host_constants()

    def ktile_cols(w, ktiles, ntiles, nwidth):
        # [K, N] -> [128, ktiles*ntiles*nwidth] with (k, t) slab layout
        return np.ascontiguousarray(
              np.concatenate([w[k * 128:(k + 1) * 128, :] for k in range(ktiles)],
                             axis=1))

    fe1 = np.asarray(fe1_w, np.float32)                       # [128,1024]
    fe2 = ktile_cols(np.asarray(fe2_w, np.float32), 8, 2, 128)
    r1 = ktile_cols(np.asarray(r1_w, np.float32), 2, 4, 128)
    r2 = ktile_cols(np.asarray(r2_w, np.float32), 4, 1, 128)
    r3 = np.asarray(r3_w, np.float32)                         # [128,1]
    b1 = np.asarray(fe1_b, np.float32).reshape(8, 128).T.copy()
    b2 = np.asarray(fe2_b, np.float32).reshape(2, 128).T.copy()
    rb1 = np.asarray(r1_b, np.float32).reshape(4, 128).T.copy()
    rb2 = np.asarray(r2_b, np.float32).reshape(1, 128).T.copy()
    rb3 = np.asarray(r3_b, np.float32).reshape(1, 1)
    noise = np.asarray(router_noise, np.float32).reshape(1, 1)

    in_maps = []
    for c in range(NCORES):
        wqkv = np.ascontiguousarray(np.concatenate(
              [Wq[:, c * 512:(c + 1) * 512],
               Wk[:, c * 128:(c + 1) * 128],
               Wv[:, c * 128:(c + 1) * 128]], axis=1)).astype(bf16)
        in_maps.append(dict(
              hsT=hsT, wqkv=wqkv, wo=Wo, cos2=cos2, sin2=sin2, ident=ident,
              diagneg=diagneg, triup=triup, ones=ones, fe1=fe1, fe2=fe2,
              r1=r1, r2=r2, r3=r3, b1=b1, b2=b2, rb1=rb1, rb2=rb2, rb3=rb3,
              noise=noise, eps=np.full((1, 1), 1e-8, np.float32)))

    res = run_bass_kernel_spmd(nc, in_maps, list(range(NCORES)))
    out = np.concatenate([res.results[c]["out_rows"] for c in range(NCORES)],
                           axis=0)
    return out.reshape(1, S, HID).astype(np.float32)



# revision 53
# speedup vs baseline: 1.4560x; 1.0252x over previous
"""Trainium2 Bass kernel for nn_LlamaAttention_61899068670751.

Sparse (streaming-LLM) attention layer, sharded tensor-parallel over heads
across 8 NeuronCores:
  - core c owns q-heads [4c..4c+3] and kv-head c (GQA group = 4)
  - QKV projections computed per-core with column-sharded weights
  - causal scores computed once per head; both softmax branches (full causal
    and sink+sliding-window) share exp(s) and are folded into a single PV
    matmul via per-row/per-region coefficients
  - the router MLP is computed redundantly on every core after a [128]
    feature AllReduce; its hard gate enters the coefficients
  - o is exchanged with an AllToAll so each core computes 256 rows of the
    final o @ Wo with the full head dimension; host concatenates row shards

All matmuls run as float32r (full-rate fp32 on the PE array).
"""
import numpy as np
from contextlib import ExitStack

import concourse.bacc as bacc
import concourse.mybir as mybir
import concourse.tile as tile
from concourse.bass_utils import run_bass_kernel_spmd

dt = mybir.dt
AF = mybir.ActivationFunctionType
ALU = mybir.AluOpType
AX = mybir.AxisListType

NCORES = 8
S, H, KV, D, HID = 2048, 32, 8, 128, 4096
SINK, WIN, POOL = 128, 1024, 100
HLOC = H // NCORES          # 4 q heads per core
NBLK = S // 128             # 16 row/col blocks
NCH = 4                     # s-chunks of 512
CH = 512
KT = HID // 128             # 32 contraction tiles
SCALE = 1.0 / float(np.sqrt(D))
NEG = -1.0e30
ROWS = S // NCORES          # 256 output rows per core


def _groups(I):
    """Right-aligned 4-block groups over causal blocks 0..I."""
    n = I + 1
    lo = n % 4
    g = [(0, lo)] if lo else []
    g += [(s, 4) for s in range(lo, n, 4)]
    return g


KNOBS = dict(phases=3, wo_bufs=16, w_bufs=4, hs_bufs=1, sc_bufs=4, pt_bufs=2,
             e_bufs=4, p_bufs=4, pT_bufs=2, o_bufs=1, tr_bufs=2)


def build():
    nc = bacc.Bacc("TRN2", target_bir_lowering=False, debug=False,
                   num_devices=NCORES)

    def din(name, shape, d=dt.float32r):
        return nc.dram_tensor(name, shape, d, kind="ExternalInput").ap()

    hsT_d = din("hsT", [HID, S], dt.bfloat16)
    wqkv_d = din("wqkv", [HID, 768], dt.bfloat16)
    wo_d = din("wo", [HID, HID], dt.bfloat16)
    cos2_d = din("cos2", [128, S], dt.float32)
    sin2_d = din("sin2", [128, S], dt.float32)
    ident_d = din("ident", [128, 128])
    diagneg_d = din("diagneg", [128, 128], dt.float32)
    triup_d = din("triup", [128, 128], dt.float32)
    ones_d = din("ones", [1, 128], dt.float32)
    fe1_d = din("fe1", [128, 1024], dt.float32)
    fe2_d = din("fe2", [128, 8 * 256], dt.float32)
    r1_d = din("r1", [128, 2 * 512], dt.float32)
    r2_d = din("r2", [128, 4 * 128], dt.float32)
    r3_d = din("r3", [128, 1], dt.float32)
    b1_d = din("b1", [128, 8], dt.float32)
    b2_d = din("b2", [128, 2], dt.float32)
    rb1_d = din("rb1", [128, 4], dt.float32)
    rb2_d = din("rb2", [128, 1], dt.float32)
    rb3_d = din("rb3", [1, 1], dt.float32)
    noise_d = din("noise", [1, 1], dt.float32)
    eps_d = din("eps", [1, 1], dt.float32)

    out_d = nc.dram_tensor("out_rows", [ROWS, HID], dt.float32,
                           kind="ExternalOutput").ap()

    with tile.TileContext(nc) as tc, ExitStack() as top:
        # ---- long-lived pools -------------------------------------------
        const = top.enter_context(tc.tile_pool(name="const", bufs=1))
        persist = top.enter_context(tc.tile_pool(name="persist", bufs=1))
        dram = top.enter_context(tc.tile_pool(name="dram", bufs=1, space="DRAM"))

        ident = const.tile([128, 128], dt.float32r)
        diagneg = const.tile([128, 128], dt.float32)
        triup = const.tile([128, 128], dt.float32)
        ones_r = const.tile([1, 128], dt.float32)
        nc.sync.dma_start(ident[:], ident_d[:])
        nc.sync.dma_start(diagneg[:], diagneg_d[:])
        nc.sync.dma_start(triup[:], triup_d[:])
        nc.sync.dma_start(ones_r[:], ones_d[:])

        qT = [persist.tile([128, S], dt.float32r, name=f"qT{h}", tag=f"qT{h}")
              for h in range(HLOC)]
        kT = persist.tile([128, S], dt.float32r)
        vN = persist.tile([128, NBLK * 128], dt.float32r)   # v natural, per block

        # collective bounce buffers
        a2a_in0 = dram.tile([NCORES, 2 * 128, ROWS], dt.bfloat16)
        a2a_out0 = dram.tile([NCORES, 2 * 128, ROWS], dt.bfloat16)
        a2a_in1 = dram.tile([NCORES, 2 * 128, ROWS], dt.bfloat16)
        a2a_out1 = dram.tile([NCORES, 2 * 128, ROWS], dt.bfloat16)
        cc_in = dram.tile([128, 1], dt.float32)
        cc_out = dram.tile([128, 1], dt.float32, addr_space="Shared")

        # ---- phase 1: hs transpose + QKV projections + rope -------------
        with ExitStack() as ph1:
            p_hs = ph1.enter_context(tc.tile_pool(name="hs", bufs=KNOBS["hs_bufs"]))
            p_hsT = ph1.enter_context(tc.tile_pool(name="hsT", bufs=2))
            p_w = ph1.enter_context(tc.tile_pool(name="wslab", bufs=KNOBS["w_bufs"]))
            p_rope = ph1.enter_context(tc.tile_pool(name="rope", bufs=2))
            p_cs = ph1.enter_context(tc.tile_pool(name="cs", bufs=2))
            ps_tr = ph1.enter_context(
                tc.tile_pool(name="ps_tr", bufs=KNOBS["tr_bufs"], space="PSUM"))
            ps_acc = ph1.enter_context(
                tc.tile_pool(name="ps_acc", bufs=1, space="PSUM"))

            KH = KT // 2        # 16 k-tiles per half
            for g in range(NCH):
                s0 = g * CH
                accs = [ps_acc.tile([128, CH], dt.float32, tag=f"acc{i}",
                                    name=f"acc{i}")
                        for i in range(6)]
                for half in range(2):
                    k0 = half * KH
                    hsT = p_hsT.tile([128, KH, CH], dt.bfloat16, tag="hsT")
                    nc.sync.dma_start(
                        hsT[:], hsT_d[k0 * 128:(k0 + KH) * 128,
                                      s0:s0 + CH].rearrange(
                            "(k p) n -> p k n", p=128))
                    for k4 in range(KH // 4):
                        wsl = p_w.tile([128, 4, 768], dt.bfloat16, tag="w")
                        kb = k0 + 4 * k4
                        nc.sync.dma_start(
                            wsl[:], wqkv_d[kb * 128:(kb + 4) * 128,
                                           :].rearrange(
                                "(k p) n -> p k n", p=128))
                        for kk in range(4):
                            kt = kb + kk
                            for i in range(6):
                                nc.tensor.matmul(
                                    accs[i][:],
                                    wsl[:, kk, i * 128:(i + 1) * 128],
                                    hsT[:, 4 * k4 + kk, :],
                                    start=(kt == 0), stop=(kt == KT - 1))

                # rope for q heads (0..3) and k (4)
                cos_sl = p_cs.tile([128, CH], dt.float32, tag="cos")
                sin_sl = p_cs.tile([128, CH], dt.float32, tag="sin")
                nc.sync.dma_start(cos_sl[:], cos2_d[:, s0:s0 + CH])
                nc.sync.dma_start(sin_sl[:], sin2_d[:, s0:s0 + CH])
                for i in range(5):
                    dest = qT[i] if i < HLOC else kT
                    lin = p_rope.tile([128, CH], dt.float32, tag="lin")
                    rot = p_rope.tile([128, CH], dt.float32, tag="rot")
                    t1 = p_rope.tile([128, CH], dt.float32, tag="t1")
                    t2 = p_rope.tile([128, CH], dt.float32, tag="t2")
                    nc.scalar.copy(lin[:], accs[i][:])
                    nc.sync.dma_start(rot[0:64, :], lin[64:128, :])
                    nc.sync.dma_start(rot[64:128, :], lin[0:64, :])
                    nc.vector.tensor_tensor(t1[:], lin[:], cos_sl[:], ALU.mult)
                    nc.vector.tensor_tensor(t2[:], rot[:], sin_sl[:], ALU.mult)
                    nc.vector.tensor_tensor(dest[:, s0:s0 + CH], t1[:], t2[:],
                                            ALU.add)
                # v: copy then transpose to natural layout
                vT = p_rope.tile([128, CH], dt.float32r, tag="vT")
                nc.scalar.copy(vT[:], accs[5][:])
                for ss in range(4):
                    ptr = ps_tr.tile([128, 128], dt.float32r, tag="tr")
                    nc.tensor.transpose(ptr[:], vT[:, ss * 128:(ss + 1) * 128],
                                        ident[:])
                    nc.any.tensor_copy(
                        vN[:, (g * 4 + ss) * 128:(g * 4 + ss + 1) * 128], ptr[:])

        # ---- phase 2: router + attention --------------------------------
        with ExitStack() as ph2:
          if KNOBS["phases"] >= 2:
              p_mlp = ph2.enter_context(tc.tile_pool(name="mlp", bufs=1))
              p_e = ph2.enter_context(tc.tile_pool(name="eband", bufs=KNOBS["e_bufs"]))
              p_p = ph2.enter_context(tc.tile_pool(name="pband", bufs=KNOBS["p_bufs"]))
              p_pT = ph2.enter_context(tc.tile_pool(name="pT", bufs=KNOBS["pT_bufs"]))
              p_sm = ph2.enter_context(tc.tile_pool(name="sums", bufs=4))
              p_ob = ph2.enter_context(tc.tile_pool(name="obuf", bufs=2))
              ps_sc = ph2.enter_context(
                  tc.tile_pool(name="ps_sc", bufs=KNOBS["sc_bufs"], space="PSUM"))
              ps_pt = ph2.enter_context(
                  tc.tile_pool(name="ps_pt", bufs=KNOBS["pt_bufs"], space="PSUM"))
              ps_o = ph2.enter_context(
                  tc.tile_pool(name="ps_o", bufs=KNOBS["o_bufs"], space="PSUM"))
              mlp_ctx = ExitStack()
              ps_m = mlp_ctx.enter_context(
                  tc.tile_pool(name="ps_m", bufs=1, space="PSUM"))

              # --- router ---
              feat_acc = p_mlp.tile([128, 8], dt.float32)
              for h in range(HLOC):
                  nc.vector.tensor_reduce(feat_acc[:, h:h + 1],
                                          qT[h][:, 0:POOL], AX.X, ALU.add)
                  nc.vector.tensor_reduce(feat_acc[:, 4 + h:5 + h],
                                          qT[h][:, S - POOL:S], AX.X, ALU.add)
              feat_s = p_mlp.tile([128, 1], dt.float32)
              nc.vector.tensor_reduce(feat_s[:], feat_acc[:], AX.X, ALU.add)
              feat_r = p_mlp.tile([128, 1], dt.float32)
              nc.scalar.activation(feat_r[:], feat_s[:], AF.Copy,
                                   scale=1.0 / (2 * POOL * H))
              nc.sync.dma_start(cc_in[:], feat_r[:])
              nc.gpsimd.collective_compute(
                  "AllReduce", ALU.add,
                  replica_groups=[list(range(NCORES))],
                  ins=[cc_in.opt()], outs=[cc_out.opt()])
              featg = p_mlp.tile([128, 1], dt.float32)
              nc.sync.dma_start(featg[:], cc_out[:])

              # MLP weights
              fe1 = p_mlp.tile([128, 1024], dt.float32)
              fe2 = p_mlp.tile([128, 8 * 256], dt.float32)
              r1w = p_mlp.tile([128, 2 * 512], dt.float32)
              r2w = p_mlp.tile([128, 4 * 128], dt.float32)
              r3w = p_mlp.tile([128, 1], dt.float32)
              b1 = p_mlp.tile([128, 8], dt.float32)
              b2 = p_mlp.tile([128, 2], dt.float32)
              rb1 = p_mlp.tile([128, 4], dt.float32)
              rb2 = p_mlp.tile([128, 1], dt.float32)
              rb3 = p_mlp.tile([1, 1], dt.float32)
              noise = p_mlp.tile([1, 1], dt.float32)
              epsb = p_mlp.tile([1, 1], dt.float32)
              nc.sync.dma_start(epsb[:], eps_d[:])
              for t_, d_ in ((fe1, fe1_d), (fe2, fe2_d), (r1w, r1_d),
                             (r2w, r2_d), (r3w, r3_d), (b1, b1_d), (b2, b2_d),
                             (rb1, rb1_d), (rb2, rb2_d), (rb3, rb3_d),
                             (noise, noise_d)):
                  nc.sync.dma_start(t_[:], d_[:])

              def mlp_layer(vec_in, w_sb, ktiles, ntiles, bias, act, nwidth=128):
                  """vec_in: [128, ktiles] fp32r columns; returns [128, ntiles]."""
                  out_r = p_mlp.tile([128, max(ntiles, 1)], dt.float32,
                                     name=f"mlpv{len(mlp_tmp)}")
                  mlp_tmp.append(out_r)
                  ps = ps_m.tile([128, max(ntiles, 1)], dt.float32, tag="mlp",
                               name="mlpps")
                  for t in range(ntiles):
                      for k in range(ktiles):
                          nc.tensor.matmul(
                              ps[:, t:t + 1],
                              w_sb[:, (k * ntiles + t) * nwidth:
                                   (k * ntiles + t) * nwidth + nwidth],
                              vec_in[:, k:k + 1],
                              start=(k == 0), stop=(k == ktiles - 1))
                  for t in range(ntiles):
                      nc.scalar.activation(out_r[:, t:t + 1], ps[:, t:t + 1],
                                           act, bias=bias[:, t:t + 1])
                  return out_r

              mlp_tmp = []
              h1 = mlp_layer(featg, fe1, 1, 8, b1, AF.Silu)
              h2 = mlp_layer(h1, fe2, 8, 2, b2, AF.Identity)
              h3 = mlp_layer(h2, r1w, 2, 4, rb1, AF.Silu)
              h4 = mlp_layer(h3, r2w, 4, 1, rb2, AF.Silu)
              lps = ps_m.tile([1, 1], dt.float32, tag="mlp")
              nc.tensor.matmul(lps[:], r3w[:], h4[:], start=True, stop=True)
              logits = p_mlp.tile([1, 1], dt.float32)
              nc.scalar.activation(logits[:], lps[:], AF.Identity, bias=rb3[:])
              l1 = p_mlp.tile([1, 1], dt.float32)
              l2 = p_mlp.tile([1, 1], dt.float32)
              nc.scalar.activation(l1[:], noise[:], AF.Ln, bias=epsb[:])
              nc.scalar.activation(l2[:], l1[:], AF.Ln, bias=epsb[:], scale=-1.0)
              zin = p_mlp.tile([1, 1], dt.float32)
              nc.vector.tensor_tensor(zin[:], logits[:], l2[:], ALU.subtract)
              zsoft = p_mlp.tile([1, 1], dt.float32)
              nc.scalar.activation(zsoft[:], zin[:], AF.Sigmoid)
              zhard = p_mlp.tile([1, 1], dt.float32)
              nc.vector.tensor_scalar(zhard[:], zsoft[:], 0.5, None, ALU.is_gt)
              mps = ps_m.tile([128, 1], dt.float32, tag="mlp")
              nc.tensor.matmul(mps[:], ones_r[:], zhard[:], start=True, stop=True)
              mix = p_mlp.tile([128, 1], dt.float32)
              nc.scalar.copy(mix[:], mps[:])
              onem = p_mlp.tile([128, 1], dt.float32)
              nc.vector.tensor_scalar(onem[:], mix[:], -1.0, 1.0, ALU.mult,
                                      ALU.add)
              mlp_ctx.close()

              # --- attention ---
              for h in range(HLOC):
                  for g in range(NCH):
                      pT = p_pT.tile([128, NBLK * CH], dt.float32r, tag="pT")
                      for Ii in range(4):
                          I = 4 * g + Ii
                          nb_tot = (I + 1) * 128
                          e = p_e.tile([128, S], dt.float32, tag="e")
                          pband = p_p.tile([128, S], dt.float32r, tag="p")
                          sums = p_sm.tile([128, 16], dt.float32, tag="sums")
                          grps = _groups(I)
                          ng = len(grps)
                          for gi, (sb, nb) in enumerate(grps):
                              w = nb * 128
                              col = 4 - ng + gi
                              sc = ps_sc.tile([128, 512], dt.float32, tag="sc")
                              nc.tensor.matmul(
                                  sc[:, 0:w], qT[h][:, I * 128:(I + 1) * 128],
                                  kT[:, sb * 128: sb * 128 + w],
                                  start=True, stop=True)
                              if gi == ng - 1:
                                  nc.vector.tensor_tensor(
                                      sc[:, w - 128:w], sc[:, w - 128:w],
                                      diagneg[:], ALU.add)
                              nc.scalar.activation(
                                  e[:, sb * 128: sb * 128 + w], sc[:, 0:w],
                                  AF.Exp, scale=SCALE,
                                  accum_out=sums[:, col:col + 1])
                          if I >= 9:
                              tmask = p_sm.tile([128, 128], dt.float32,
                                                tag="tmask")
                              nc.vector.tensor_reduce(
                                  sums[:, 4:5], e[:, 0:128], AX.X, ALU.add)
                              nc.vector.tensor_tensor(
                                  tmask[:], e[:, (I - 8) * 128:(I - 7) * 128],
                                  triup[:], ALU.mult)
                              nc.vector.tensor_reduce(
                                  sums[:, 5:6], tmask[:], AX.X, ALU.add)
                              nc.vector.tensor_reduce(
                                  sums[:, 6:7], sums[:, 4 - ng:4], AX.X, ALU.add)
                              nc.vector.tensor_reduce(
                                  sums[:, 7:8], sums[:, 2:6], AX.X, ALU.add)
                              nc.vector.reciprocal(sums[:, 8:9], sums[:, 6:7])
                              nc.vector.reciprocal(sums[:, 9:10], sums[:, 7:8])
                              nc.vector.tensor_tensor(
                                  sums[:, 10:11], sums[:, 8:9], onem[:], ALU.mult)
                              nc.vector.tensor_tensor(
                                  sums[:, 11:12], sums[:, 9:10], mix[:], ALU.mult)
                              nc.vector.tensor_tensor(
                                  sums[:, 12:13], sums[:, 10:11], sums[:, 11:12],
                                  ALU.add)
                              a_ap = sums[:, 12:13]
                              b_ap = sums[:, 10:11]
                              amb_ap = sums[:, 11:12]
                              nc.vector.tensor_scalar(
                                  pband[:, 0:128], e[:, 0:128], a_ap, None,
                                  ALU.mult)
                              if I >= 10:
                                  nc.vector.tensor_scalar(
                                      pband[:, 128:(I - 8) * 128],
                                      e[:, 128:(I - 8) * 128], b_ap, None,
                                      ALU.mult)
                              nc.vector.tensor_scalar(
                                  pband[:, (I - 8) * 128:(I - 7) * 128],
                                  e[:, (I - 8) * 128:(I - 7) * 128], b_ap, None,
                                  ALU.mult)
                              nc.vector.scalar_tensor_tensor(
                                  pband[:, (I - 8) * 128:(I - 7) * 128],
                                  tmask[:], amb_ap,
                                  pband[:, (I - 8) * 128:(I - 7) * 128],
                                  ALU.mult, ALU.add)
                              nc.vector.tensor_scalar(
                                  pband[:, (I - 7) * 128:nb_tot],
                                  e[:, (I - 7) * 128:nb_tot], a_ap, None,
                                  ALU.mult)
                          else:
                              nc.vector.tensor_reduce(
                                  sums[:, 6:7], sums[:, 4 - ng:4], AX.X, ALU.add)
                              nc.vector.reciprocal(sums[:, 8:9], sums[:, 6:7])
                              nc.vector.tensor_scalar(
                                  pband[:, 0:nb_tot], e[:, 0:nb_tot],
                                  sums[:, 8:9], None, ALU.mult)
                          pT3 = pT[:, :].rearrange("p (J c) -> p J c", c=CH)
                          for J0 in range(0, I + 1, 4):
                              nb4 = min(4, I + 1 - J0)
                              ptp = ps_pt.tile([128, 512], dt.float32r, tag="pt")
                              for jj in range(nb4):
                                  nc.tensor.transpose(
                                      ptp[:, jj * 128:(jj + 1) * 128],
                                      pband[:, (J0 + jj) * 128:
                                            (J0 + jj + 1) * 128],
                                      ident[:])
                              nc.any.tensor_copy(
                                  pT3[:, J0:J0 + nb4,
                                      Ii * 128:(Ii + 1) * 128],
                                  ptp[:, 0:nb4 * 128].rearrange(
                                      "p (J c) -> p J c", c=128))
                      # PV for this (h, chunk)
                      ops = ps_o.tile([128, CH], dt.float32, tag="o")
                      last_J = 4 * g + 3
                      for J in range(last_J + 1):
                          k = J - 4 * g
                          if k <= 0:
                              nc.tensor.matmul(
                                  ops[:], vN[:, J * 128:(J + 1) * 128],
                                  pT[:, J * CH:(J + 1) * CH],
                                  start=(J == 0), stop=(J == last_J))
                          else:
                              nc.tensor.matmul(
                                  ops[:, k * 128:CH],
                                  vN[:, J * 128:(J + 1) * 128],
                                  pT[:, J * CH + k * 128:(J + 1) * CH],
                                  start=False, stop=(J == last_J))
                      osb = p_ob.tile([128, CH], dt.bfloat16, tag="osb")
                      nc.scalar.copy(osb[:], ops[:])
                      a2a_in_h = a2a_in0 if h < 2 else a2a_in1
                      hh = h % 2
                      nc.sync.dma_start(
                          a2a_in_h[2 * g, hh * 128:(hh + 1) * 128, :],
                          osb[:, 0:ROWS])
                      nc.sync.dma_start(
                          a2a_in_h[2 * g + 1, hh * 128:(hh + 1) * 128, :],
                          osb[:, ROWS:CH])
                  if h == 1:
                      nc.gpsimd.collective_compute(
                          "AllToAll", ALU.bypass,
                          replica_groups=[list(range(NCORES))],
                          ins=[a2a_in0.opt()], outs=[a2a_out0.opt()])
                  if h == 3:
                      nc.gpsimd.collective_compute(
                          "AllToAll", ALU.bypass,
                          replica_groups=[list(range(NCORES))],
                          ins=[a2a_in1.opt()], outs=[a2a_out1.opt()])

        # ---- phase 3: AllToAll + output projection ----------------------
        with ExitStack() as ph3:
          if KNOBS["phases"] >= 3:
              p_oT = ph3.enter_context(tc.tile_pool(name="oT", bufs=1))
              p_part = ph3.enter_context(tc.tile_pool(name="part", bufs=1))
              p_wo = ph3.enter_context(tc.tile_pool(name="wo", bufs=KNOBS["wo_bufs"]))
              p_os = ph3.enter_context(tc.tile_pool(name="outsb", bufs=2))
              ps_w = ph3.enter_context(
                  tc.tile_pool(name="ps_w", bufs=2, space="PSUM"))

              # two 16-ktile halves: half 0 (heads 0-1, from a2a_out0)
              # overlaps AllToAll#2; half 1 adds the partials and stores.
              oTb = p_oT.tile([128, KT * ROWS], dt.bfloat16, name="oTb",
                              tag="oTb")
              parts = [p_part.tile([128, 512], dt.float32, name=f"part{i}",
                                   tag=f"part{i}") for i in range(16)]
              for half in range(2):
                  buf = a2a_out0 if half == 0 else a2a_out1
                  for p in range(NCORES):
                      for tt in range(2):
                          kt = 4 * p + 2 * half + tt
                          nc.sync.dma_start(
                              oTb[:, kt * ROWS:(kt + 1) * ROWS],
                              buf[p, tt * 128:(tt + 1) * 128, :])
                  for ngi in range(8):
                      pso = [ps_w.tile([128, 512], dt.float32, tag=f"wo{st}",
                                      name=f"wo{st}")
                             for st in range(2)]
                      for b2 in range(NCORES):
                          r0 = (4 * b2 + 2 * half) * 128
                          wsl = p_wo.tile([128, 2, 512], dt.bfloat16,
                                          tag="wo")
                          nc.sync.dma_start(
                              wsl[:], wo_d[r0:r0 + 256,
                                           ngi * 512:(ngi + 1) * 512
                                           ].rearrange(
                                  "(k p) n -> p k n", p=128))
                          for kk in range(2):
                              kt = 4 * b2 + 2 * half + kk
                              ki = 2 * b2 + kk
                              for st in range(2):
                                  nc.tensor.matmul(
                                      pso[st][:],
                                      oTb[:, kt * ROWS + st * 128:
                                          kt * ROWS + (st + 1) * 128],
                                      wsl[:, kk, :], start=(ki == 0),
                                      stop=(ki == 15))
                      for st in range(2):
                          if half == 0:
                              nc.scalar.copy(parts[2 * ngi + st][:],
                                             pso[st][:])
                          else:
                              osb = p_os.tile([128, 512], dt.float32,
                                              tag="os")
                              nc.vector.tensor_tensor(
                                  osb[:], pso[st][:],
                                  parts[2 * ngi + st][:], ALU.add)
                              nc.sync.dma_start(
                                  out_d[st * 128:(st + 1) * 128,
                                        ngi * 512:(ngi + 1) * 512], osb[:])

    nc.compile()
    return nc


_CACHE = {}


def _host_constants():
    inv = 10000.0 ** (-np.arange(0, D, 2, dtype=np.float64) / D)
    t = np.arange(S, dtype=np.float64)
    fr = np.outer(t, inv)                      # [S, 64]
    cos = np.cos(fr).T.astype(np.float32)      # [64, S]
    sin = np.sin(fr).T.astype(np.float32)
    cos2 = np.vstack([cos, cos])
    sin2 = np.vstack([-sin, sin])
    ident = np.eye(128, dtype=np.float32)
    a = np.arange(128)
    diagneg = np.where(a[None, :] <= a[:, None], 0.0, NEG).astype(np.float32)
    triup = (a[None, :] > a[:, None]).astype(np.float32)
    ones = np.ones((1, 128), dtype=np.float32)
    return cos2, sin2, ident, diagneg, triup, ones


def kernel(hidden_states, Wq, Wk, Wv, Wo, fe1_w, fe1_b, fe2_w, fe2_b,
           r1_w, r1_b, r2_w, r2_b, r3_w, r3_b, router_noise):
    if "nc" not in _CACHE:
        _CACHE["nc"] = build()
    nc = _CACHE["nc"]

    import ml_dtypes
    bf16 = ml_dtypes.bfloat16
    hsT = np.ascontiguousarray(
        np.asarray(hidden_states, dtype=np.float32).reshape(S, HID).T
    ).astype(bf16)
    Wq = np.asarray(Wq, np.float32)
    Wk = np.asarray(Wk, np.float32)
    Wv = np.asarray(Wv, np.float32)
    Wo = np.ascontiguousarray(np.asarray(Wo, np.float32)).astype(bf16)
    cos2, sin2, ident, diagneg, triup, ones = _host_constants()

    def ktile_cols(w, ktiles, ntiles, nwidth):
        # [K, N] -> [128, ktiles*ntiles*nwidth] with (k, t) slab layout
        return np.ascontiguousarray(
              np.concatenate([w[k * 128:(k + 1) * 128, :] for k in range(ktiles)],
                             axis=1))

    fe1 = np.asarray(fe1_w, np.float32)                       # [128,1024]
    fe2 = ktile_cols(np.asarray(fe2_w, np.float32), 8, 2, 128)
    r1 = ktile_cols(np.asarray(r1_w, np.float32), 2, 4, 128)
    r2 = ktile_cols(np.asarray(r2_w, np.float32), 4, 1, 128)
    r3 = np.asarray(r3_w, np.float32)                         # [128,1]
    b1 = np.asarray(fe1_b, np.float32).reshape(8, 128).T.copy()
    b2 = np.asarray(fe2_b, np.float32).reshape(2, 128).T.copy()
    rb1 = np.asarray(r1_b, np.float32).reshape(4, 128).T.copy()
    rb2 = np.asarray(r2_b, np.float32).reshape(1, 128).T.copy()
    rb3 = np.asarray(r3_b, np.float32).reshape(1, 1)
    noise = np.asarray(router_noise, np.float32).reshape(1, 1)

    in_maps = []
    for c in range(NCORES):
        wqkv = np.ascontiguousarray(np.concatenate(
              [Wq[:, c * 512:(c + 1) * 512],
               Wk[:, c * 128:(c + 1) * 128],
               Wv[:, c * 128:(c + 1) * 128]], axis=1)).astype(bf16)
        in_maps.append(dict(
              hsT=hsT, wqkv=wqkv, wo=Wo, cos2=cos2, sin2=sin2, ident=ident,
              diagneg=diagneg, triup=triup, ones=ones, fe1=fe1, fe2=fe2,
              r1=r1, r2=r2, r3=r3, b1=b1, b2=b2, rb1=rb1, rb2=rb2, rb3=rb3,
              noise=noise, eps=np.full((1, 1), 1e-8, np.float32)))

    res = run_bass_kernel_spmd(nc, in_maps, list(range(NCORES)))
    out = np.concatenate([res.results[c]["out_rows"] for c in range(NCORES)],
                           axis=0)
    return out.reshape(1, S, HID).astype(np.float32)



# revision 59
# speedup vs baseline: 1.4635x; 1.0051x over previous
"""Trainium2 Bass kernel for nn_LlamaAttention_61899068670751.

Sparse (streaming-LLM) attention layer, sharded tensor-parallel over heads
across 8 NeuronCores:
  - core c owns q-heads [4c..4c+3] and kv-head c (GQA group = 4)
  - QKV projections computed per-core with column-sharded weights
  - causal scores computed once per head; both softmax branches (full causal
    and sink+sliding-window) share exp(s) and are folded into a single PV
    matmul via per-row/per-region coefficients
  - the router MLP is computed redundantly on every core after a [128]
    feature AllReduce; its hard gate enters the coefficients
  - o is exchanged with an AllToAll so each core computes 256 rows of the
    final o @ Wo with the full head dimension; host concatenates row shards

All matmuls run as float32r (full-rate fp32 on the PE array).
"""
import numpy as np
from contextlib import ExitStack

import concourse.bacc as bacc
import concourse.mybir as mybir
import concourse.tile as tile
from concourse.bass_utils import run_bass_kernel_spmd

dt = mybir.dt
AF = mybir.ActivationFunctionType
ALU = mybir.AluOpType
AX = mybir.AxisListType

NCORES = 8
S, H, KV, D, HID = 2048, 32, 8, 128, 4096
SINK, WIN, POOL = 128, 1024, 100
HLOC = H // NCORES          # 4 q heads per core
NBLK = S // 128             # 16 row/col blocks
NCH = 4                     # s-chunks of 512
CH = 512
KT = HID // 128             # 32 contraction tiles
SCALE = 1.0 / float(np.sqrt(D))
NEG = -1.0e30
ROWS = S // NCORES          # 256 output rows per core


def _groups(I):
    """Right-aligned 4-block groups over causal blocks 0..I."""
    n = I + 1
    lo = n % 4
    g = [(0, lo)] if lo else []
    g += [(s, 4) for s in range(lo, n, 4)]
    return g


KNOBS = dict(phases=3, wo_bufs=16, w_bufs=4, hs_bufs=1, sc_bufs=4, pt_bufs=2,
             e_bufs=5, p_bufs=5, pT_bufs=2, o_bufs=1, tr_bufs=2)


def build():
    nc = bacc.Bacc("TRN2", target_bir_lowering=False, debug=False,
                   num_devices=NCORES)

    def din(name, shape, d=dt.float32r):
        return nc.dram_tensor(name, shape, d, kind="ExternalInput").ap()

    hsT_d = din("hsT", [HID, S], dt.bfloat16)
    wqkv_d = din("wqkv", [HID, 768], dt.bfloat16)
    wo_d = din("wo", [HID, HID], dt.bfloat16)
    cos2_d = din("cos2", [128, S], dt.float32)
    sin2_d = din("sin2", [128, S], dt.float32)
    ident_d = din("ident", [128, 128])
    diagneg_d = din("diagneg", [128, 128], dt.float32)
    triup_d = din("triup", [128, 128], dt.float32)
    ones_d = din("ones", [1, 128], dt.float32)
    fe1_d = din("fe1", [128, 1024], dt.float32)
    fe2_d = din("fe2", [128, 8 * 256], dt.float32)
    r1_d = din("r1", [128, 2 * 512], dt.float32)
    r2_d = din("r2", [128, 4 * 128], dt.float32)
    r3_d = din("r3", [128, 1], dt.float32)
    b1_d = din("b1", [128, 8], dt.float32)
    b2_d = din("b2", [128, 2], dt.float32)
    rb1_d = din("rb1", [128, 4], dt.float32)
    rb2_d = din("rb2", [128, 1], dt.float32)
    rb3_d = din("rb3", [1, 1], dt.float32)
    noise_d = din("noise", [1, 1], dt.float32)
    eps_d = din("eps", [1, 1], dt.float32)

    out_d = nc.dram_tensor("out_rows", [ROWS, HID], dt.float32,
                           kind="ExternalOutput").ap()

    with tile.TileContext(nc) as tc, ExitStack() as top:
        # ---- long-lived pools -------------------------------------------
        const = top.enter_context(tc.tile_pool(name="const", bufs=1))
        persist = top.enter_context(tc.tile_pool(name="persist", bufs=1))
        dram = top.enter_context(tc.tile_pool(name="dram", bufs=1, space="DRAM"))

        ident = const.tile([128, 128], dt.float32r)
        diagneg = const.tile([128, 128], dt.float32)
        triup = const.tile([128, 128], dt.float32)
        ones_r = const.tile([1, 128], dt.float32)
        nc.sync.dma_start(ident[:], ident_d[:])
        nc.sync.dma_start(diagneg[:], diagneg_d[:])
        nc.sync.dma_start(triup[:], triup_d[:])
        nc.sync.dma_start(ones_r[:], ones_d[:])

        qT = [persist.tile([128, S], dt.bfloat16, name=f"qT{h}", tag=f"qT{h}")
              for h in range(HLOC)]
        kT = persist.tile([128, S], dt.bfloat16)
        vN = persist.tile([128, NBLK * 128], dt.float32r)   # v natural, per block

        # collective bounce buffers
        a2a_in0 = dram.tile([NCORES, 2 * 128, ROWS], dt.bfloat16)
        a2a_out0 = dram.tile([NCORES, 2 * 128, ROWS], dt.bfloat16)
        a2a_in1 = dram.tile([NCORES, 2 * 128, ROWS], dt.bfloat16)
        a2a_out1 = dram.tile([NCORES, 2 * 128, ROWS], dt.bfloat16)
        cc_in = dram.tile([128, 1], dt.float32)
        cc_out = dram.tile([128, 1], dt.float32, addr_space="Shared")

        # ---- phase 1: hs transpose + QKV projections + rope -------------
        with ExitStack() as ph1:
            p_hs = ph1.enter_context(tc.tile_pool(name="hs", bufs=KNOBS["hs_bufs"]))
            p_hsT = ph1.enter_context(tc.tile_pool(name="hsT", bufs=2))
            p_w = ph1.enter_context(tc.tile_pool(name="wslab", bufs=KNOBS["w_bufs"]))
            p_rope = ph1.enter_context(tc.tile_pool(name="rope", bufs=2))
            p_cs = ph1.enter_context(tc.tile_pool(name="cs", bufs=2))
            ps_tr = ph1.enter_context(
                tc.tile_pool(name="ps_tr", bufs=KNOBS["tr_bufs"], space="PSUM"))
            ps_acc = ph1.enter_context(
                tc.tile_pool(name="ps_acc", bufs=1, space="PSUM"))

            KH = KT // 2        # 16 k-tiles per half
            for g in range(NCH):
                s0 = g * CH
                accs = [ps_acc.tile([128, CH], dt.float32, tag=f"acc{i}",
                                    name=f"acc{i}")
                        for i in range(6)]
                for half in range(2):
                    k0 = half * KH
                    hsT = p_hsT.tile([128, KH, CH], dt.bfloat16, tag="hsT")
                    nc.sync.dma_start(
                        hsT[:], hsT_d[k0 * 128:(k0 + KH) * 128,
                                      s0:s0 + CH].rearrange(
                            "(k p) n -> p k n", p=128))
                    for k4 in range(KH // 4):
                        wsl = p_w.tile([128, 4, 768], dt.bfloat16, tag="w")
                        kb = k0 + 4 * k4
                        nc.sync.dma_start(
                            wsl[:], wqkv_d[kb * 128:(kb + 4) * 128,
                                           :].rearrange(
                                "(k p) n -> p k n", p=128))
                        for kk in range(4):
                            kt = kb + kk
                            for i in range(6):
                                nc.tensor.matmul(
                                    accs[i][:],
                                    wsl[:, kk, i * 128:(i + 1) * 128],
                                    hsT[:, 4 * k4 + kk, :],
                                    start=(kt == 0), stop=(kt == KT - 1))

                # rope for q heads (0..3) and k (4)
                cos_sl = p_cs.tile([128, CH], dt.float32, tag="cos")
                sin_sl = p_cs.tile([128, CH], dt.float32, tag="sin")
                nc.sync.dma_start(cos_sl[:], cos2_d[:, s0:s0 + CH])
                nc.sync.dma_start(sin_sl[:], sin2_d[:, s0:s0 + CH])
                for i in range(5):
                    dest = qT[i] if i < HLOC else kT
                    lin = p_rope.tile([128, CH], dt.float32, tag="lin")
                    rot = p_rope.tile([128, CH], dt.float32, tag="rot")
                    t1 = p_rope.tile([128, CH], dt.float32, tag="t1")
                    t2 = p_rope.tile([128, CH], dt.float32, tag="t2")
                    nc.scalar.copy(lin[:], accs[i][:])
                    nc.sync.dma_start(rot[0:64, :], lin[64:128, :])
                    nc.sync.dma_start(rot[64:128, :], lin[0:64, :])
                    nc.vector.tensor_tensor(t1[:], lin[:], cos_sl[:], ALU.mult)
                    nc.vector.tensor_tensor(t2[:], rot[:], sin_sl[:], ALU.mult)
                    nc.vector.tensor_tensor(dest[:, s0:s0 + CH], t1[:], t2[:],
                                            ALU.add)
                # v: copy then transpose to natural layout
                vT = p_rope.tile([128, CH], dt.float32r, tag="vT")
                nc.scalar.copy(vT[:], accs[5][:])
                for ss in range(4):
                    ptr = ps_tr.tile([128, 128], dt.float32r, tag="tr")
                    nc.tensor.transpose(ptr[:], vT[:, ss * 128:(ss + 1) * 128],
                                        ident[:])
                    nc.any.tensor_copy(
                        vN[:, (g * 4 + ss) * 128:(g * 4 + ss + 1) * 128], ptr[:])

        # ---- phase 2: router + attention --------------------------------
        with ExitStack() as ph2:
          if KNOBS["phases"] >= 2:
              p_mlp = ph2.enter_context(tc.tile_pool(name="mlp", bufs=1))
              p_e = ph2.enter_context(tc.tile_pool(name="eband", bufs=KNOBS["e_bufs"]))
              p_p = ph2.enter_context(tc.tile_pool(name="pband", bufs=KNOBS["p_bufs"]))
              p_pT = ph2.enter_context(tc.tile_pool(name="pT", bufs=KNOBS["pT_bufs"]))
              p_sm = ph2.enter_context(tc.tile_pool(name="sums", bufs=4))
              p_ob = ph2.enter_context(tc.tile_pool(name="obuf", bufs=2))
              ps_sc = ph2.enter_context(
                  tc.tile_pool(name="ps_sc", bufs=KNOBS["sc_bufs"], space="PSUM"))
              ps_pt = ph2.enter_context(
                  tc.tile_pool(name="ps_pt", bufs=KNOBS["pt_bufs"], space="PSUM"))
              ps_o = ph2.enter_context(
                  tc.tile_pool(name="ps_o", bufs=KNOBS["o_bufs"], space="PSUM"))
              mlp_ctx = ExitStack()
              ps_m = mlp_ctx.enter_context(
                  tc.tile_pool(name="ps_m", bufs=1, space="PSUM"))

              # --- router ---
              feat_acc = p_mlp.tile([128, 8], dt.float32)
              for h in range(HLOC):
                  nc.vector.tensor_reduce(feat_acc[:, h:h + 1],
                                          qT[h][:, 0:POOL], AX.X, ALU.add)
                  nc.vector.tensor_reduce(feat_acc[:, 4 + h:5 + h],
                                          qT[h][:, S - POOL:S], AX.X, ALU.add)
              feat_s = p_mlp.tile([128, 1], dt.float32)
              nc.vector.tensor_reduce(feat_s[:], feat_acc[:], AX.X, ALU.add)
              feat_r = p_mlp.tile([128, 1], dt.float32)
              nc.scalar.activation(feat_r[:], feat_s[:], AF.Copy,
                                   scale=1.0 / (2 * POOL * H))
              nc.sync.dma_start(cc_in[:], feat_r[:])
              nc.gpsimd.collective_compute(
                  "AllReduce", ALU.add,
                  replica_groups=[list(range(NCORES))],
                  ins=[cc_in.opt()], outs=[cc_out.opt()])
              featg = p_mlp.tile([128, 1], dt.float32)
              nc.sync.dma_start(featg[:], cc_out[:])

              # MLP weights
              fe1 = p_mlp.tile([128, 1024], dt.float32)
              fe2 = p_mlp.tile([128, 8 * 256], dt.float32)
              r1w = p_mlp.tile([128, 2 * 512], dt.float32)
              r2w = p_mlp.tile([128, 4 * 128], dt.float32)
              r3w = p_mlp.tile([128, 1], dt.float32)
              b1 = p_mlp.tile([128, 8], dt.float32)
              b2 = p_mlp.tile([128, 2], dt.float32)
              rb1 = p_mlp.tile([128, 4], dt.float32)
              rb2 = p_mlp.tile([128, 1], dt.float32)
              rb3 = p_mlp.tile([1, 1], dt.float32)
              noise = p_mlp.tile([1, 1], dt.float32)
              epsb = p_mlp.tile([1, 1], dt.float32)
              nc.sync.dma_start(epsb[:], eps_d[:])
              for t_, d_ in ((fe1, fe1_d), (fe2, fe2_d), (r1w, r1_d),
                             (r2w, r2_d), (r3w, r3_d), (b1, b1_d), (b2, b2_d),
                             (rb1, rb1_d), (rb2, rb2_d), (rb3, rb3_d),
                             (noise, noise_d)):
                  nc.sync.dma_start(t_[:], d_[:])

              def mlp_layer(vec_in, w_sb, ktiles, ntiles, bias, act, nwidth=128):
                  """vec_in: [128, ktiles] fp32r columns; returns [128, ntiles]."""
                  out_r = p_mlp.tile([128, max(ntiles, 1)], dt.float32,
                                     name=f"mlpv{len(mlp_tmp)}")
                  mlp_tmp.append(out_r)
                  ps = ps_m.tile([128, max(ntiles, 1)], dt.float32, tag="mlp",
                               name="mlpps")
                  for t in range(ntiles):
                      for k in range(ktiles):
                          nc.tensor.matmul(
                              ps[:, t:t + 1],
                              w_sb[:, (k * ntiles + t) * nwidth:
                                   (k * ntiles + t) * nwidth + nwidth],
                              vec_in[:, k:k + 1],
                              start=(k == 0), stop=(k == ktiles - 1))
                  for t in range(ntiles):
                      nc.scalar.activation(out_r[:, t:t + 1], ps[:, t:t + 1],
                                           act, bias=bias[:, t:t + 1])
                  return out_r

              mlp_tmp = []
              h1 = mlp_layer(featg, fe1, 1, 8, b1, AF.Silu)
              h2 = mlp_layer(h1, fe2, 8, 2, b2, AF.Identity)
              h3 = mlp_layer(h2, r1w, 2, 4, rb1, AF.Silu)
              h4 = mlp_layer(h3, r2w, 4, 1, rb2, AF.Silu)
              lps = ps_m.tile([1, 1], dt.float32, tag="mlp")
              nc.tensor.matmul(lps[:], r3w[:], h4[:], start=True, stop=True)
              logits = p_mlp.tile([1, 1], dt.float32)
              nc.scalar.activation(logits[:], lps[:], AF.Identity, bias=rb3[:])
              l1 = p_mlp.tile([1, 1], dt.float32)
              l2 = p_mlp.tile([1, 1], dt.float32)
              nc.scalar.activation(l1[:], noise[:], AF.Ln, bias=epsb[:])
              nc.scalar.activation(l2[:], l1[:], AF.Ln, bias=epsb[:], scale=-1.0)
              zin = p_mlp.tile([1, 1], dt.float32)
              nc.vector.tensor_tensor(zin[:], logits[:], l2[:], ALU.subtract)
              zsoft = p_mlp.tile([1, 1], dt.float32)
              nc.scalar.activation(zsoft[:], zin[:], AF.Sigmoid)
              zhard = p_mlp.tile([1, 1], dt.float32)
              nc.vector.tensor_scalar(zhard[:], zsoft[:], 0.5, None, ALU.is_gt)
              mps = ps_m.tile([128, 1], dt.float32, tag="mlp")
              nc.tensor.matmul(mps[:], ones_r[:], zhard[:], start=True, stop=True)
              mix = p_mlp.tile([128, 1], dt.float32)
              nc.scalar.copy(mix[:], mps[:])
              onem = p_mlp.tile([128, 1], dt.float32)
              nc.vector.tensor_scalar(onem[:], mix[:], -1.0, 1.0, ALU.mult,
                                      ALU.add)
              mlp_ctx.close()

              # --- attention ---
              for h in range(HLOC):
                  for g in range(NCH):
                      pT = p_pT.tile([128, NBLK * CH], dt.float32r, tag="pT")
                      for Ii in range(4):
                          I = 4 * g + Ii
                          nb_tot = (I + 1) * 128
                          e = p_e.tile([128, S], dt.float32, tag="e")
                          pband = p_p.tile([128, S], dt.float32r, tag="p")
                          sums = p_sm.tile([128, 16], dt.float32, tag="sums")
                          grps = _groups(I)
                          ng = len(grps)
                          for gi, (sb, nb) in enumerate(grps):
                              w = nb * 128
                              col = 4 - ng + gi
                              sc = ps_sc.tile([128, 512], dt.float32, tag="sc")
                              nc.tensor.matmul(
                                  sc[:, 0:w], qT[h][:, I * 128:(I + 1) * 128],
                                  kT[:, sb * 128: sb * 128 + w],
                                  start=True, stop=True)
                              if gi == ng - 1:
                                  nc.vector.tensor_tensor(
                                      sc[:, w - 128:w], sc[:, w - 128:w],
                                      diagneg[:], ALU.add)
                              nc.scalar.activation(
                                  e[:, sb * 128: sb * 128 + w], sc[:, 0:w],
                                  AF.Exp, scale=SCALE,
                                  accum_out=sums[:, col:col + 1])
                          if I >= 9:
                              tmask = p_sm.tile([128, 128], dt.float32,
                                                tag="tmask")
                              nc.vector.tensor_reduce(
                                  sums[:, 4:5], e[:, 0:128], AX.X, ALU.add)
                              nc.vector.tensor_tensor(
                                  tmask[:], e[:, (I - 8) * 128:(I - 7) * 128],
                                  triup[:], ALU.mult)
                              nc.vector.tensor_reduce(
                                  sums[:, 5:6], tmask[:], AX.X, ALU.add)
                              nc.vector.tensor_reduce(
                                  sums[:, 6:7], sums[:, 4 - ng:4], AX.X, ALU.add)
                              nc.vector.tensor_reduce(
                                  sums[:, 7:8], sums[:, 2:6], AX.X, ALU.add)
                              nc.vector.reciprocal(sums[:, 8:9], sums[:, 6:7])
                              nc.vector.reciprocal(sums[:, 9:10], sums[:, 7:8])
                              nc.vector.tensor_tensor(
                                  sums[:, 10:11], sums[:, 8:9], onem[:], ALU.mult)
                              nc.vector.tensor_tensor(
                                  sums[:, 11:12], sums[:, 9:10], mix[:], ALU.mult)
                              nc.vector.tensor_tensor(
                                  sums[:, 12:13], sums[:, 10:11], sums[:, 11:12],
                                  ALU.add)
                              a_ap = sums[:, 12:13]
                              b_ap = sums[:, 10:11]
                              amb_ap = sums[:, 11:12]
                              nc.vector.tensor_scalar(
                                  pband[:, 0:128], e[:, 0:128], a_ap, None,
                                  ALU.mult)
                              if I >= 10:
                                  nc.vector.tensor_scalar(
                                      pband[:, 128:(I - 8) * 128],
                                      e[:, 128:(I - 8) * 128], b_ap, None,
                                      ALU.mult)
                              nc.vector.tensor_scalar(
                                  pband[:, (I - 8) * 128:(I - 7) * 128],
                                  e[:, (I - 8) * 128:(I - 7) * 128], b_ap, None,
                                  ALU.mult)
                              nc.vector.scalar_tensor_tensor(
                                  pband[:, (I - 8) * 128:(I - 7) * 128],
                                  tmask[:], amb_ap,
                                  pband[:, (I - 8) * 128:(I - 7) * 128],
                                  ALU.mult, ALU.add)
                              nc.vector.tensor_scalar(
                                  pband[:, (I - 7) * 128:nb_tot],
                                  e[:, (I - 7) * 128:nb_tot], a_ap, None,
                                  ALU.mult)
                          else:
                              nc.vector.tensor_reduce(
                                  sums[:, 6:7], sums[:, 4 - ng:4], AX.X, ALU.add)
                              nc.vector.reciprocal(sums[:, 8:9], sums[:, 6:7])
                              nc.vector.tensor_scalar(
                                  pband[:, 0:nb_tot], e[:, 0:nb_tot],
                                  sums[:, 8:9], None, ALU.mult)
                          pT3 = pT[:, :].rearrange("p (J c) -> p J c", c=CH)
                          for J0 in range(0, I + 1, 4):
                              nb4 = min(4, I + 1 - J0)
                              ptp = ps_pt.tile([128, 512], dt.float32r, tag="pt")
                              for jj in range(nb4):
                                  nc.tensor.transpose(
                                      ptp[:, jj * 128:(jj + 1) * 128],
                                      pband[:, (J0 + jj) * 128:
                                            (J0 + jj + 1) * 128],
                                      ident[:])
                              nc.any.tensor_copy(
                                  pT3[:, J0:J0 + nb4,
                                      Ii * 128:(Ii + 1) * 128],
                                  ptp[:, 0:nb4 * 128].rearrange(
                                      "p (J c) -> p J c", c=128))
                      # PV for this (h, chunk)
                      ops = ps_o.tile([128, CH], dt.float32, tag="o")
                      last_J = 4 * g + 3
                      for J in range(last_J + 1):
                          k = J - 4 * g
                          if k <= 0:
                              nc.tensor.matmul(
                                  ops[:], vN[:, J * 128:(J + 1) * 128],
                                  pT[:, J * CH:(J + 1) * CH],
                                  start=(J == 0), stop=(J == last_J))
                          else:
                              nc.tensor.matmul(
                                  ops[:, k * 128:CH],
                                  vN[:, J * 128:(J + 1) * 128],
                                  pT[:, J * CH + k * 128:(J + 1) * CH],
                                  start=False, stop=(J == last_J))
                      osb = p_ob.tile([128, CH], dt.bfloat16, tag="osb")
                      nc.scalar.copy(osb[:], ops[:])
                      a2a_in_h = a2a_in0 if h < 2 else a2a_in1
                      hh = h % 2
                      nc.sync.dma_start(
                          a2a_in_h[2 * g, hh * 128:(hh + 1) * 128, :],
                          osb[:, 0:ROWS])
                      nc.sync.dma_start(
                          a2a_in_h[2 * g + 1, hh * 128:(hh + 1) * 128, :],
                          osb[:, ROWS:CH])
                  if h == 1:
                      nc.gpsimd.collective_compute(
                          "AllToAll", ALU.bypass,
                          replica_groups=[list(range(NCORES))],
                          ins=[a2a_in0.opt()], outs=[a2a_out0.opt()])
                  if h == 3:
                      nc.gpsimd.collective_compute(
                          "AllToAll", ALU.bypass,
                          replica_groups=[list(range(NCORES))],
                          ins=[a2a_in1.opt()], outs=[a2a_out1.opt()])

        # ---- phase 3: AllToAll + output projection ----------------------
        with ExitStack() as ph3:
          if KNOBS["phases"] >= 3:
              p_oT = ph3.enter_context(tc.tile_pool(name="oT", bufs=1))
              p_part = ph3.enter_context(tc.tile_pool(name="part", bufs=1))
              p_wo = ph3.enter_context(tc.tile_pool(name="wo", bufs=KNOBS["wo_bufs"]))
              p_os = ph3.enter_context(tc.tile_pool(name="outsb", bufs=2))
              ps_w = ph3.enter_context(
                  tc.tile_pool(name="ps_w", bufs=2, space="PSUM"))

              # two 16-ktile halves: half 0 (heads 0-1, from a2a_out0)
              # overlaps AllToAll#2; half 1 adds the partials and stores.
              oTb = p_oT.tile([128, KT * ROWS], dt.bfloat16, name="oTb",
                              tag="oTb")
              parts = [p_part.tile([128, 512], dt.float32, name=f"part{i}",
                                   tag=f"part{i}") for i in range(16)]
              for half in range(2):
                  buf = a2a_out0 if half == 0 else a2a_out1
                  for p in range(NCORES):
                      for tt in range(2):
                          kt = 4 * p + 2 * half + tt
                          nc.sync.dma_start(
                              oTb[:, kt * ROWS:(kt + 1) * ROWS],
                              buf[p, tt * 128:(tt + 1) * 128, :])
                  for ngi in range(8):
                      pso = [ps_w.tile([128, 512], dt.float32, tag=f"wo{st}",
                                      name=f"wo{st}")
                             for st in range(2)]
                      for b2 in range(NCORES):
                          r0 = (4 * b2 + 2 * half) * 128
                          wsl = p_wo.tile([128, 2, 512], dt.bfloat16,
                                          tag="wo")
                          nc.sync.dma_start(
                              wsl[:], wo_d[r0:r0 + 256,
                                           ngi * 512:(ngi + 1) * 512
                                           ].rearrange(
                                  "(k p) n -> p k n", p=128))
                          for kk in range(2):
                              kt = 4 * b2 + 2 * half + kk
                              ki = 2 * b2 + kk
                              for st in range(2):
                                  nc.tensor.matmul(
                                      pso[st][:],
                                      oTb[:, kt * ROWS + st * 128:
                                          kt * ROWS + (st + 1) * 128],
                                      wsl[:, kk, :], start=(ki == 0),
                                      stop=(ki == 15))
                      for st in range(2):
                          if half == 0:
                              nc.scalar.copy(parts[2 * ngi + st][:],
                                             pso[st][:])
                          else:
                              osb = p_os.tile([128, 512], dt.float32,
                                              tag="os")
                              nc.vector.tensor_tensor(
                                  osb[:], pso[st][:],
                                  parts[2 * ngi + st][:], ALU.add)
                              nc.sync.dma_start(
                                  out_d[st * 128:(st + 1) * 128,
                                        ngi * 512:(ngi + 1) * 512], osb[:])

    nc.compile()
    return nc


_CACHE = {}


def _host_constants():
    inv = 10000.0 ** (-np.arange(0, D, 2, dtype=np.float64) / D)
    t = np.arange(S, dtype=np.float64)
    fr = np.outer(t, inv)                      # [S, 64]
    cos = np.cos(fr).T.astype(np.float32)      # [64, S]
    sin = np.sin(fr).T.astype(np.float32)
    cos2 = np.vstack([cos, cos])
    sin2 = np.vstack([-sin, sin])
    ident = np.eye(128, dtype=np.float32)
    a = np.arange(128)
    diagneg = np.where(a[None, :] <= a[:, None], 0.0, NEG).astype(np.float32)
    triup = (a[None, :] > a[:, None]).astype(np.float32)
    ones = np.ones((1, 128), dtype=np.float32)
    return cos2, sin2, ident, diagneg, triup, ones


def kernel(hidden_states, Wq, Wk, Wv, Wo, fe1_w, fe1_b, fe2_w, fe2_b,
           r1_w, r1_b, r2_w, r2_b, r3_w, r3_b, router_noise):
    if "nc" not in _CACHE:
        _CACHE["nc"] = build()
    nc = _CACHE["nc"]

    import ml_dtypes
    bf16 = ml_dtypes.bfloat16
    hsT = np.ascontiguousarray(
        np.asarray(hidden_states, dtype=np.float32).reshape(S, HID).T
    ).astype(bf16)
    Wq = np.asarray(Wq, np.float32)
    Wk = np.asarray(Wk, np.float32)
    Wv = np.asarray(Wv, np.float32)
    Wo = np.ascontiguousarray(np.asarray(Wo, np.float32)).astype(bf16)
    cos2, sin2, ident, diagneg, triup, ones = _host_constants()

    def ktile_cols(w, ktiles, ntiles, nwidth):
        # [K, N] -> [128, ktiles*ntiles*nwidth] with (k, t) slab layout
        return np.ascontiguousarray(
              np.concatenate([w[k * 128:(k + 1) * 128, :] for k in range(ktiles)],
                             axis=1))

    fe1 = np.asarray(fe1_w, np.float32)                       # [128,1024]
    fe2 = ktile_cols(np.asarray(fe2_w, np.float32), 8, 2, 128)
    r1 = ktile_cols(np.asarray(r1_w, np.float32), 2, 4, 128)
    r2 = ktile_cols(np.asarray(r2_w, np.float32), 4, 1, 128)
    r3 = np.asarray(r3_w, np.float32)                         # [128,1]
    b1 = np.asarray(fe1_b, np.float32).reshape(8, 128).T.copy()
    b2 = np.asarray(fe2_b, np.float32).reshape(2, 128).T.copy()
    rb1 = np.asarray(r1_b, np.float32).reshape(4, 128).T.copy()
    rb2 = np.asarray(r2_b, np.float32).reshape(1, 128).T.copy()
    rb3 = np.asarray(r3_b, np.float32).reshape(1, 1)
    noise = np.asarray(router_noise, np.float32).reshape(1, 1)

    in_maps = []
    for c in range(NCORES):
        wqkv = np.ascontiguousarray(np.concatenate(
              [Wq[:, c * 512:(c + 1) * 512],
               Wk[:, c * 128:(c + 1) * 128],
               Wv[:, c * 128:(c + 1) * 128]], axis=1)).astype(bf16)
        in_maps.append(dict(
              hsT=hsT, wqkv=wqkv, wo=Wo, cos2=cos2, sin2=sin2, ident=ident,
              diagneg=diagneg, triup=triup, ones=ones, fe1=fe1, fe2=fe2,
              r1=r1, r2=r2, r3=r3, b1=b1, b2=b2, rb1=rb1, rb2=rb2, rb3=rb3,
              noise=noise, eps=np.full((1, 1), 1e-8, np.float32)))

    res = run_bass_kernel_spmd(nc, in_maps, list(range(NCORES)))
    out = np.concatenate([res.results[c]["out_rows"] for c in range(NCORES)],
                           axis=0)
    return out.reshape(1, S, HID).astype(np.float32)

